# revision 1
# baseline (speedup 1.0000x reference)
"""Trainium2 Bass kernel for nn_MixGNN (TransformerConv + 3x SAGEConv + BN + gated residual).

Strategy (8 NeuronCores, dst-node sharding):
  - Pad N 10000 -> 10240; core r owns 1280 dst nodes = 10 tiles of 128.
  - Host preprocessing (graph structure only): sort edges by dst, bucket per
    dst-tile, pad each tile's edge list to S*128 slots, build wrapped int16
    gather indices, per-chunk local-dst columns, 1/deg, packed weights and
    broadcast bias/affine vectors.
  - Device per layer: dense matmuls on PE; per-edge work via dma_gather of
    source-node rows + indicator matmuls (Ind[e,n] = (dst_e==n) built by DVE
    is_equal against an iota tile); attention scores as KgT.T @ qT on PE from
    a transposed bf16 gather; softmax without max-subtraction (logits are
    O(1)); normalization by the PSUM-accumulated exp-sum / in-degree.
  - Halo exchange: AllGather of each core's h shard (bf16) into a full table
    in shared DRAM before every aggregation.
Output: fp32 [10000, 256].
"""
import os
import sys
import time

import numpy as np

for _p in ("/opt/trn_rl_repo",):
    if _p not in sys.path:
        sys.path.insert(0, _p)

import ml_dtypes  # noqa: E402
import concourse.bacc as bacc  # noqa: E402
import concourse.mybir as mybir  # noqa: E402
import concourse.tile as tile  # noqa: E402
from concourse.bass_utils import run_bass_kernel_spmd  # noqa: E402

P = 128
D = 256
DJ = D // P           # 2 d-chunks of 128
NC = 8                # cores
L = 3                 # SAGE layers
BN_EPS = 1e-5

# dtype knobs for gathered tables (accuracy vs bandwidth)
V_BF16 = True         # v table + attention agg in bf16
H_BF16 = True         # h tables + SAGE agg in bf16

F32 = mybir.dt.float32
BF16 = mybir.dt.bfloat16
I16 = mybir.dt.int16
V_DT = BF16 if V_BF16 else F32
H_DT = BF16 if H_BF16 else F32

_nc_cache = {}


def _wrap_idx(a):
    """[S*128] int array -> [128, S*8] int16 wrapped gather-index layout."""
    w16 = a.reshape(-1, 16).T.astype(np.int16)   # [16, S*8]
    return np.tile(w16, (8, 1))                  # replicate to 8 Q7 stripes


def build_nc(n_pad, sh, nt, S, scale, oma):
    stages = int(os.environ.get("KSTAGES", "5"))
    nocc = os.environ.get("KNOCC") == "1"
    ksm = int(os.environ.get("KSM", "6"))
    kgp = int(os.environ.get("KGP", "2"))
    kpsc = int(os.environ.get("KPSC", "3"))
    kptr = int(os.environ.get("KPTR", "1"))
    kpagg = int(os.environ.get("KPAGG", "2"))
    kpmm = int(os.environ.get("KPMM", "2"))
    khalf = int(os.environ.get("KHALF", "6"))  # gather splits per tile
    kabl = os.environ.get("KABL", "")
    key = (n_pad, sh, nt, S, round(scale, 9), round(oma, 9), V_BF16, H_BF16, stages,
           nocc, ksm, kgp, kpsc, kptr, kpagg, kpmm, khalf, kabl,
           os.environ.get("KHALFT"),
           os.environ.get("KKGT"), os.environ.get("KVG"))
    if key in _nc_cache:
        return _nc_cache[key]

    ET = S * P  # padded edges per tile
    ndev = 1 if nocc else NC
    nc = bacc.Bacc("TRN2", target_bir_lowering=False, debug=False, num_devices=ndev)

    xt_in = nc.dram_tensor("xt_in", [P, DJ * sh], F32, kind="ExternalInput")
    wpack_in = nc.dram_tensor("wpack_in", [P, 10 * DJ * D], F32, kind="ExternalInput")
    vpack_in = nc.dram_tensor("vpack_in", [P, 9 * D + DJ], F32, kind="ExternalInput")
    idx_in = nc.dram_tensor("idx_in", [P, nt * S * 8], I16, kind="ExternalInput")
    dst_in = nc.dram_tensor("dst_in", [P, nt * S], F32, kind="ExternalInput")
    invdeg_in = nc.dram_tensor("invdeg_in", [P, nt], F32, kind="ExternalInput")
    out_dram = nc.dram_tensor("out", [sh, D], F32, kind="ExternalOutput")

    WQ, WK, WV, WS = 0, 1, 2, 3
    WL = [4, 6, 8]
    WR = [5, 7, 9]
    VBK, VBV, VBS = 0, 1, 2

    with tile.TileContext(nc) as tc:
        with (
            tc.tile_pool(name="cst", bufs=1) as cst,
            tc.tile_pool(name="sb", bufs=1) as sb,
            tc.tile_pool(name="g", bufs=kgp) as gp,
            tc.tile_pool(name="sm", bufs=ksm) as smp,
            tc.tile_pool(name="ps", bufs=2, space="PSUM") as ps,
            tc.tile_pool(name="dr", bufs=1, space="DRAM") as dr,
        ):
            # ---------------- constants / inputs to SBUF ----------------
            wp = cst.tile([P, 10 * DJ * D], F32)
            nc.sync.dma_start(out=wp[:], in_=wpack_in[:])
            vp = cst.tile([P, 9 * D + DJ], F32)
            nc.sync.dma_start(out=vp[:], in_=vpack_in[:])
            xt = cst.tile([P, DJ * sh], F32)
            for _xi in range(4):
                _c0 = _xi * (DJ * sh // 4)
                _c1 = (_xi + 1) * (DJ * sh // 4)
                nc.sync.dma_start(out=xt[:, _c0:_c1], in_=xt_in[:, _c0:_c1])
            dstc = cst.tile([P, nt * S], F32)
            nc.sync.dma_start(out=dstc[:], in_=dst_in[:])
            invd = cst.tile([P, nt], F32)
            nc.sync.dma_start(out=invd[:], in_=invdeg_in[:])
            idx_sb = cst.tile([P, nt * S * 8], I16)
            nc.sync.dma_start(out=idx_sb[:], in_=idx_in[:])

            iota_i = cst.tile([P, P], mybir.dt.int32)
            nc.gpsimd.iota(iota_i[:], pattern=[[1, P]], base=0, channel_multiplier=0)
            ones_v = cst.tile([P, 1], V_DT)
            nc.vector.memset(ones_v[:], 1.0)
            # identity for PE transposes: (iota_row == partition_idx)
            iota_part = cst.tile([P, 1], mybir.dt.int32)
            nc.gpsimd.iota(iota_part[:], pattern=[[1, 1]], base=0, channel_multiplier=1)
            iota_part_f = cst.tile([P, 1], F32)
            nc.vector.tensor_copy(out=iota_part_f[:], in_=iota_part[:])
            iota_f = cst.tile([P, P], F32)
            nc.vector.tensor_copy(out=iota_f[:], in_=iota_i[:])
            ident_b = cst.tile([P, P], H_DT)
            nc.vector.memset(ident_b[:], 0.0)
            ident = cst.tile([P, P], F32)
            nc.vector.tensor_scalar(
                out=ident[:], in0=iota_f[:], scalar1=iota_part_f[:, :1], scalar2=None,
                op0=mybir.AluOpType.is_equal,
            )

            def wslice(w, j):
                return wp[:, (w * DJ + j) * D:(w * DJ + j + 1) * D]

            def vslice(k):
                return vp[:, k * D:(k + 1) * D]

            def xtile(j, t):
                return xt[:, j * sh + t * P: j * sh + (t + 1) * P]

            # ---------------- DRAM tables ----------------
            k_ag_in = dr.tile([sh, D], BF16)
            v_ag_in = dr.tile([sh, D], V_DT)
            k_full = dr.tile([n_pad, D], BF16, addr_space="Shared")
            v_full = dr.tile([n_pad, D], V_DT, addr_space="Shared")
            hag_in = [dr.tile([sh, D], H_DT, name=f"hag_in_{i}") for i in range(L)]
            h_full = [dr.tile([n_pad, D], H_DT, name=f"h_full_{i}", addr_space="Shared")
                      for i in range(L)]

            def allgather(in_t, out_t):
                if nocc:
                    nc.sync.dma_start(out=out_t[:sh], in_=in_t[:])
                else:
                    nc.gpsimd.collective_compute(
                        "AllGather", mybir.AluOpType.bypass,
                        replica_groups=[list(range(NC))],
                        ins=[in_t[:]], outs=[out_t[:]],
                    )

            # ---------------- stage 0: k,v shard tables + AG, then qT ----------------
            for t in range(nt):
                pk = ps.tile([P, D], F32, name="pk", tag="pmm", bufs=kpmm)
                for ji in range(DJ):
                    nc.tensor.matmul(pk[:], lhsT=xtile(ji, t), rhs=wslice(WK, ji),
                                     start=(ji == 0), stop=(ji == DJ - 1))
                k_sb = smp.tile([P, D], BF16, name="k_sb")
                nc.vector.tensor_tensor(out=k_sb[:], in0=pk[:], in1=vslice(VBK),
                                        op=mybir.AluOpType.add)
                nc.sync.dma_start(out=k_ag_in[t * P:(t + 1) * P, :], in_=k_sb[:])

                pv = ps.tile([P, D], F32, name="pv", tag="pmm", bufs=kpmm)
                for ji in range(DJ):
                    nc.tensor.matmul(pv[:], lhsT=xtile(ji, t), rhs=wslice(WV, ji),
                                     start=(ji == 0), stop=(ji == DJ - 1))
                v_sb = smp.tile([P, D], V_DT, name="v_sb")
                nc.vector.tensor_tensor(out=v_sb[:], in0=pv[:], in1=vslice(VBV),
                                        op=mybir.AluOpType.add)
                nc.sync.dma_start(out=v_ag_in[t * P:(t + 1) * P, :], in_=v_sb[:])

            allgather(k_ag_in, k_full)
            allgather(v_ag_in, v_full)

            qT = []
            for j in range(DJ):
                qTj = sb.tile([P, sh], BF16, name=f"qT_{j}")
                n0 = 0
                while n0 < sh:
                    nn = min(512, sh - n0)
                    pq = ps.tile([P, 512], F32, name="pq", tag="pmm", bufs=kpmm)
                    for ji in range(DJ):
                        nc.tensor.matmul(
                            pq[:, :nn],
                            lhsT=wslice(WQ, ji)[:, j * P:(j + 1) * P],
                            rhs=xt[:, ji * sh + n0: ji * sh + n0 + nn],
                            start=(ji == 0), stop=(ji == DJ - 1),
                        )
                    nc.vector.tensor_scalar(
                        out=qTj[:, n0:n0 + nn], in0=pq[:, :nn],
                        scalar1=vp[:, 9 * D + j: 9 * D + j + 1], scalar2=None,
                        op0=mybir.AluOpType.add,
                    )
                    n0 += nn
                qT.append(qTj)

            # shard-resident activations
            h_cur = sb.tile([P, nt * D], F32)
            h_nxt = sb.tile([P, nt * D], F32)
            hT_cur = sb.tile([P, DJ * sh], F32)
            hT_nxt = sb.tile([P, DJ * sh], F32)

            def agg_pass(layer, h_prev, hT_prev, h_out, hT_out):
                """layer -1: transformer (h_prev/hT_prev unused); 0..L-1: SAGE."""
                li = layer + 1  # h table index this pass WRITES (0 for transformer)
                kh = khalf if layer >= 0 else int(os.environ.get("KHALFT", "1"))
                splits = []  # (c0, c1) chunk ranges per gather piece
                base = (S + kh - 1) // kh
                c0 = 0
                while c0 < S:
                    splits.append((c0, min(S, c0 + base)))
                    c0 += base
                for t in range(nt):
                    if layer < 0:
                        kgt = gp.tile([P, DJ, ET], BF16, name="kgt", tag="kgt",
                                      bufs=int(os.environ.get("KKGT", "2")))
                        vg = gp.tile([P, S, D], V_DT, name="vg", tag="vg",
                                     bufs=int(os.environ.get("KVG", "2")))
                    else:
                        kgt = None
                        vg = gp.tile([P, S, D], H_DT, name="hg", tag="vg",
                                     bufs=int(os.environ.get("KVG", "2")))
                    if layer < 0:
                        idx_tt = idx_sb[:, t * S * 8:(t + 1) * S * 8]
                        nc.gpsimd.dma_gather(
                            out_ap=kgt[:], in_ap=k_full[:], idxs_ap=idx_tt,
                            num_idxs=ET, num_idxs_reg=ET, elem_size=D,
                            transpose=True, single_packet=False)
                    src_tab = v_full if layer < 0 else h_full[layer]
                    for (ca, cb) in splits:
                        nn_i = (cb - ca) * P
                        idx_t = idx_sb[:, t * S * 8 + ca * 8: t * S * 8 + cb * 8]
                        nc.gpsimd.dma_gather(
                            out_ap=vg[:, ca:cb, :], in_ap=src_tab[:], idxs_ap=idx_t,
                            num_idxs=nn_i, num_idxs_reg=nn_i, elem_size=D,
                            single_packet=False)

                    pagg = ps.tile([P, D + 1], F32, name="pagg", tag="pagg", bufs=kpagg)
                    for c in range(S):
                        dcol = dstc[:, t * S + c: t * S + c + 1]
                        if layer < 0:
                            psc = ps.tile([P, P], F32, name="psc", tag="psc", bufs=kpsc)
                            nsc = 1 if kabl == "sc1" else DJ
                            for j in range(nsc):
                                nc.tensor.matmul(
                                    psc[:],
                                    lhsT=kgt[:, j, c * P:(c + 1) * P],
                                    rhs=qT[j][:, t * P:(t + 1) * P],
                                    start=(j == 0), stop=(j == nsc - 1))
                            exps = smp.tile([P, P], F32, name="exps")
                            nc.scalar.activation(exps[:], psc[:],
                                                 mybir.ActivationFunctionType.Exp,
                                                 scale=scale)
                            w_b = smp.tile([P, P], V_DT, name="w_b", tag="w_b")
                            nc.vector.scalar_tensor_tensor(
                                out=w_b[:], in0=iota_f[:], scalar=dcol, in1=exps[:],
                                op0=mybir.AluOpType.is_equal,
                                op1=mybir.AluOpType.mult)
                            nc.tensor.matmul(pagg[:, :D], lhsT=w_b[:], rhs=vg[:, c, :],
                                             start=(c == 0), stop=(c == S - 1))
                            if kabl != "ones":
                                nc.tensor.matmul(pagg[:, D:D + 1], lhsT=w_b[:],
                                                 rhs=ones_v[:],
                                                 start=False, stop=(c == S - 1))
                        else:
                            if kabl == "noind":
                                ind_b = ident_b
                            else:
                                ind_b = smp.tile([P, P], H_DT, name="ind_b", tag="w_b")
                                nc.vector.tensor_scalar(
                                    out=ind_b[:], in0=iota_f[:], scalar1=dcol,
                                    scalar2=None, op0=mybir.AluOpType.is_equal)
                            nc.tensor.matmul(pagg[:, :D], lhsT=ind_b[:],
                                             rhs=vg[:, c, :],
                                             start=(c == 0), stop=(c == S - 1))

                    # ---- tile epilogue -> h_out tile [node, d] ----
                    if layer < 0:
                        smax = smp.tile([P, 1], F32, name="smax")
                        nc.vector.tensor_scalar(
                            out=smax[:], in0=pagg[:, D:D + 1], scalar1=1e-30,
                            scalar2=None, op0=mybir.AluOpType.max)
                        rs = smp.tile([P, 1], F32, name="rs")
                        nc.vector.reciprocal(rs[:], smax[:])
                        pskip = ps.tile([P, D], F32, name="pskip", tag="pmm", bufs=kpmm)
                        for ji in range(DJ):
                            nc.tensor.matmul(pskip[:], lhsT=xtile(ji, t),
                                             rhs=wslice(WS, ji),
                                             start=(ji == 0), stop=(ji == DJ - 1))
                        t1 = smp.tile([P, D], F32, name="t1", tag="t1")
                        nc.scalar.activation(t1[:], pagg[:, :D],
                                             mybir.ActivationFunctionType.Copy,
                                             scale=rs[:, :1])
                        t2 = smp.tile([P, D], F32, name="t2", tag="t2")
                        nc.vector.tensor_tensor(out=t2[:], in0=t1[:], in1=pskip[:],
                                                op=mybir.AluOpType.add)
                        t3 = smp.tile([P, D], F32, name="t3", tag="t3")
                        nc.vector.tensor_tensor(out=t3[:], in0=t2[:], in1=vslice(VBS),
                                                op=mybir.AluOpType.add)
                        nc.scalar.activation(h_out[:, t * D:(t + 1) * D], t3[:],
                                             mybir.ActivationFunctionType.Relu)
                    else:
                        mean_sb = smp.tile([P, D], F32, name="mean_sb", tag="t1")
                        nc.scalar.activation(mean_sb[:], pagg[:, :D],
                                             mybir.ActivationFunctionType.Copy,
                                             scale=invd[:, t:t + 1])
                        pz = ps.tile([P, D], F32, name="pz", tag="pmm", bufs=kpmm)
                        for j in range(DJ):
                            ptr = ps.tile([P, P], F32, name="ptr", tag="ptr", bufs=kptr)
                            nc.tensor.transpose(out=ptr[:],
                                                in_=mean_sb[:, j * P:(j + 1) * P],
                                                identity=ident[:])
                            mT = smp.tile([P, P], F32, name="mT", tag="mT")
                            nc.scalar.copy(out=mT[:], in_=ptr[:])
                            nc.tensor.matmul(pz[:], lhsT=mT[:],
                                             rhs=wslice(WL[layer], j),
                                             start=(j == 0), stop=False)
                        for j in range(DJ):
                            nc.tensor.matmul(
                                pz[:],
                                lhsT=hT_prev[:, j * sh + t * P: j * sh + (t + 1) * P],
                                rhs=wslice(WR[layer], j),
                                start=False, stop=(j == DJ - 1))
                        gx = vslice(3 + 2 * layer)
                        bx = vslice(4 + 2 * layer)
                        t1 = smp.tile([P, D], F32, name="t1s", tag="t2")
                        nc.vector.tensor_tensor(out=t1[:], in0=pz[:], in1=gx,
                                                op=mybir.AluOpType.mult)
                        t2 = smp.tile([P, D], F32, name="t2s", tag="t3")
                        nc.vector.tensor_tensor(out=t2[:], in0=t1[:], in1=bx,
                                                op=mybir.AluOpType.add)
                        t3 = smp.tile([P, D], F32, name="t3s", tag="t4")
                        nc.vector.scalar_tensor_tensor(
                            out=t3[:], in0=h_prev[:, t * D:(t + 1) * D], scalar=oma,
                            in1=t2[:], op0=mybir.AluOpType.mult,
                            op1=mybir.AluOpType.add)
                        nc.scalar.activation(h_out[:, t * D:(t + 1) * D], t3[:],
                                             mybir.ActivationFunctionType.Relu)

                    if layer < L - 1:
                        hstage = smp.tile([P, D], H_DT, name="hstage")
                        nc.scalar.copy(out=hstage[:],
                                       in_=h_out[:, t * D:(t + 1) * D])
                        nc.sync.dma_start(out=hag_in[li][t * P:(t + 1) * P, :],
                                          in_=hstage[:])
                        for j in range(DJ):
                            ptr2 = ps.tile([P, P], F32, name="ptr2", tag="ptr", bufs=kptr)
                            nc.tensor.transpose(
                                out=ptr2[:],
                                in_=h_out[:, t * D + j * P: t * D + (j + 1) * P],
                                identity=ident[:])
                            nc.scalar.copy(
                                out=hT_out[:, j * sh + t * P: j * sh + (t + 1) * P],
                                in_=ptr2[:])
                    else:
                        nc.sync.dma_start(out=out_dram[t * P:(t + 1) * P, :],
                                          in_=h_out[:, t * D:(t + 1) * D])

                if layer < L - 1:
                    allgather(hag_in[li], h_full[li])

            if stages <= 1:
                # dump k_full slice so the program has an output
                tmpo = smp.tile([P, D], F32, name="tmpo")
                for t in range(nt):
                    nc.vector.tensor_copy(out=tmpo[:], in_=xt[:, :D])
                    nc.sync.dma_start(out=out_dram[t * P:(t + 1) * P, :], in_=tmpo[:])
            else:
                agg_pass(-1, None, None, h_cur, hT_cur)
                bufs = [(h_cur, hT_cur), (h_nxt, hT_nxt)]
                for i in range(min(L, stages - 2)):
                    h_prev, hT_prev = bufs[i % 2]
                    h_out, hT_out = bufs[(i + 1) % 2]
                    agg_pass(i, h_prev, hT_prev, h_out, hT_out)
                if stages - 2 < L:
                    hsrc, _ = bufs[max(0, stages - 2) % 2]
                    for t in range(nt):
                        nc.sync.dma_start(out=out_dram[t * P:(t + 1) * P, :],
                                          in_=hsrc[:, t * D:(t + 1) * D])

    nc.compile()
    _nc_cache[key] = nc
    return nc


def _host_prep(x, src, dst, Wq, bq, Wk, bk, Wv, bv, Ws, bs, Wl, bl, Wr,
               gamma, beta, alpha_res):
    n, d = x.shape
    n_pad = ((n + NC * P - 1) // (NC * P)) * (NC * P)
    sh = n_pad // NC
    nt = sh // P
    n_tiles = n_pad // P

    order = np.argsort(dst, kind="stable")
    src_s, dst_s = src[order], dst[order]
    tile_of = dst_s // P
    counts = np.bincount(tile_of, minlength=n_tiles)
    starts = np.concatenate([[0], np.cumsum(counts)])
    S = int(max(1, (counts.max() + P - 1) // P))
    ET = S * P

    deg = np.bincount(dst, minlength=n_pad).astype(np.float32)
    invdeg_full = 1.0 / np.maximum(deg, 1.0)

    al = 1.0 / (1.0 + np.exp(-alpha_res))
    oma = float(1.0 - al)
    bn_scale = 1.0 / np.sqrt(1.0 + BN_EPS)
    scale = 1.0 / np.sqrt(float(d))

    x_pad = np.zeros((n_pad, D), np.float32)
    x_pad[:n] = x
    xT = x_pad.T.copy()

    weights = [Wq, Wk, Wv, Ws, Wl[0], Wr[0], Wl[1], Wr[1], Wl[2], Wr[2]]
    wpack = np.empty((P, 10 * DJ * D), np.float32)
    for w, W in enumerate(weights):
        for j in range(DJ):
            wpack[:, (w * DJ + j) * D:(w * DJ + j + 1) * D] = W[j * P:(j + 1) * P, :]

    Gx = [al * bn_scale * gamma[i] for i in range(L)]
    Bx = [al * (bl[i] * bn_scale * gamma[i] + beta[i]) for i in range(L)]
    vecs = [bk, bv, bs, Gx[0], Bx[0], Gx[1], Bx[1], Gx[2], Bx[2]]
    vpack = np.empty((P, 9 * D + DJ), np.float32)
    for k, v in enumerate(vecs):
        vpack[:, k * D:(k + 1) * D] = np.tile(v[None, :], (P, 1))
    for j in range(DJ):
        vpack[:, 9 * D + j] = bq[j * P:(j + 1) * P]

    in_maps = []
    for r in range(NC):
        idx_arr = np.zeros((P, nt * S * 8), np.int16)
        dst_arr = np.full((P, nt * S), 128.0, np.float32)
        for tloc in range(nt):
            g = r * nt + tloc
            e0, e1 = starts[g], starts[g + 1]
            cnt = e1 - e0
            srcs = np.zeros(ET, np.int64)
            srcs[:cnt] = src_s[e0:e1]
            dl = np.full(ET, 128, np.int64)
            dl[:cnt] = dst_s[e0:e1] - g * P
            idx_arr[:, tloc * S * 8:(tloc + 1) * S * 8] = _wrap_idx(srcs)
            dst_arr[:, tloc * S:(tloc + 1) * S] = dl.reshape(S, P).T
        invdeg_r = invdeg_full[r * sh:(r + 1) * sh].reshape(nt, P).T.copy()

        xt_r = np.empty((P, DJ * sh), np.float32)
        for j in range(DJ):
            xt_r[:, j * sh:(j + 1) * sh] = xT[j * P:(j + 1) * P, r * sh:(r + 1) * sh]

        in_maps.append({
            "xt_in": xt_r,
            "wpack_in": wpack,
            "vpack_in": vpack,
            "idx_in": idx_arr,
            "dst_in": dst_arr,
            "invdeg_in": np.ascontiguousarray(invdeg_r),
        })
    return in_maps, (n_pad, sh, nt, S, scale, oma)


def kernel(**inputs):
    x = np.asarray(inputs["x"], np.float32)
    edge_index = np.asarray(inputs["edge_index"])
    args = dict(
        Wq=np.asarray(inputs["Wq"], np.float32), bq=np.asarray(inputs["bq"], np.float32),
        Wk=np.asarray(inputs["Wk"], np.float32), bk=np.asarray(inputs["bk"], np.float32),
        Wv=np.asarray(inputs["Wv"], np.float32), bv=np.asarray(inputs["bv"], np.float32),
        Ws=np.asarray(inputs["Ws"], np.float32), bs=np.asarray(inputs["bs"], np.float32),
        Wl=np.asarray(inputs["Wl"], np.float32), bl=np.asarray(inputs["bl"], np.float32),
        Wr=np.asarray(inputs["Wr"], np.float32),
        gamma=np.asarray(inputs["gamma"], np.float32),
        beta=np.asarray(inputs["beta"], np.float32),
        alpha_res=float(np.asarray(inputs["alpha_res"])),
    )
    src = edge_index[0].astype(np.int64)
    dst = edge_index[1].astype(np.int64)

    in_maps, (n_pad, sh, nt, S, scale, oma) = _host_prep(x, src, dst, **args)
    t0 = time.time()
    nc = build_nc(n_pad, sh, nt, S, scale, oma)
    print(f"[kernel] build+compile {time.time()-t0:.1f}s", flush=True)
    t0 = time.time()
    res = run_bass_kernel_spmd(nc, in_maps, core_ids=list(range(NC)))
    print(f"[kernel] run {time.time()-t0:.1f}s", flush=True)
    out = np.concatenate([res.results[r]["out"] for r in range(NC)], axis=0)
    return out[:x.shape[0]]



# revision 25
# speedup vs baseline: 1.1460x; 1.1460x over previous
"""Trainium2 Bass kernel for nn_MixGNN (TransformerConv + 3x SAGEConv + BN + gated residual).

Strategy (8 NeuronCores, dst-node sharding):
  - Pad N 10000 -> 10240; core r owns 1280 dst nodes = 10 tiles of 128.
  - TransformerConv: per-edge gather machinery — dma_gather of source k rows
    (transposed, bf16) + v rows; attention scores as KgT.T @ qT on PE;
    softmax without max-subtraction; exp-weighted indicator matmuls on PE.
  - SAGEConv x3: dense fp8 DoubleRow aggregation. Host precomputes the
    mean-normalized adjacency A[src, dst] = mult/deg (fp8e4) in PE DoubleRow
    block layout; each pass computes meanT[d, dst] = sum_src H[src, d] *
    A[src, dst] as 256-src-deep fp8 matmuls at 0.5 cyc/row. The full H table
    (fp8, tile-major) lives in SBUF, refreshed per pass via AllGather.
  - Halo exchange: AllGather of bf16 k/v tables (transformer) and fp8 packed
    h tables (SAGE) through shared DRAM.
Output: fp32 [10000, 256].
"""
import os
import sys
import time

import numpy as np

for _p in ("/opt/trn_rl_repo",):
    if _p not in sys.path:
        sys.path.insert(0, _p)

import ml_dtypes  # noqa: E402
import concourse.bacc as bacc  # noqa: E402
import concourse.mybir as mybir  # noqa: E402
import concourse.tile as tile  # noqa: E402
from concourse.bass_utils import run_bass_kernel_spmd  # noqa: E402

P = 128
D = 256
DJ = D // P           # 2 d-chunks of 128
NC = 8                # cores
L = 3                 # SAGE layers
BN_EPS = 1e-5

F32 = mybir.dt.float32
BF16 = mybir.dt.bfloat16
FP8 = mybir.dt.float8e4
I16 = mybir.dt.int16
NP_FP8 = ml_dtypes.float8_e4m3
NP_BF16 = ml_dtypes.bfloat16

_nc_cache = {}


def _wrap_idx(a):
    """[S*128] int array -> [128, S*8] int16 wrapped gather-index layout."""
    w16 = a.reshape(-1, 16).T.astype(np.int16)   # [16, S*8]
    return np.tile(w16, (8, 1))                  # replicate to 8 Q7 stripes


def build_nc(n_pad, sh, nt, s_list, scale, oma):
    stages = int(os.environ.get("KSTAGES", "5"))
    nocc = os.environ.get("KNOCC") == "1"
    ksm = int(os.environ.get("KSM", "4"))
    kgp = int(os.environ.get("KGP", "2"))
    kpsc = int(os.environ.get("KPSC", "2"))
    kpagg = int(os.environ.get("KPAGG", "2"))
    kpmm = int(os.environ.get("KPMM", "2"))
    kptr = int(os.environ.get("KPTR", "1"))
    khalft = int(os.environ.get("KHALFT", "2"))  # transformer v-gather splits
    kapc = int(os.environ.get("KAPC", "8"))      # A-table load pieces
    key = (n_pad, sh, nt, s_list, round(scale, 9), round(oma, 9), stages,
           nocc, ksm, kgp, kpsc, kpagg, kpmm, kptr, khalft, kapc,
           os.environ.get("KKGT"), os.environ.get("KVG"))
    if key in _nc_cache:
        return _nc_cache[key]

    S = max(s_list)
    soff = [0]
    for st_ in s_list:
        soff.append(soff[-1] + st_)
    stot = soff[-1]

    NCH = n_pad // P          # 80 source chunks of 128
    NC2 = NCH // 2            # 40 double-chunks of 256
    NBLK = NC2 * nt           # 400 A-blocks per core

    ndev = 1 if nocc else NC
    nc = bacc.Bacc("TRN2", target_bir_lowering=False, debug=False, num_devices=ndev)

    xt_in = nc.dram_tensor("xt_in", [P, DJ * sh], BF16, kind="ExternalInput")
    wpack_in = nc.dram_tensor("wpack_in", [P, 10 * DJ * D], BF16, kind="ExternalInput")
    vpack_in = nc.dram_tensor("vpack_in", [P, 9 * D + DJ], F32, kind="ExternalInput")
    idx_in = nc.dram_tensor("idx_in", [P, stot * 8], I16, kind="ExternalInput")
    dst_in = nc.dram_tensor("dst_in", [P, stot], F32, kind="ExternalInput")
    invdeg_in = nc.dram_tensor("invdeg_in", [P, nt], F32, kind="ExternalInput")
    a_in = nc.dram_tensor("a_in", [P, NBLK * 2 * P], FP8, kind="ExternalInput")
    out_dram = nc.dram_tensor("out", [sh, D], F32, kind="ExternalOutput")

    WQ, WK, WV, WS = 0, 1, 2, 3
    WL = [4, 6, 8]
    WR = [5, 7, 9]
    VBK, VBV, VBS = 0, 1, 2

    with tile.TileContext(nc) as tc:
        with (
            tc.tile_pool(name="cst", bufs=1) as cst,
            tc.tile_pool(name="sb", bufs=1) as sb,
            tc.tile_pool(name="sm", bufs=ksm) as smp,
            tc.tile_pool(name="ps", bufs=2, space="PSUM") as ps,
            tc.tile_pool(name="dr", bufs=1, space="DRAM") as dr,
        ):
            # ---------------- constants / inputs to SBUF ----------------
            wp = cst.tile([P, 10 * DJ * D], BF16)
            nc.sync.dma_start(out=wp[:], in_=wpack_in[:])
            vp = cst.tile([P, 9 * D + DJ], F32)
            nc.sync.dma_start(out=vp[:], in_=vpack_in[:])
            xt = cst.tile([P, DJ * sh], BF16)
            for _xi in range(2):
                _c0 = _xi * (DJ * sh // 2)
                _c1 = (_xi + 1) * (DJ * sh // 2)
                nc.sync.dma_start(out=xt[:, _c0:_c1], in_=xt_in[:, _c0:_c1])
            invd = cst.tile([P, nt], F32)
            nc.sync.dma_start(out=invd[:], in_=invdeg_in[:])

            iota_i = cst.tile([P, P], mybir.dt.int32)
            nc.gpsimd.iota(iota_i[:], pattern=[[1, P]], base=0, channel_multiplier=0)
            ones_v = cst.tile([P, 1], BF16)
            nc.vector.memset(ones_v[:], 1.0)
            iota_part = cst.tile([P, 1], mybir.dt.int32)
            nc.gpsimd.iota(iota_part[:], pattern=[[1, 1]], base=0, channel_multiplier=1)
            iota_part_f = cst.tile([P, 1], F32)
            nc.vector.tensor_copy(out=iota_part_f[:], in_=iota_part[:])
            iota_f = cst.tile([P, P], F32)
            nc.vector.tensor_copy(out=iota_f[:], in_=iota_i[:])
            ident = cst.tile([P, P], F32)
            nc.vector.tensor_scalar(
                out=ident[:], in0=iota_f[:], scalar1=iota_part_f[:, :1], scalar2=None,
                op0=mybir.AluOpType.is_equal,
            )
            ident_b = cst.tile([P, P], BF16)
            nc.vector.tensor_copy(out=ident_b[:], in_=ident[:])

            def wslice(w, j):
                return wp[:, (w * DJ + j) * D:(w * DJ + j + 1) * D]

            def vslice(k):
                return vp[:, k * D:(k + 1) * D]

            def xtile(j, t):
                return xt[:, j * sh + t * P: j * sh + (t + 1) * P]

            # ---------------- long-lived SBUF state ----------------
            qT = [sb.tile([P, sh], BF16, name=f"qT_{j}") for j in range(DJ)]
            h_cur = sb.tile([P, nt * D], BF16)
            h_nxt = sb.tile([P, nt * D], BF16)
            hT_cur = sb.tile([P, DJ * sh], BF16)
            hT_nxt = sb.tile([P, DJ * sh], BF16)
            h_sb = sb.tile([P, NCH, D], FP8)     # full H table, tile-major
            meanT_all = sb.tile([P, nt, D], BF16)  # paggT staging per pass

            # ---------------- DRAM tables ----------------
            k_ag_in = dr.tile([sh, D], BF16)
            v_ag_in = dr.tile([sh, D], BF16)
            k_full = dr.tile([n_pad, D], BF16, addr_space="Shared")
            v_full = dr.tile([n_pad, D], BF16, addr_space="Shared")
            hag_in = [dr.tile([P, nt * D], FP8, name=f"hag_in_{i}") for i in range(L)]
            h_full = [dr.tile([NC * P, nt * D], FP8, name=f"h_full_{i}",
                              addr_space="Shared") for i in range(L)]

            def allgather(in_t, out_t):
                if nocc:
                    nc.sync.dma_start(out=out_t[:in_t.shape[0]], in_=in_t[:])
                else:
                    nc.gpsimd.collective_compute(
                        "AllGather", mybir.AluOpType.bypass,
                        replica_groups=[list(range(NC))],
                        ins=[in_t[:]], outs=[out_t[:]],
                    )

            # ---------------- stage 0: k,v shard tables + AG, then qT --------
            for t in range(nt):
                pk = ps.tile([P, D], F32, name="pk", tag="pmm", bufs=kpmm)
                for ji in range(DJ):
                    nc.tensor.matmul(pk[:], lhsT=xtile(ji, t), rhs=wslice(WK, ji),
                                     start=(ji == 0), stop=(ji == DJ - 1))
                k_sb = smp.tile([P, D], BF16, name="k_sb")
                nc.vector.tensor_tensor(out=k_sb[:], in0=pk[:], in1=vslice(VBK),
                                        op=mybir.AluOpType.add)
                nc.sync.dma_start(out=k_ag_in[t * P:(t + 1) * P, :], in_=k_sb[:])

                pv = ps.tile([P, D], F32, name="pv", tag="pmm", bufs=kpmm)
                for ji in range(DJ):
                    nc.tensor.matmul(pv[:], lhsT=xtile(ji, t), rhs=wslice(WV, ji),
                                     start=(ji == 0), stop=(ji == DJ - 1))
                v_sb = smp.tile([P, D], BF16, name="v_sb")
                nc.vector.tensor_tensor(out=v_sb[:], in0=pv[:], in1=vslice(VBV),
                                        op=mybir.AluOpType.add)
                nc.sync.dma_start(out=v_ag_in[t * P:(t + 1) * P, :], in_=v_sb[:])

            allgather(k_ag_in, k_full)
            allgather(v_ag_in, v_full)

            for j in range(DJ):
                n0 = 0
                while n0 < sh:
                    nn = min(512, sh - n0)
                    pq = ps.tile([P, 512], F32, name="pq", tag="pmm", bufs=kpmm)
                    for ji in range(DJ):
                        nc.tensor.matmul(
                            pq[:, :nn],
                            lhsT=wslice(WQ, ji)[:, j * P:(j + 1) * P],
                            rhs=xt[:, ji * sh + n0: ji * sh + n0 + nn],
                            start=(ji == 0), stop=(ji == DJ - 1),
                        )
                    nc.vector.tensor_scalar(
                        out=qT[j][:, n0:n0 + nn], in0=pq[:, :nn],
                        scalar1=vp[:, 9 * D + j: 9 * D + j + 1], scalar2=None,
                        op0=mybir.AluOpType.add,
                    )
                    n0 += nn

            # ---------------- transformer pass (gather-based) ----------------
            def transformer_pass(gp, pst, h_out, hT_out):
                dstc = gp.tile([P, stot], F32, name="dstc", bufs=1)
                nc.sync.dma_start(out=dstc[:], in_=dst_in[:])
                idx_sb = gp.tile([P, stot * 8], I16, name="idx_sb", bufs=1)
                nc.sync.dma_start(out=idx_sb[:], in_=idx_in[:])
                for t in range(nt):
                    st = s_list[t]
                    et_t = st * P
                    splits = []
                    base = (st + khalft - 1) // khalft
                    c0 = 0
                    while c0 < st:
                        splits.append((c0, min(st, c0 + base)))
                        c0 += base
                    kgt = gp.tile([P, DJ, et_t], BF16, name="kgt", tag="kgt",
                                  bufs=int(os.environ.get("KKGT", "2")))
                    vg = gp.tile([P, S, D], BF16, name="vg", tag="vg",
                                 bufs=int(os.environ.get("KVG", "2")))
                    idx_tt = idx_sb[:, soff[t] * 8:(soff[t] + st) * 8]
                    nc.gpsimd.dma_gather(
                        out_ap=kgt[:], in_ap=k_full[:], idxs_ap=idx_tt,
                        num_idxs=et_t, num_idxs_reg=et_t, elem_size=D,
                        transpose=True, single_packet=False)
                    for (ca, cb) in splits:
                        nn_i = (cb - ca) * P
                        idx_t = idx_sb[:, (soff[t] + ca) * 8: (soff[t] + cb) * 8]
                        nc.gpsimd.dma_gather(
                            out_ap=vg[:, ca:cb, :], in_ap=v_full[:], idxs_ap=idx_t,
                            num_idxs=nn_i, num_idxs_reg=nn_i, elem_size=D,
                            single_packet=False)

                    pagg = pst.tile([P, D + 1], F32, name="pagg", tag="pagg",
                                    bufs=kpagg)
                    for c in range(st):
                        dcol = dstc[:, soff[t] + c: soff[t] + c + 1]
                        psc = pst.tile([P, P], F32, name="psc", tag="psc", bufs=kpsc)
                        for j in range(DJ):
                            nc.tensor.matmul(
                                psc[:],
                                lhsT=kgt[:, j, c * P:(c + 1) * P],
                                rhs=qT[j][:, t * P:(t + 1) * P],
                                start=(j == 0), stop=(j == DJ - 1))
                        exps = smp.tile([P, P], F32, name="exps")
                        nc.scalar.activation(exps[:], psc[:],
                                             mybir.ActivationFunctionType.Exp,
                                             scale=scale)
                        w_b = smp.tile([P, P], BF16, name="w_b", tag="w_b")
                        nc.vector.scalar_tensor_tensor(
                            out=w_b[:], in0=iota_f[:], scalar=dcol, in1=exps[:],
                            op0=mybir.AluOpType.is_equal,
                            op1=mybir.AluOpType.mult)
                        nc.tensor.matmul(pagg[:, :D], lhsT=w_b[:], rhs=vg[:, c, :],
                                         start=(c == 0), stop=(c == st - 1))
                        nc.tensor.matmul(pagg[:, D:D + 1], lhsT=w_b[:],
                                         rhs=ones_v[:],
                                         start=False, stop=(c == st - 1))

                    smax = smp.tile([P, 1], F32, name="smax")
                    nc.vector.tensor_scalar(
                        out=smax[:], in0=pagg[:, D:D + 1], scalar1=1e-30,
                        scalar2=None, op0=mybir.AluOpType.max)
                    rs = smp.tile([P, 1], F32, name="rs")
                    nc.vector.reciprocal(rs[:], smax[:])
                    pskip = ps.tile([P, D], F32, name="pskip", tag="pmm", bufs=kpmm)
                    for ji in range(DJ):
                        nc.tensor.matmul(pskip[:], lhsT=xtile(ji, t),
                                         rhs=wslice(WS, ji),
                                         start=(ji == 0), stop=(ji == DJ - 1))
                    t1 = smp.tile([P, D], F32, name="t1", tag="t1")
                    nc.scalar.activation(t1[:], pagg[:, :D],
                                         mybir.ActivationFunctionType.Copy,
                                         scale=rs[:, :1])
                    t2 = smp.tile([P, D], F32, name="t2", tag="t2")
                    nc.vector.tensor_tensor(out=t2[:], in0=t1[:], in1=pskip[:],
                                            op=mybir.AluOpType.add)
                    t3 = smp.tile([P, D], F32, name="t3", tag="t3")
                    nc.vector.tensor_tensor(out=t3[:], in0=t2[:], in1=vslice(VBS),
                                            op=mybir.AluOpType.add)
                    nc.scalar.activation(h_out[:, t * D:(t + 1) * D], t3[:],
                                         mybir.ActivationFunctionType.Relu)
                    epilogue_store(0, t, h_out, hT_out)
                allgather(hag_in[0], h_full[0])

            def epilogue_store(li, t, h_out, hT_out):
                """Write fp8 pack row + transposed shard for layer output li."""
                hstage = smp.tile([P, D], FP8, name="hstage")
                nc.scalar.copy(out=hstage[:], in_=h_out[:, t * D:(t + 1) * D])
                nc.sync.dma_start(out=hag_in[li][:, t * D:(t + 1) * D],
                                  in_=hstage[:])
                for j in range(DJ):
                    ptr2 = ps.tile([P, P], BF16, name="ptr2", tag="ptr", bufs=kptr)
                    nc.tensor.transpose(
                        out=ptr2[:],
                        in_=h_out[:, t * D + j * P: t * D + (j + 1) * P],
                        identity=ident_b[:])
                    nc.scalar.copy(
                        out=hT_out[:, j * sh + t * P: j * sh + (t + 1) * P],
                        in_=ptr2[:])

            # ---------------- SAGE pass (dense fp8 DoubleRow) ----------------
            def sage_pass(layer, pd, a_sb, h_prev, hT_prev, h_out, hT_out):
                li = layer + 1
                # load H table (tile-major fp8) from the AllGathered pack
                for r in range(NC):
                    nc.sync.dma_start(
                        out=h_sb[:, r * nt:(r + 1) * nt, :],
                        in_=h_full[layer][r * P:(r + 1) * P, :])

                pbank = [pd.tile([P, 4 * P], F32, name=f"pdb_{b}", tag=f"pdb_{b}",
                                 bufs=1) for b in range((nt + 3) // 4)]

                def ptile(t):
                    return pbank[t // 4][:, (t % 4) * P:(t % 4 + 1) * P]

                for dh in range(DJ):
                    for b in range(len(pbank)):
                        nc.vector.memset(pbank[b][:], 0.0)
                    for c2 in range(NC2):
                        lhs = h_sb[:, 2 * c2:2 * c2 + 2, dh * P:(dh + 1) * P]
                        for t in range(nt):
                            blk = c2 * nt + t
                            nc.tensor.matmul(
                                ptile(t),
                                lhsT=lhs,
                                rhs=a_sb[:, 2 * blk:2 * blk + 2, :],
                                start=False, stop=(c2 == NC2 - 1),
                                perf_mode=mybir.MatmulPerfMode.DoubleRow,
                                skip_group_check=True)
                    for t in range(nt):
                        nc.scalar.copy(
                            out=meanT_all[:, t, dh * P:(dh + 1) * P],
                            in_=ptile(t))

                for t in range(nt):
                    pzm = ps.tile([P, D], F32, name="pzm", tag="pzm", bufs=1)
                    for j in range(DJ):
                        nc.tensor.matmul(pzm[:],
                                         lhsT=meanT_all[:, t, j * P:(j + 1) * P],
                                         rhs=wslice(WL[layer], j),
                                         start=(j == 0), stop=(j == DJ - 1))
                    pz = ps.tile([P, D], F32, name="pz", tag="pmm", bufs=kpmm)
                    for j in range(DJ):
                        nc.tensor.matmul(
                            pz[:],
                            lhsT=hT_prev[:, j * sh + t * P: j * sh + (t + 1) * P],
                            rhs=wslice(WR[layer], j),
                            start=(j == 0), stop=(j == DJ - 1))
                    bx = vslice(4 + 2 * layer)
                    t0 = smp.tile([P, D], F32, name="t0s", tag="t0")
                    nc.vector.tensor_scalar(
                        out=t0[:], in0=pzm[:], scalar1=invd[:, t:t + 1],
                        scalar2=None, op0=mybir.AluOpType.mult)
                    t1 = smp.tile([P, D], F32, name="t1s", tag="t2")
                    nc.vector.tensor_tensor(out=t1[:], in0=t0[:], in1=pz[:],
                                            op=mybir.AluOpType.add)
                    t2 = smp.tile([P, D], F32, name="t2s", tag="t3")
                    nc.vector.tensor_tensor(out=t2[:], in0=t1[:], in1=bx,
                                            op=mybir.AluOpType.add)
                    t3 = smp.tile([P, D], F32, name="t3s", tag="t4")
                    nc.vector.scalar_tensor_tensor(
                        out=t3[:], in0=h_prev[:, t * D:(t + 1) * D], scalar=oma,
                        in1=t2[:], op0=mybir.AluOpType.mult,
                        op1=mybir.AluOpType.add)
                    if layer < L - 1:
                        nc.scalar.activation(h_out[:, t * D:(t + 1) * D], t3[:],
                                             mybir.ActivationFunctionType.Relu)
                        epilogue_store(li, t, h_out, hT_out)
                    else:
                        hof = smp.tile([P, D], F32, name="hof", tag="t1")
                        nc.scalar.activation(hof[:], t3[:],
                                             mybir.ActivationFunctionType.Relu)
                        nc.sync.dma_start(out=out_dram[t * P:(t + 1) * P, :],
                                          in_=hof[:])
                if layer < L - 1:
                    allgather(hag_in[li], h_full[li])

            if stages <= 1:
                tmpo = smp.tile([P, D], F32, name="tmpo")
                for t in range(nt):
                    nc.vector.tensor_copy(out=tmpo[:], in_=xt[:, :D])
                    nc.sync.dma_start(out=out_dram[t * P:(t + 1) * P, :],
                                      in_=tmpo[:])
            else:
                with (
                    tc.tile_pool(name="g", bufs=kgp) as gp,
                    tc.tile_pool(name="pst", bufs=1, space="PSUM") as pst,
                ):
                    transformer_pass(gp, pst, h_cur, hT_cur)
                if stages >= 3:
                    with (
                        tc.tile_pool(name="ap", bufs=1) as app,
                        tc.tile_pool(name="pd", bufs=1, space="PSUM") as pd,
                    ):
                        a_sb = app.tile([P, NBLK * 2, P], FP8)
                        pw = (NBLK + kapc - 1) // kapc
                        for p_ in range(kapc):
                            b0, b1 = p_ * pw, min(NBLK, (p_ + 1) * pw)
                            nc.sync.dma_start(
                                out=a_sb[:, 2 * b0:2 * b1, :],
                                in_=a_in[:, 2 * b0 * P:2 * b1 * P])
                        bufs = [(h_cur, hT_cur), (h_nxt, hT_nxt)]
                        for i in range(min(L, stages - 2)):
                            h_prev, hT_prev = bufs[i % 2]
                            h_out, hT_out = bufs[(i + 1) % 2]
                            sage_pass(i, pd, a_sb, h_prev, hT_prev, h_out, hT_out)
                        if stages - 2 < L:
                            hsrc, _ = bufs[max(0, stages - 2) % 2]
                            for t in range(nt):
                                hof = smp.tile([P, D], F32, name="hof", tag="t1")
                                nc.vector.tensor_copy(
                                    out=hof[:], in_=hsrc[:, t * D:(t + 1) * D])
                                nc.sync.dma_start(
                                    out=out_dram[t * P:(t + 1) * P, :], in_=hof[:])
                else:
                    for t in range(nt):
                        hof = smp.tile([P, D], F32, name="hof", tag="t1")
                        nc.vector.tensor_copy(out=hof[:],
                                              in_=h_cur[:, t * D:(t + 1) * D])
                        nc.sync.dma_start(out=out_dram[t * P:(t + 1) * P, :],
                                          in_=hof[:])

    nc.compile()
    _nc_cache[key] = nc
    return nc


def _host_prep(x, src, dst, Wq, bq, Wk, bk, Wv, bv, Ws, bs, Wl, bl, Wr,
               gamma, beta, alpha_res):
    n, d = x.shape
    n_pad = ((n + NC * P - 1) // (NC * P)) * (NC * P)
    sh = n_pad // NC
    nt = sh // P
    n_tiles = n_pad // P

    order = np.argsort(dst, kind="stable")
    src_s, dst_s = src[order], dst[order]
    tile_of = dst_s // P
    counts = np.bincount(tile_of, minlength=n_tiles)
    starts = np.concatenate([[0], np.cumsum(counts)])
    s_all = np.maximum(1, (counts + P - 1) // P).astype(np.int64)
    s_pos = s_all.reshape(NC, nt).max(axis=0)
    s_list = tuple(int(v) for v in s_pos)
    soff = np.concatenate([[0], np.cumsum(s_pos)])
    stot = int(soff[-1])

    deg = np.bincount(dst, minlength=n_pad).astype(np.float32)
    invdeg_full = 1.0 / np.maximum(deg, 1.0)

    al = 1.0 / (1.0 + np.exp(-alpha_res))
    oma = float(1.0 - al)
    bn_scale = 1.0 / np.sqrt(1.0 + BN_EPS)
    scale = 1.0 / np.sqrt(float(d))

    x_pad = np.zeros((n_pad, D), np.float32)
    x_pad[:n] = x
    xT = x_pad.T.astype(NP_BF16)

    # fold the BN gain into the SAGE weights: z*Gx == mean@(Wl*Gx) + h@(Wr*Gx)
    GxF = [al * bn_scale * gamma[i] for i in range(L)]
    Wlg = [Wl[i] * GxF[i][None, :] for i in range(L)]
    Wrg = [Wr[i] * GxF[i][None, :] for i in range(L)]
    weights = [Wq, Wk, Wv, Ws, Wlg[0], Wrg[0], Wlg[1], Wrg[1], Wlg[2], Wrg[2]]
    wpack = np.empty((P, 10 * DJ * D), NP_BF16)
    for w, W in enumerate(weights):
        for j in range(DJ):
            wpack[:, (w * DJ + j) * D:(w * DJ + j + 1) * D] = \
                W[j * P:(j + 1) * P, :].astype(NP_BF16)

    Gx = [al * bn_scale * gamma[i] for i in range(L)]
    Bx = [al * (bl[i] * bn_scale * gamma[i] + beta[i]) for i in range(L)]
    vecs = [bk, bv, bs, Gx[0], Bx[0], Gx[1], Bx[1], Gx[2], Bx[2]]
    vpack = np.empty((P, 9 * D + DJ), np.float32)
    for k, v in enumerate(vecs):
        vpack[:, k * D:(k + 1) * D] = np.tile(v[None, :], (P, 1))
    for j in range(DJ):
        vpack[:, 9 * D + j] = bq[j * P:(j + 1) * P]

    # dense mean-normalized adjacency in DoubleRow block layout, per core:
    # a[k, ((c2*nt + t)*2 + i)*P + m] = mult(src=256*c2+128*i+k -> dst) / deg
    NC2 = (n_pad // P) // 2
    NBLK = NC2 * nt
    kk = (src_s % P).astype(np.int64)
    ii = (src_s // P) % 2
    c2 = src_s // (2 * P)
    w_e = np.ones(len(dst_s), np.float32)  # integer counts; exact in fp8

    in_maps = []
    for r in range(NC):
        idx_arr = np.zeros((P, stot * 8), np.int16)
        dst_arr = np.full((P, stot), 128.0, np.float32)
        for tloc in range(nt):
            g = r * nt + tloc
            e0, e1 = starts[g], starts[g + 1]
            cnt = e1 - e0
            st_ = int(s_pos[tloc])
            et_t = st_ * P
            srcs = np.zeros(et_t, np.int64)
            srcs[:cnt] = src_s[e0:e1]
            dl = np.full(et_t, 128, np.int64)
            dl[:cnt] = dst_s[e0:e1] - g * P
            o = int(soff[tloc])
            idx_arr[:, o * 8:(o + st_) * 8] = _wrap_idx(srcs)
            dst_arr[:, o:o + st_] = dl.reshape(st_, P).T

        e0, e1 = starts[r * nt], starts[(r + 1) * nt]
        tl = (dst_s[e0:e1] - r * sh) // P
        mm = (dst_s[e0:e1] - r * sh) % P
        flat = ((c2[e0:e1] * nt + tl) * 2 + ii[e0:e1]) * P + mm
        a_arr = np.zeros((P, NBLK * 2 * P), np.float32)
        np.add.at(a_arr, (kk[e0:e1], flat), w_e[e0:e1])
        a_arr = a_arr.astype(NP_FP8)
        invdeg_r = invdeg_full[r * sh:(r + 1) * sh].reshape(nt, P).T.copy()

        xt_r = np.empty((P, DJ * sh), NP_BF16)
        for j in range(DJ):
            xt_r[:, j * sh:(j + 1) * sh] = xT[j * P:(j + 1) * P, r * sh:(r + 1) * sh]

        in_maps.append({
            "xt_in": xt_r,
            "wpack_in": wpack,
            "vpack_in": vpack,
            "idx_in": idx_arr,
            "dst_in": dst_arr,
            "invdeg_in": np.ascontiguousarray(invdeg_r),
            "a_in": a_arr,
        })
    return in_maps, (n_pad, sh, nt, s_list, scale, oma)


def kernel(**inputs):
    x = np.asarray(inputs["x"], np.float32)
    edge_index = np.asarray(inputs["edge_index"])
    args = dict(
        Wq=np.asarray(inputs["Wq"], np.float32), bq=np.asarray(inputs["bq"], np.float32),
        Wk=np.asarray(inputs["Wk"], np.float32), bk=np.asarray(inputs["bk"], np.float32),
        Wv=np.asarray(inputs["Wv"], np.float32), bv=np.asarray(inputs["bv"], np.float32),
        Ws=np.asarray(inputs["Ws"], np.float32), bs=np.asarray(inputs["bs"], np.float32),
        Wl=np.asarray(inputs["Wl"], np.float32), bl=np.asarray(inputs["bl"], np.float32),
        Wr=np.asarray(inputs["Wr"], np.float32),
        gamma=np.asarray(inputs["gamma"], np.float32),
        beta=np.asarray(inputs["beta"], np.float32),
        alpha_res=float(np.asarray(inputs["alpha_res"])),
    )
    src = edge_index[0].astype(np.int64)
    dst = edge_index[1].astype(np.int64)

    in_maps, params = _host_prep(x, src, dst, **args)
    t0 = time.time()
    nc = build_nc(*params)
    print(f"[kernel] build+compile {time.time()-t0:.1f}s", flush=True)
    t0 = time.time()
    res = run_bass_kernel_spmd(nc, in_maps, core_ids=list(range(NC)))
    print(f"[kernel] run {time.time()-t0:.1f}s", flush=True)
    out = np.concatenate([res.results[r]["out"] for r in range(NC)], axis=0)
    return out[:x.shape[0]]


# revision 43
# speedup vs baseline: 1.2267x; 1.0704x over previous
"""Trainium2 Bass kernel for nn_MixGNN (TransformerConv + 3x SAGEConv + BN + gated residual).

Strategy (8 NeuronCores, dst-node sharding):
  - Pad N 10000 -> 10240; core r owns 1280 dst nodes = 10 tiles of 128.
  - TransformerConv: per-edge gather machinery — dma_gather of source k rows
    (transposed, bf16) + v rows; attention scores as KgT.T @ qT on PE;
    softmax without max-subtraction; exp-weighted indicator matmuls on PE.
  - SAGEConv x3: dense fp8 DoubleRow aggregation. Host precomputes the
    mean-normalized adjacency A[src, dst] = mult/deg (fp8e4) in PE DoubleRow
    block layout; each pass computes meanT[d, dst] = sum_src H[src, d] *
    A[src, dst] as 256-src-deep fp8 matmuls at 0.5 cyc/row. The full H table
    (fp8, tile-major) lives in SBUF, refreshed per pass via AllGather.
  - Halo exchange: AllGather of bf16 k/v tables (transformer) and fp8 packed
    h tables (SAGE) through shared DRAM.
Output: fp32 [10000, 256].
"""
import os
import sys
import time

import numpy as np

for _p in ("/opt/trn_rl_repo",):
    if _p not in sys.path:
        sys.path.insert(0, _p)

import ml_dtypes  # noqa: E402
import concourse.bacc as bacc  # noqa: E402
import concourse.mybir as mybir  # noqa: E402
import concourse.tile as tile  # noqa: E402
from concourse.bass_utils import run_bass_kernel_spmd  # noqa: E402

P = 128
D = 256
DJ = D // P           # 2 d-chunks of 128
NC = 8                # cores
L = 3                 # SAGE layers
BN_EPS = 1e-5

F32 = mybir.dt.float32
BF16 = mybir.dt.bfloat16
FP8 = mybir.dt.float8e4
I16 = mybir.dt.int16
NP_FP8 = ml_dtypes.float8_e4m3
NP_BF16 = ml_dtypes.bfloat16

_nc_cache = {}


def _wrap_idx(a):
    """[S*128] int array -> [128, S*8] int16 wrapped gather-index layout."""
    w16 = a.reshape(-1, 16).T.astype(np.int16)   # [16, S*8]
    return np.tile(w16, (8, 1))                  # replicate to 8 Q7 stripes


def build_nc(n_pad, sh, nt, s_list, scale, oma):
    stages = int(os.environ.get("KSTAGES", "5"))
    nocc = os.environ.get("KNOCC") == "1"
    ksm = int(os.environ.get("KSM", "4"))
    kgp = int(os.environ.get("KGP", "2"))
    kpsc = int(os.environ.get("KPSC", "3"))
    kpagg = int(os.environ.get("KPAGG", "2"))
    kpmm = int(os.environ.get("KPMM", "2"))
    kptr = int(os.environ.get("KPTR", "1"))
    khalft = int(os.environ.get("KHALFT", "2"))  # transformer v-gather splits
    kapc = int(os.environ.get("KAPC", "8"))      # A-table load pieces
    key = (n_pad, sh, nt, s_list, round(scale, 9), round(oma, 9), stages,
           nocc, ksm, kgp, kpsc, kpagg, kpmm, kptr, khalft, kapc,
           os.environ.get("KKGT"), os.environ.get("KVG"))
    if key in _nc_cache:
        return _nc_cache[key]

    S = max(s_list)
    soff = [0]
    for st_ in s_list:
        soff.append(soff[-1] + st_)
    stot = soff[-1]

    NCH = n_pad // P          # 80 source chunks of 128
    NC2 = NCH // 2            # 40 double-chunks of 256
    NBLK = NC2 * nt           # 400 A-blocks per core

    ndev = 1 if nocc else NC
    nc = bacc.Bacc("TRN2", target_bir_lowering=False, debug=False, num_devices=ndev)

    xt_in = nc.dram_tensor("xt_in", [P, DJ * sh], BF16, kind="ExternalInput")
    wpack_in = nc.dram_tensor("wpack_in", [P, (10 * DJ + 4) * D], BF16,
                              kind="ExternalInput")
    vpack_in = nc.dram_tensor("vpack_in", [P, 9 * D + DJ], F32, kind="ExternalInput")
    idx_in = nc.dram_tensor("idx_in", [P, stot * 8], I16, kind="ExternalInput")
    dst_in = nc.dram_tensor("dst_in", [P, stot], F32, kind="ExternalInput")
    invdeg_in = nc.dram_tensor("invdeg_in", [P, nt], F32, kind="ExternalInput")
    a_in = nc.dram_tensor("a_in", [P, NBLK * 2 * P], FP8, kind="ExternalInput")
    out_dram = nc.dram_tensor("out", [sh, D], F32, kind="ExternalOutput")

    WQ, WK, WV, WS = 0, 1, 2, 3
    WL = [4, 6, 8]
    WR = [5, 7, 9]
    WBIAS = 10 * DJ          # 4 bias rows (bs, Bx0..2) in wpack row 0
    VBK, VBV, VBS = 0, 1, 2

    with tile.TileContext(nc) as tc:
        with (
            tc.tile_pool(name="cst", bufs=1) as cst,
            tc.tile_pool(name="sb", bufs=1) as sb,
            tc.tile_pool(name="sm", bufs=ksm) as smp,
            tc.tile_pool(name="ps", bufs=2, space="PSUM") as ps,
            tc.tile_pool(name="dr", bufs=1, space="DRAM") as dr,
        ):
            # ---------------- constants / inputs to SBUF ----------------
            xt = cst.tile([P, DJ * sh], BF16)
            for _xi in range(2):
                _c0 = _xi * (DJ * sh // 2)
                _c1 = (_xi + 1) * (DJ * sh // 2)
                nc.sync.dma_start(out=xt[:, _c0:_c1], in_=xt_in[:, _c0:_c1])
            wp = cst.tile([P, (10 * DJ + 4) * D], BF16)
            nc.sync.dma_start(out=wp[:], in_=wpack_in[:])
            vp = cst.tile([P, 9 * D + DJ], F32)
            nc.sync.dma_start(out=vp[:], in_=vpack_in[:])
            invd = cst.tile([P, nt], F32)
            nc.sync.dma_start(out=invd[:], in_=invdeg_in[:])

            iota_i = cst.tile([P, P], mybir.dt.int32)
            nc.gpsimd.iota(iota_i[:], pattern=[[1, P]], base=0, channel_multiplier=0)
            ones_v = cst.tile([P, 1], BF16)
            nc.vector.memset(ones_v[:], 1.0)
            iota_part = cst.tile([P, 1], mybir.dt.int32)
            nc.gpsimd.iota(iota_part[:], pattern=[[1, 1]], base=0, channel_multiplier=1)
            iota_part_f = cst.tile([P, 1], F32)
            nc.vector.tensor_copy(out=iota_part_f[:], in_=iota_part[:])
            iota_f = cst.tile([P, P], F32)
            nc.vector.tensor_copy(out=iota_f[:], in_=iota_i[:])
            ident = cst.tile([P, P], F32)
            nc.vector.tensor_scalar(
                out=ident[:], in0=iota_f[:], scalar1=iota_part_f[:, :1], scalar2=None,
                op0=mybir.AluOpType.is_equal,
            )
            ident_b = cst.tile([P, P], BF16)
            nc.vector.tensor_copy(out=ident_b[:], in_=ident[:])

            def wslice(w, j):
                return wp[:, (w * DJ + j) * D:(w * DJ + j + 1) * D]

            def vslice(k):
                return vp[:, k * D:(k + 1) * D]

            def xtile(j, t):
                return xt[:, j * sh + t * P: j * sh + (t + 1) * P]

            # ---------------- long-lived SBUF state ----------------
            qT = [sb.tile([P, sh], BF16, name=f"qT_{j}") for j in range(DJ)]
            h_cur = sb.tile([P, nt * D], BF16)
            h_nxt = sb.tile([P, nt * D], BF16)
            hT_cur = sb.tile([P, DJ * sh], BF16)
            hT_nxt = sb.tile([P, DJ * sh], BF16)
            h_sb = sb.tile([P, NCH, D], FP8)     # full H table, tile-major
            meanT_all = sb.tile([P, nt, D], BF16)  # paggT staging per pass

            ones_row = cst.tile([1, P], BF16)
            nc.vector.memset(ones_row[:], 1.0)

            # ---------------- DRAM tables ----------------
            k_ag_in = dr.tile([sh, D], BF16)
            v_ag_in = dr.tile([sh, D], BF16)
            k_full = dr.tile([n_pad, D], BF16, addr_space="Shared")
            v_full = dr.tile([n_pad, D], BF16, addr_space="Shared")
            hag_in = [dr.tile([P, nt * D], FP8, name=f"hag_in_{i}") for i in range(L)]
            h_full = [dr.tile([NC * P, nt * D], FP8, name=f"h_full_{i}",
                              addr_space="Shared") for i in range(L)]

            def allgather(in_t, out_t):
                if nocc:
                    nc.sync.dma_start(out=out_t[:in_t.shape[0]], in_=in_t[:])
                else:
                    nc.gpsimd.collective_compute(
                        "AllGather", mybir.AluOpType.bypass,
                        replica_groups=[list(range(NC))],
                        ins=[in_t[:]], outs=[out_t[:]],
                    )

            # ---------------- stage 0: k,v shard tables + AG, then qT --------
            for t in range(nt):
                pk = ps.tile([P, D], F32, name="pk", tag="pmm", bufs=kpmm)
                for ji in range(DJ):
                    nc.tensor.matmul(pk[:], lhsT=xtile(ji, t), rhs=wslice(WK, ji),
                                     start=(ji == 0), stop=(ji == DJ - 1))
                k_sb = smp.tile([P, D], BF16, name="k_sb")
                nc.vector.tensor_tensor(out=k_sb[:], in0=pk[:], in1=vslice(VBK),
                                        op=mybir.AluOpType.add)
                nc.sync.dma_start(out=k_ag_in[t * P:(t + 1) * P, :], in_=k_sb[:])

                pv = ps.tile([P, D], F32, name="pv", tag="pmm", bufs=kpmm)
                for ji in range(DJ):
                    nc.tensor.matmul(pv[:], lhsT=xtile(ji, t), rhs=wslice(WV, ji),
                                     start=(ji == 0), stop=(ji == DJ - 1))
                v_sb = smp.tile([P, D], BF16, name="v_sb")
                nc.vector.tensor_tensor(out=v_sb[:], in0=pv[:], in1=vslice(VBV),
                                        op=mybir.AluOpType.add)
                nc.sync.dma_start(out=v_ag_in[t * P:(t + 1) * P, :], in_=v_sb[:])

            allgather(k_ag_in, k_full)
            allgather(v_ag_in, v_full)

            for j in range(DJ):
                n0 = 0
                while n0 < sh:
                    nn = min(512, sh - n0)
                    pq = ps.tile([P, 512], F32, name="pq", tag="pmm", bufs=kpmm)
                    for ji in range(DJ):
                        nc.tensor.matmul(
                            pq[:, :nn],
                            lhsT=wslice(WQ, ji)[:, j * P:(j + 1) * P],
                            rhs=xt[:, ji * sh + n0: ji * sh + n0 + nn],
                            start=(ji == 0), stop=(ji == DJ - 1),
                        )
                    nc.vector.tensor_scalar(
                        out=qT[j][:, n0:n0 + nn], in0=pq[:, :nn],
                        scalar1=vp[:, 9 * D + j: 9 * D + j + 1], scalar2=None,
                        op0=mybir.AluOpType.add,
                    )
                    n0 += nn

            # ---------------- transformer pass (gather-based) ----------------
            def transformer_pass(gp, pst, h_out, hT_out):
                dstc = gp.tile([P, stot], F32, name="dstc", bufs=1)
                nc.sync.dma_start(out=dstc[:], in_=dst_in[:])
                idx_sb = gp.tile([P, stot * 8], I16, name="idx_sb", bufs=1)
                nc.sync.dma_start(out=idx_sb[:], in_=idx_in[:])
                for t in range(nt):
                    st = s_list[t]
                    et_t = st * P
                    splits = []
                    base = (st + khalft - 1) // khalft
                    c0 = 0
                    while c0 < st:
                        splits.append((c0, min(st, c0 + base)))
                        c0 += base
                    kgt = gp.tile([P, DJ, et_t], BF16, name="kgt", tag="kgt",
                                  bufs=int(os.environ.get("KKGT", "2")))
                    vg = gp.tile([P, S, D], BF16, name="vg", tag="vg",
                                 bufs=int(os.environ.get("KVG", "2")))
                    idx_tt = idx_sb[:, soff[t] * 8:(soff[t] + st) * 8]
                    nc.gpsimd.dma_gather(
                        out_ap=kgt[:], in_ap=k_full[:], idxs_ap=idx_tt,
                        num_idxs=et_t, num_idxs_reg=et_t, elem_size=D,
                        transpose=True, single_packet=False)
                    for (ca, cb) in splits:
                        nn_i = (cb - ca) * P
                        idx_t = idx_sb[:, (soff[t] + ca) * 8: (soff[t] + cb) * 8]
                        nc.gpsimd.dma_gather(
                            out_ap=vg[:, ca:cb, :], in_ap=v_full[:], idxs_ap=idx_t,
                            num_idxs=nn_i, num_idxs_reg=nn_i, elem_size=D,
                            single_packet=False)

                    pagg = pst.tile([P, D + 1], F32, name="pagg", tag="pagg",
                                    bufs=kpagg)
                    for c in range(st):
                        dcol = dstc[:, soff[t] + c: soff[t] + c + 1]
                        psc = pst.tile([P, P], F32, name="psc", tag="psc", bufs=kpsc)
                        for j in range(DJ):
                            nc.tensor.matmul(
                                psc[:],
                                lhsT=kgt[:, j, c * P:(c + 1) * P],
                                rhs=qT[j][:, t * P:(t + 1) * P],
                                start=(j == 0), stop=(j == DJ - 1))
                        exps = smp.tile([P, P], F32, name="exps")
                        nc.scalar.activation(exps[:], psc[:],
                                             mybir.ActivationFunctionType.Exp,
                                             scale=scale)
                        w_b = smp.tile([P, P], BF16, name="w_b", tag="w_b")
                        nc.vector.scalar_tensor_tensor(
                            out=w_b[:], in0=iota_f[:], scalar=dcol, in1=exps[:],
                            op0=mybir.AluOpType.is_equal,
                            op1=mybir.AluOpType.mult)
                        nc.tensor.matmul(pagg[:, :D], lhsT=w_b[:], rhs=vg[:, c, :],
                                         start=(c == 0), stop=(c == st - 1))
                        nc.tensor.matmul(pagg[:, D:D + 1], lhsT=w_b[:],
                                         rhs=ones_v[:],
                                         start=False, stop=(c == st - 1))

                    smax = smp.tile([P, 1], F32, name="smax")
                    nc.vector.tensor_scalar(
                        out=smax[:], in0=pagg[:, D:D + 1], scalar1=1e-30,
                        scalar2=None, op0=mybir.AluOpType.max)
                    rs = smp.tile([P, 1], F32, name="rs")
                    nc.vector.reciprocal(rs[:], smax[:])
                    pskip = ps.tile([P, D], F32, name="pskip", tag="pmm", bufs=kpmm)
                    for ji in range(DJ):
                        nc.tensor.matmul(pskip[:], lhsT=xtile(ji, t),
                                         rhs=wslice(WS, ji),
                                         start=(ji == 0), stop=False)
                    nc.tensor.matmul(pskip[:], lhsT=ones_row[:],
                                     rhs=wp[:1, WBIAS * D:(WBIAS + 1) * D],
                                     start=False, stop=True)
                    t1 = smp.tile([P, D], F32, name="t1", tag="t1")
                    nc.scalar.activation(t1[:], pagg[:, :D],
                                         mybir.ActivationFunctionType.Copy,
                                         scale=rs[:, :1])
                    t2 = smp.tile([P, D], F32, name="t2", tag="t2")
                    nc.vector.tensor_tensor(out=t2[:], in0=t1[:], in1=pskip[:],
                                            op=mybir.AluOpType.add)
                    nc.scalar.activation(h_out[:, t * D:(t + 1) * D], t2[:],
                                         mybir.ActivationFunctionType.Relu)
                    epilogue_store(0, t, h_out, hT_out)
                allgather(hag_in[0], h_full[0])

            def epilogue_store(li, t, h_out, hT_out):
                """Write fp8 pack row + transposed shard for layer output li."""
                hstage = smp.tile([P, D], FP8, name="hstage")
                nc.scalar.copy(out=hstage[:], in_=h_out[:, t * D:(t + 1) * D])
                nc.sync.dma_start(out=hag_in[li][:, t * D:(t + 1) * D],
                                  in_=hstage[:])
                for j in range(DJ):
                    ptr2 = ps.tile([P, P], BF16, name="ptr2", tag="ptr", bufs=kptr)
                    nc.tensor.transpose(
                        out=ptr2[:],
                        in_=h_out[:, t * D + j * P: t * D + (j + 1) * P],
                        identity=ident_b[:])
                    nc.scalar.copy(
                        out=hT_out[:, j * sh + t * P: j * sh + (t + 1) * P],
                        in_=ptr2[:])

            # ---------------- SAGE pass (dense fp8 DoubleRow) ----------------
            # dst tiles processed in groups of 4; one [P, 512] PSUM bank per
            # (group, dh), double-buffered so group g+1's adjacency sweep
            # overlaps group g's epilogue.
            def sage_pass(layer, pd, a_sb, h_prev, hT_prev, h_out, hT_out):
                li = layer + 1
                # load H table (tile-major fp8) from the AllGathered pack
                for r in range(NC):
                    nc.sync.dma_start(
                        out=h_sb[:, r * nt:(r + 1) * nt, :],
                        in_=h_full[layer][r * P:(r + 1) * P, :])
                groups = [(g * 4, min(nt, (g + 1) * 4)) for g in range((nt + 3) // 4)]

                def sage_epilogue(t):
                    pzm = pd.tile([P, D], F32, name="pzm", tag="pzm", bufs=1)
                    for j in range(DJ):
                        nc.tensor.matmul(pzm[:],
                                         lhsT=meanT_all[:, t, j * P:(j + 1) * P],
                                         rhs=wslice(WL[layer], j),
                                         start=(j == 0), stop=(j == DJ - 1))
                    pz = ps.tile([P, D], F32, name="pz", tag="pmm", bufs=kpmm)
                    for j in range(DJ):
                        nc.tensor.matmul(
                            pz[:],
                            lhsT=hT_prev[:, j * sh + t * P: j * sh + (t + 1) * P],
                            rhs=wslice(WR[layer], j),
                            start=(j == 0), stop=False)
                    nc.tensor.matmul(
                        pz[:], lhsT=ones_row[:],
                        rhs=wp[:1, (WBIAS + 1 + layer) * D:(WBIAS + 2 + layer) * D],
                        start=False, stop=True)
                    t0 = smp.tile([P, D], F32, name="t0s", tag="t0")
                    nc.scalar.activation(t0[:], pzm[:],
                                         mybir.ActivationFunctionType.Copy,
                                         scale=invd[:, t:t + 1])
                    t1 = smp.tile([P, D], F32, name="t1s", tag="t2")
                    nc.vector.tensor_tensor(out=t1[:], in0=t0[:], in1=pz[:],
                                            op=mybir.AluOpType.add)
                    t3 = smp.tile([P, D], F32, name="t3s", tag="t4")
                    nc.vector.scalar_tensor_tensor(
                        out=t3[:], in0=h_prev[:, t * D:(t + 1) * D], scalar=oma,
                        in1=t1[:], op0=mybir.AluOpType.mult,
                        op1=mybir.AluOpType.add)
                    if layer < L - 1:
                        nc.scalar.activation(h_out[:, t * D:(t + 1) * D], t3[:],
                                             mybir.ActivationFunctionType.Relu)
                        epilogue_store(li, t, h_out, hT_out)
                    else:
                        hof = smp.tile([P, D], F32, name="hof", tag="t1")
                        nc.scalar.activation(hof[:], t3[:],
                                             mybir.ActivationFunctionType.Relu)
                        nc.sync.dma_start(out=out_dram[t * P:(t + 1) * P, :],
                                          in_=hof[:])

                for gi, (ta, tb) in enumerate(groups):
                    tw = (tb - ta) * P
                    pb = [pd.tile([P, 4 * P], F32, name=f"pdb_{gi % 2}_{dh}",
                                  tag=f"pdb_{gi % 2}_{dh}", bufs=1)
                          for dh in range(DJ)]
                    for dh in range(DJ):
                        for c2 in range(NC2):
                            nc.tensor.matmul(
                                pb[dh][:, :tw],
                                lhsT=h_sb[:, 2 * c2:2 * c2 + 2, dh * P:(dh + 1) * P],
                                rhs=a_sb[:, c2, :, ta * P:tb * P],
                                start=(c2 == 0), stop=(c2 == NC2 - 1),
                                perf_mode=mybir.MatmulPerfMode.DoubleRow)
                        nc.vector.tensor_copy(
                            out=meanT_all[:, ta:tb, dh * P:(dh + 1) * P],
                            in_=pb[dh][:, :tw])
                    for t in range(ta, tb):
                        sage_epilogue(t)
                if layer < L - 1:
                    allgather(hag_in[li], h_full[li])

            if stages <= 1:
                tmpo = smp.tile([P, D], F32, name="tmpo")
                for t in range(nt):
                    nc.vector.tensor_copy(out=tmpo[:], in_=xt[:, :D])
                    nc.sync.dma_start(out=out_dram[t * P:(t + 1) * P, :],
                                      in_=tmpo[:])
            else:
                with (
                    tc.tile_pool(name="g", bufs=kgp) as gp,
                    tc.tile_pool(name="pst", bufs=1, space="PSUM") as pst,
                ):
                    transformer_pass(gp, pst, h_cur, hT_cur)
                if stages >= 3:
                    with (
                        tc.tile_pool(name="ap", bufs=1) as app,
                        tc.tile_pool(name="pd", bufs=1, space="PSUM") as pd,
                    ):
                        a_sb = app.tile([P, NC2, 2, nt * P], FP8)
                        cw = (NC2 + kapc - 1) // kapc
                        for p_ in range(kapc):
                            c0, c1 = p_ * cw, min(NC2, (p_ + 1) * cw)
                            # ACT-issued so H-table loads (SP queue) interleave
                            nc.scalar.dma_start(
                                out=a_sb[:, c0:c1, :, :],
                                in_=a_in[:, c0 * 2 * nt * P:c1 * 2 * nt * P])
                        bufs = [(h_cur, hT_cur), (h_nxt, hT_nxt)]
                        for i in range(min(L, stages - 2)):
                            h_prev, hT_prev = bufs[i % 2]
                            h_out, hT_out = bufs[(i + 1) % 2]
                            sage_pass(i, pd, a_sb, h_prev, hT_prev, h_out, hT_out)
                        if stages - 2 < L:
                            hsrc, _ = bufs[max(0, stages - 2) % 2]
                            for t in range(nt):
                                hof = smp.tile([P, D], F32, name="hof", tag="t1")
                                nc.vector.tensor_copy(
                                    out=hof[:], in_=hsrc[:, t * D:(t + 1) * D])
                                nc.sync.dma_start(
                                    out=out_dram[t * P:(t + 1) * P, :], in_=hof[:])
                else:
                    for t in range(nt):
                        hof = smp.tile([P, D], F32, name="hof", tag="t1")
                        nc.vector.tensor_copy(out=hof[:],
                                              in_=h_cur[:, t * D:(t + 1) * D])
                        nc.sync.dma_start(out=out_dram[t * P:(t + 1) * P, :],
                                          in_=hof[:])

    nc.compile()
    _nc_cache[key] = nc
    return nc


def _host_prep(x, src, dst, Wq, bq, Wk, bk, Wv, bv, Ws, bs, Wl, bl, Wr,
               gamma, beta, alpha_res):
    n, d = x.shape
    n_pad = ((n + NC * P - 1) // (NC * P)) * (NC * P)
    sh = n_pad // NC
    nt = sh // P
    n_tiles = n_pad // P

    order = np.argsort(dst, kind="stable")
    src_s, dst_s = src[order], dst[order]
    tile_of = dst_s // P
    counts = np.bincount(tile_of, minlength=n_tiles)
    starts = np.concatenate([[0], np.cumsum(counts)])
    s_all = np.maximum(1, (counts + P - 1) // P).astype(np.int64)
    s_pos = s_all.reshape(NC, nt).max(axis=0)
    s_list = tuple(int(v) for v in s_pos)
    soff = np.concatenate([[0], np.cumsum(s_pos)])
    stot = int(soff[-1])

    deg = np.bincount(dst, minlength=n_pad).astype(np.float32)
    invdeg_full = 1.0 / np.maximum(deg, 1.0)

    al = 1.0 / (1.0 + np.exp(-alpha_res))
    oma = float(1.0 - al)
    bn_scale = 1.0 / np.sqrt(1.0 + BN_EPS)
    scale = 1.0 / np.sqrt(float(d))

    x_pad = np.zeros((n_pad, D), np.float32)
    x_pad[:n] = x
    xT = x_pad.T.astype(NP_BF16)

    # fold the BN gain into the SAGE weights: z*Gx == mean@(Wl*Gx) + h@(Wr*Gx)
    GxF = [al * bn_scale * gamma[i] for i in range(L)]
    Wlg = [Wl[i] * GxF[i][None, :] for i in range(L)]
    Wrg = [Wr[i] * GxF[i][None, :] for i in range(L)]
    weights = [Wq, Wk, Wv, Ws, Wlg[0], Wrg[0], Wlg[1], Wrg[1], Wlg[2], Wrg[2]]
    wpack = np.zeros((P, (10 * DJ + 4) * D), NP_BF16)
    for w, W in enumerate(weights):
        for j in range(DJ):
            wpack[:, (w * DJ + j) * D:(w * DJ + j + 1) * D] = \
                W[j * P:(j + 1) * P, :].astype(NP_BF16)
    # bias rows (read from partition 0 only): bs, Bx0..Bx2
    BxF = [al * (bl[i] * bn_scale * gamma[i] + beta[i]) for i in range(L)]
    for bi, vec in enumerate([bs] + BxF):
        wpack[0, (10 * DJ + bi) * D:(10 * DJ + bi + 1) * D] = vec.astype(NP_BF16)

    Gx = [al * bn_scale * gamma[i] for i in range(L)]
    Bx = [al * (bl[i] * bn_scale * gamma[i] + beta[i]) for i in range(L)]
    vecs = [bk, bv, bs, Gx[0], Bx[0], Gx[1], Bx[1], Gx[2], Bx[2]]
    vpack = np.empty((P, 9 * D + DJ), np.float32)
    for k, v in enumerate(vecs):
        vpack[:, k * D:(k + 1) * D] = np.tile(v[None, :], (P, 1))
    for j in range(DJ):
        vpack[:, 9 * D + j] = bq[j * P:(j + 1) * P]

    # dense mean-normalized adjacency in DoubleRow block layout, per core:
    # a[k, ((c2*nt + t)*2 + i)*P + m] = mult(src=256*c2+128*i+k -> dst) / deg
    NC2 = (n_pad // P) // 2
    NBLK = NC2 * nt
    kk = (src_s % P).astype(np.int64)
    ii = (src_s // P) % 2
    c2 = src_s // (2 * P)
    w_e = np.ones(len(dst_s), np.float32)  # integer counts; exact in fp8

    in_maps = []
    for r in range(NC):
        idx_arr = np.zeros((P, stot * 8), np.int16)
        dst_arr = np.full((P, stot), 128.0, np.float32)
        for tloc in range(nt):
            g = r * nt + tloc
            e0, e1 = starts[g], starts[g + 1]
            cnt = e1 - e0
            st_ = int(s_pos[tloc])
            et_t = st_ * P
            srcs = np.zeros(et_t, np.int64)
            srcs[:cnt] = src_s[e0:e1]
            dl = np.full(et_t, 128, np.int64)
            dl[:cnt] = dst_s[e0:e1] - g * P
            o = int(soff[tloc])
            idx_arr[:, o * 8:(o + st_) * 8] = _wrap_idx(srcs)
            dst_arr[:, o:o + st_] = dl.reshape(st_, P).T

        e0, e1 = starts[r * nt], starts[(r + 1) * nt]
        tl = (dst_s[e0:e1] - r * sh) // P
        mm = (dst_s[e0:e1] - r * sh) % P
        # layout [c2][i][t*P + m]
        flat = (c2[e0:e1] * 2 + ii[e0:e1]) * (nt * P) + tl * P + mm
        a_arr = np.zeros((P, NBLK * 2 * P), np.float32)
        np.add.at(a_arr, (kk[e0:e1], flat), w_e[e0:e1])
        a_arr = a_arr.astype(NP_FP8)
        invdeg_r = invdeg_full[r * sh:(r + 1) * sh].reshape(nt, P).T.copy()

        xt_r = np.empty((P, DJ * sh), NP_BF16)
        for j in range(DJ):
            xt_r[:, j * sh:(j + 1) * sh] = xT[j * P:(j + 1) * P, r * sh:(r + 1) * sh]

        in_maps.append({
            "xt_in": xt_r,
            "wpack_in": wpack,
            "vpack_in": vpack,
            "idx_in": idx_arr,
            "dst_in": dst_arr,
            "invdeg_in": np.ascontiguousarray(invdeg_r),
            "a_in": a_arr,
        })
    return in_maps, (n_pad, sh, nt, s_list, scale, oma)


def kernel(**inputs):
    x = np.asarray(inputs["x"], np.float32)
    edge_index = np.asarray(inputs["edge_index"])
    args = dict(
        Wq=np.asarray(inputs["Wq"], np.float32), bq=np.asarray(inputs["bq"], np.float32),
        Wk=np.asarray(inputs["Wk"], np.float32), bk=np.asarray(inputs["bk"], np.float32),
        Wv=np.asarray(inputs["Wv"], np.float32), bv=np.asarray(inputs["bv"], np.float32),
        Ws=np.asarray(inputs["Ws"], np.float32), bs=np.asarray(inputs["bs"], np.float32),
        Wl=np.asarray(inputs["Wl"], np.float32), bl=np.asarray(inputs["bl"], np.float32),
        Wr=np.asarray(inputs["Wr"], np.float32),
        gamma=np.asarray(inputs["gamma"], np.float32),
        beta=np.asarray(inputs["beta"], np.float32),
        alpha_res=float(np.asarray(inputs["alpha_res"])),
    )
    src = edge_index[0].astype(np.int64)
    dst = edge_index[1].astype(np.int64)

    in_maps, params = _host_prep(x, src, dst, **args)
    t0 = time.time()
    nc = build_nc(*params)
    print(f"[kernel] build+compile {time.time()-t0:.1f}s", flush=True)
    t0 = time.time()
    res = run_bass_kernel_spmd(nc, in_maps, core_ids=list(range(NC)))
    print(f"[kernel] run {time.time()-t0:.1f}s", flush=True)
    out = np.concatenate([res.results[r]["out"] for r in range(NC)], axis=0)
    return out[:x.shape[0]]


# revision 55
# speedup vs baseline: 1.2607x; 1.0277x over previous
"""Trainium2 Bass kernel for nn_MixGNN (TransformerConv + 3x SAGEConv + BN + gated residual).

Strategy (8 NeuronCores, dst-node sharding):
  - Pad N 10000 -> 10240; core r owns 1280 dst nodes = 10 tiles of 128.
  - TransformerConv: per-edge gather machinery — dma_gather of source k rows
    (transposed, bf16) + v rows; attention scores as KgT.T @ qT on PE;
    softmax without max-subtraction; exp-weighted indicator matmuls on PE.
  - SAGEConv x3: dense fp8 DoubleRow aggregation. Host precomputes the
    mean-normalized adjacency A[src, dst] = mult/deg (fp8e4) in PE DoubleRow
    block layout; each pass computes meanT[d, dst] = sum_src H[src, d] *
    A[src, dst] as 256-src-deep fp8 matmuls at 0.5 cyc/row. The full H table
    (fp8, tile-major) lives in SBUF, refreshed per pass via AllGather.
  - Halo exchange: AllGather of bf16 k/v tables (transformer) and fp8 packed
    h tables (SAGE) through shared DRAM.
Output: fp32 [10000, 256].
"""
import os
import sys
import time

import numpy as np

for _p in ("/opt/trn_rl_repo",):
    if _p not in sys.path:
        sys.path.insert(0, _p)

import ml_dtypes  # noqa: E402
import concourse.bacc as bacc  # noqa: E402
import concourse.mybir as mybir  # noqa: E402
import concourse.tile as tile  # noqa: E402
from concourse.bass_utils import run_bass_kernel_spmd  # noqa: E402

P = 128
D = 256
DJ = D // P           # 2 d-chunks of 128
NC = 8                # cores
L = 3                 # SAGE layers
BN_EPS = 1e-5

F32 = mybir.dt.float32
BF16 = mybir.dt.bfloat16
FP8 = mybir.dt.float8e4
I16 = mybir.dt.int16
NP_FP8 = ml_dtypes.float8_e4m3
NP_BF16 = ml_dtypes.bfloat16

_nc_cache = {}


def _wrap_idx(a):
    """[S*128] int array -> [128, S*8] int16 wrapped gather-index layout."""
    w16 = a.reshape(-1, 16).T.astype(np.int16)   # [16, S*8]
    return np.tile(w16, (8, 1))                  # replicate to 8 Q7 stripes


def build_nc(n_pad, sh, nt, s_list, scale, oma):
    stages = int(os.environ.get("KSTAGES", "5"))
    nocc = os.environ.get("KNOCC") == "1"
    ksm = int(os.environ.get("KSM", "4"))
    kgp = int(os.environ.get("KGP", "2"))
    kpsc = int(os.environ.get("KPSC", "3"))
    kpagg = int(os.environ.get("KPAGG", "2"))
    kpmm = int(os.environ.get("KPMM", "2"))
    kptr = int(os.environ.get("KPTR", "1"))
    khalft = int(os.environ.get("KHALFT", "2"))  # transformer v-gather splits
    kapc = int(os.environ.get("KAPC", "8"))      # A-table late-load pieces
    kae = int(os.environ.get("KAEARLY", "12"))   # A c2-blocks loaded early
    key = (n_pad, sh, nt, s_list, round(scale, 9), round(oma, 9), stages,
           nocc, ksm, kgp, kpsc, kpagg, kpmm, kptr, khalft, kapc, kae,
           os.environ.get("KKGT"), os.environ.get("KVG"))
    if key in _nc_cache:
        return _nc_cache[key]

    S = max(s_list)
    soff = [0]
    for st_ in s_list:
        soff.append(soff[-1] + st_)
    stot = soff[-1]

    NCH = n_pad // P          # 80 source chunks of 128
    NC2 = NCH // 2            # 40 double-chunks of 256
    NBLK = NC2 * nt           # 400 A-blocks per core

    ndev = 1 if nocc else NC
    nc = bacc.Bacc("TRN2", target_bir_lowering=False, debug=False, num_devices=ndev)

    xt_in = nc.dram_tensor("xt_in", [P, DJ * sh], BF16, kind="ExternalInput")
    wpack_in = nc.dram_tensor("wpack_in", [P, (10 * DJ + 4) * D], BF16,
                              kind="ExternalInput")
    vpack_in = nc.dram_tensor("vpack_in", [P, 9 * D + DJ], F32, kind="ExternalInput")
    idx_in = nc.dram_tensor("idx_in", [P, stot * 8], I16, kind="ExternalInput")
    dst_in = nc.dram_tensor("dst_in", [P, stot], F32, kind="ExternalInput")
    invdeg_in = nc.dram_tensor("invdeg_in", [P, nt], F32, kind="ExternalInput")
    a_in = nc.dram_tensor("a_in", [P, NBLK * 2 * P], FP8, kind="ExternalInput")
    out_dram = nc.dram_tensor("out", [sh, D], F32, kind="ExternalOutput")

    WQ, WK, WV, WS = 0, 1, 2, 3
    WL = [4, 6, 8]
    WR = [5, 7, 9]
    WBIAS = 10 * DJ          # 4 bias rows (bs, Bx0..2) in wpack row 0
    VBK, VBV, VBS = 0, 1, 2

    with tile.TileContext(nc) as tc:
        with (
            tc.tile_pool(name="cst", bufs=1) as cst,
            tc.tile_pool(name="sb", bufs=1) as sb,
            tc.tile_pool(name="sm", bufs=ksm) as smp,
            tc.tile_pool(name="ps", bufs=2, space="PSUM") as ps,
            tc.tile_pool(name="dr", bufs=1, space="DRAM") as dr,
        ):
            # ---------------- constants / inputs to SBUF ----------------
            xt = cst.tile([P, DJ * sh], BF16)
            for _xi in range(2):
                _c0 = _xi * (DJ * sh // 2)
                _c1 = (_xi + 1) * (DJ * sh // 2)
                nc.sync.dma_start(out=xt[:, _c0:_c1], in_=xt_in[:, _c0:_c1])
            wp = cst.tile([P, (10 * DJ + 4) * D], BF16)
            nc.sync.dma_start(out=wp[:], in_=wpack_in[:])
            vp = cst.tile([P, 9 * D + DJ], F32)
            nc.sync.dma_start(out=vp[:], in_=vpack_in[:])
            invd = cst.tile([P, nt], F32)
            nc.sync.dma_start(out=invd[:], in_=invdeg_in[:])

            iota_i = cst.tile([P, P], mybir.dt.int32)
            nc.gpsimd.iota(iota_i[:], pattern=[[1, P]], base=0, channel_multiplier=0)
            ones_v = cst.tile([P, 1], BF16)
            nc.vector.memset(ones_v[:], 1.0)
            iota_part = cst.tile([P, 1], mybir.dt.int32)
            nc.gpsimd.iota(iota_part[:], pattern=[[1, 1]], base=0, channel_multiplier=1)
            iota_part_f = cst.tile([P, 1], F32)
            nc.vector.tensor_copy(out=iota_part_f[:], in_=iota_part[:])
            iota_f = cst.tile([P, P], F32)
            nc.vector.tensor_copy(out=iota_f[:], in_=iota_i[:])
            ident = cst.tile([P, P], F32)
            nc.vector.tensor_scalar(
                out=ident[:], in0=iota_f[:], scalar1=iota_part_f[:, :1], scalar2=None,
                op0=mybir.AluOpType.is_equal,
            )
            ident_b = cst.tile([P, P], BF16)
            nc.vector.tensor_copy(out=ident_b[:], in_=ident[:])
            iota_b = cst.tile([P, P], BF16)
            nc.vector.tensor_copy(out=iota_b[:], in_=iota_f[:])

            def wslice(w, j):
                return wp[:, (w * DJ + j) * D:(w * DJ + j + 1) * D]

            def vslice(k):
                return vp[:, k * D:(k + 1) * D]

            def xtile(j, t):
                return xt[:, j * sh + t * P: j * sh + (t + 1) * P]

            # ---------------- long-lived SBUF state ----------------
            qT = [sb.tile([P, sh], BF16, name=f"qT_{j}") for j in range(DJ)]
            h_cur = sb.tile([P, nt * D], BF16)
            h_nxt = sb.tile([P, nt * D], BF16)
            hT_cur = sb.tile([P, DJ * sh], BF16)
            hT_nxt = sb.tile([P, DJ * sh], BF16)
            h_sb = sb.tile([P, NCH, D], FP8)     # full H table, tile-major
            meanT_all = sb.tile([P, nt, D], BF16)  # paggT staging per pass

            ones_row = cst.tile([1, P], BF16)
            nc.vector.memset(ones_row[:], 1.0)

            # ---------------- DRAM tables ----------------
            k_ag_in = dr.tile([sh, D], BF16)
            v_ag_in = dr.tile([sh, D], BF16)
            k_full = dr.tile([n_pad, D], BF16, addr_space="Shared")
            v_full = dr.tile([n_pad, D], BF16, addr_space="Shared")
            hag_in = [dr.tile([P, nt * D], FP8, name=f"hag_in_{i}") for i in range(L)]
            h_full = [dr.tile([NC * P, nt * D], FP8, name=f"h_full_{i}",
                              addr_space="Shared") for i in range(L)]

            def allgather(in_t, out_t):
                if nocc:
                    nc.sync.dma_start(out=out_t[:in_t.shape[0]], in_=in_t[:])
                else:
                    nc.gpsimd.collective_compute(
                        "AllGather", mybir.AluOpType.bypass,
                        replica_groups=[list(range(NC))],
                        ins=[in_t[:]], outs=[out_t[:]],
                    )

            # ---------------- stage 0: k,v shard tables + AG, then qT --------
            for t in range(nt):
                pk = ps.tile([P, D], F32, name="pk", tag="pmm", bufs=kpmm)
                for ji in range(DJ):
                    nc.tensor.matmul(pk[:], lhsT=xtile(ji, t), rhs=wslice(WK, ji),
                                     start=(ji == 0), stop=(ji == DJ - 1))
                k_sb = smp.tile([P, D], BF16, name="k_sb")
                nc.vector.tensor_tensor(out=k_sb[:], in0=pk[:], in1=vslice(VBK),
                                        op=mybir.AluOpType.add)
                nc.sync.dma_start(out=k_ag_in[t * P:(t + 1) * P, :], in_=k_sb[:])
            allgather(k_ag_in, k_full)

            for t in range(nt):
                pv = ps.tile([P, D], F32, name="pv", tag="pmm", bufs=kpmm)
                for ji in range(DJ):
                    nc.tensor.matmul(pv[:], lhsT=xtile(ji, t), rhs=wslice(WV, ji),
                                     start=(ji == 0), stop=(ji == DJ - 1))
                v_sb = smp.tile([P, D], BF16, name="v_sb")
                nc.vector.tensor_tensor(out=v_sb[:], in0=pv[:], in1=vslice(VBV),
                                        op=mybir.AluOpType.add)
                nc.sync.dma_start(out=v_ag_in[t * P:(t + 1) * P, :], in_=v_sb[:])
            allgather(v_ag_in, v_full)

            for j in range(DJ):
                n0 = 0
                while n0 < sh:
                    nn = min(512, sh - n0)
                    pq = ps.tile([P, 512], F32, name="pq", tag="pmm", bufs=kpmm)
                    for ji in range(DJ):
                        nc.tensor.matmul(
                            pq[:, :nn],
                            lhsT=wslice(WQ, ji)[:, j * P:(j + 1) * P],
                            rhs=xt[:, ji * sh + n0: ji * sh + n0 + nn],
                            start=(ji == 0), stop=(ji == DJ - 1),
                        )
                    nc.vector.tensor_scalar(
                        out=qT[j][:, n0:n0 + nn], in0=pq[:, :nn],
                        scalar1=vp[:, 9 * D + j: 9 * D + j + 1], scalar2=None,
                        op0=mybir.AluOpType.add,
                    )
                    n0 += nn

            # ---------------- transformer pass (gather-based) ----------------
            def transformer_pass(gp, pst, h_out, hT_out, tile_hook):
                dstc = gp.tile([P, stot], F32, name="dstc", bufs=1)
                nc.sync.dma_start(out=dstc[:], in_=dst_in[:])
                idx_sb = gp.tile([P, stot * 8], I16, name="idx_sb", bufs=1)
                nc.sync.dma_start(out=idx_sb[:], in_=idx_in[:])
                for t in range(nt):
                    st = s_list[t]
                    et_t = st * P
                    splits = []
                    base = (st + khalft - 1) // khalft
                    c0 = 0
                    while c0 < st:
                        splits.append((c0, min(st, c0 + base)))
                        c0 += base
                    kgt = gp.tile([P, DJ, et_t], BF16, name="kgt", tag="kgt",
                                  bufs=int(os.environ.get("KKGT", "2")))
                    vg = gp.tile([P, S, D], BF16, name="vg", tag="vg",
                                 bufs=int(os.environ.get("KVG", "2")))
                    idx_tt = idx_sb[:, soff[t] * 8:(soff[t] + st) * 8]
                    nc.gpsimd.dma_gather(
                        out_ap=kgt[:], in_ap=k_full[:], idxs_ap=idx_tt,
                        num_idxs=et_t, num_idxs_reg=et_t, elem_size=D,
                        transpose=True, single_packet=False)
                    for (ca, cb) in splits:
                        nn_i = (cb - ca) * P
                        idx_t = idx_sb[:, (soff[t] + ca) * 8: (soff[t] + cb) * 8]
                        nc.gpsimd.dma_gather(
                            out_ap=vg[:, ca:cb, :], in_ap=v_full[:], idxs_ap=idx_t,
                            num_idxs=nn_i, num_idxs_reg=nn_i, elem_size=D,
                            single_packet=False)

                    pagg = pst.tile([P, D + 1], F32, name="pagg", tag="pagg",
                                    bufs=kpagg)
                    for c in range(st):
                        dcol = dstc[:, soff[t] + c: soff[t] + c + 1]
                        psc = pst.tile([P, P], F32, name="psc", tag="psc", bufs=kpsc)
                        for j in range(DJ):
                            nc.tensor.matmul(
                                psc[:],
                                lhsT=kgt[:, j, c * P:(c + 1) * P],
                                rhs=qT[j][:, t * P:(t + 1) * P],
                                start=(j == 0), stop=(j == DJ - 1))
                        exps = smp.tile([P, P], BF16, name="exps")
                        nc.scalar.activation(exps[:], psc[:],
                                             mybir.ActivationFunctionType.Exp,
                                             scale=scale)
                        w_b = smp.tile([P, P], BF16, name="w_b", tag="w_b")
                        nc.vector.scalar_tensor_tensor(
                            out=w_b[:], in0=iota_b[:], scalar=dcol, in1=exps[:],
                            op0=mybir.AluOpType.is_equal,
                            op1=mybir.AluOpType.mult)
                        nc.tensor.matmul(pagg[:, :D], lhsT=w_b[:], rhs=vg[:, c, :],
                                         start=(c == 0), stop=(c == st - 1))
                        nc.tensor.matmul(pagg[:, D:D + 1], lhsT=w_b[:],
                                         rhs=ones_v[:],
                                         start=False, stop=(c == st - 1))

                    smax = smp.tile([P, 1], F32, name="smax")
                    nc.vector.tensor_scalar(
                        out=smax[:], in0=pagg[:, D:D + 1], scalar1=1e-30,
                        scalar2=None, op0=mybir.AluOpType.max)
                    rs = smp.tile([P, 1], F32, name="rs")
                    nc.vector.reciprocal(rs[:], smax[:])
                    pskip = ps.tile([P, D], F32, name="pskip", tag="pmm", bufs=kpmm)
                    for ji in range(DJ):
                        nc.tensor.matmul(pskip[:], lhsT=xtile(ji, t),
                                         rhs=wslice(WS, ji),
                                         start=(ji == 0), stop=False)
                    nc.tensor.matmul(pskip[:], lhsT=ones_row[:],
                                     rhs=wp[:1, WBIAS * D:(WBIAS + 1) * D],
                                     start=False, stop=True)
                    t1 = smp.tile([P, D], F32, name="t1", tag="t1")
                    nc.scalar.activation(t1[:], pagg[:, :D],
                                         mybir.ActivationFunctionType.Copy,
                                         scale=rs[:, :1])
                    t2 = smp.tile([P, D], F32, name="t2", tag="t2")
                    nc.vector.tensor_tensor(out=t2[:], in0=t1[:], in1=pskip[:],
                                            op=mybir.AluOpType.add)
                    nc.scalar.activation(h_out[:, t * D:(t + 1) * D], t2[:],
                                         mybir.ActivationFunctionType.Relu)
                    epilogue_store(0, t, h_out, hT_out)
                    tile_hook(t)
                allgather(hag_in[0], h_full[0])

            def epilogue_store(li, t, h_out, hT_out):
                """Write fp8 pack row + transposed shard for layer output li."""
                hstage = smp.tile([P, D], FP8, name="hstage")
                nc.vector.tensor_copy(out=hstage[:],
                                      in_=h_out[:, t * D:(t + 1) * D])
                nc.sync.dma_start(out=hag_in[li][:, t * D:(t + 1) * D],
                                  in_=hstage[:])
                for j in range(DJ):
                    ptr2 = ps.tile([P, P], BF16, name="ptr2", tag="ptr", bufs=kptr)
                    nc.tensor.transpose(
                        out=ptr2[:],
                        in_=h_out[:, t * D + j * P: t * D + (j + 1) * P],
                        identity=ident_b[:])
                    nc.vector.tensor_copy(
                        out=hT_out[:, j * sh + t * P: j * sh + (t + 1) * P],
                        in_=ptr2[:])

            # ---------------- SAGE pass (dense fp8 DoubleRow) ----------------
            # dst tiles processed in groups of 4; one [P, 512] PSUM bank per
            # (group, dh), double-buffered so group g+1's adjacency sweep
            # overlaps group g's epilogue.
            def sage_pass(layer, pd, a_rhs, h_prev, hT_prev, h_out, hT_out):
                li = layer + 1
                # load H table (tile-major fp8) from the AllGathered pack
                for r in range(NC):
                    nc.sync.dma_start(
                        out=h_sb[:, r * nt:(r + 1) * nt, :],
                        in_=h_full[layer][r * P:(r + 1) * P, :])
                groups = [(g * 4, min(nt, (g + 1) * 4)) for g in range((nt + 3) // 4)]

                def sage_epilogue(t):
                    pzm = pd.tile([P, D], F32, name="pzm", tag="pzm", bufs=1)
                    for j in range(DJ):
                        nc.tensor.matmul(pzm[:],
                                         lhsT=meanT_all[:, t, j * P:(j + 1) * P],
                                         rhs=wslice(WL[layer], j),
                                         start=(j == 0), stop=(j == DJ - 1))
                    pz = ps.tile([P, D], F32, name="pz", tag="pmm", bufs=kpmm)
                    for j in range(DJ):
                        nc.tensor.matmul(
                            pz[:],
                            lhsT=hT_prev[:, j * sh + t * P: j * sh + (t + 1) * P],
                            rhs=wslice(WR[layer], j),
                            start=(j == 0), stop=False)
                    nc.tensor.matmul(
                        pz[:], lhsT=ones_row[:],
                        rhs=wp[:1, (WBIAS + 1 + layer) * D:(WBIAS + 2 + layer) * D],
                        start=False, stop=True)
                    t0 = smp.tile([P, D], F32, name="t0s", tag="t0")
                    nc.scalar.activation(t0[:], pzm[:],
                                         mybir.ActivationFunctionType.Copy,
                                         scale=invd[:, t:t + 1])
                    t1 = smp.tile([P, D], F32, name="t1s", tag="t2")
                    nc.vector.tensor_tensor(out=t1[:], in0=t0[:], in1=pz[:],
                                            op=mybir.AluOpType.add)
                    t3 = smp.tile([P, D], F32, name="t3s", tag="t4")
                    nc.vector.scalar_tensor_tensor(
                        out=t3[:], in0=h_prev[:, t * D:(t + 1) * D], scalar=oma,
                        in1=t1[:], op0=mybir.AluOpType.mult,
                        op1=mybir.AluOpType.add)
                    if layer < L - 1:
                        nc.scalar.activation(h_out[:, t * D:(t + 1) * D], t3[:],
                                             mybir.ActivationFunctionType.Relu)
                        epilogue_store(li, t, h_out, hT_out)
                    else:
                        hof = smp.tile([P, D], F32, name="hof", tag="t1")
                        nc.scalar.activation(hof[:], t3[:],
                                             mybir.ActivationFunctionType.Relu)
                        nc.sync.dma_start(out=out_dram[t * P:(t + 1) * P, :],
                                          in_=hof[:])

                for gi, (ta, tb) in enumerate(groups):
                    tw = (tb - ta) * P
                    pb = [pd.tile([P, 4 * P], F32, name=f"pdb_{gi % 2}_{dh}",
                                  tag=f"pdb_{gi % 2}_{dh}", bufs=1)
                          for dh in range(DJ)]
                    for dh in range(DJ):
                        for c2 in range(NC2):
                            nc.tensor.matmul(
                                pb[dh][:, :tw],
                                lhsT=h_sb[:, 2 * c2:2 * c2 + 2, dh * P:(dh + 1) * P],
                                rhs=a_rhs(c2, ta, tb),
                                start=(c2 == 0), stop=(c2 == NC2 - 1),
                                perf_mode=mybir.MatmulPerfMode.DoubleRow)
                        nc.vector.tensor_copy(
                            out=meanT_all[:, ta:tb, dh * P:(dh + 1) * P],
                            in_=pb[dh][:, :tw])
                    for t in range(ta, tb):
                        sage_epilogue(t)
                if layer < L - 1:
                    allgather(hag_in[li], h_full[li])

            if stages <= 1:
                tmpo = smp.tile([P, D], F32, name="tmpo")
                for t in range(nt):
                    nc.vector.tensor_copy(out=tmpo[:], in_=xt[:, :D])
                    nc.sync.dma_start(out=out_dram[t * P:(t + 1) * P, :],
                                      in_=tmpo[:])
            else:
                with tc.tile_pool(name="ae", bufs=1) as ape:
                    a_early = None
                    if stages >= 3 and kae > 0:
                        a_early = ape.tile([P, kae, 2, nt * P], FP8)

                    def tile_hook(t):
                        # trickle early A-table c2-blocks through transformer
                        if a_early is not None and t < kae:
                            nc.scalar.dma_start(
                                out=a_early[:, t, :, :],
                                in_=a_in[:, t * 2 * nt * P:(t + 1) * 2 * nt * P])

                    with (
                        tc.tile_pool(name="g", bufs=kgp) as gp,
                        tc.tile_pool(name="pst", bufs=1, space="PSUM") as pst,
                    ):
                        transformer_pass(gp, pst, h_cur, hT_cur, tile_hook)
                    if stages >= 3:
                        with (
                            tc.tile_pool(name="ap", bufs=1) as app,
                            tc.tile_pool(name="pd", bufs=1, space="PSUM") as pd,
                        ):
                            nlate = NC2 - kae
                            a_late = app.tile([P, max(1, nlate), 2, nt * P], FP8)
                            cw = (nlate + kapc - 1) // kapc
                            for p_ in range(kapc):
                                c0 = min(nlate, p_ * cw)
                                c1 = min(nlate, (p_ + 1) * cw)
                                if c0 >= c1:
                                    continue
                                # ACT-issued so H loads (SP queue) interleave
                                nc.scalar.dma_start(
                                    out=a_late[:, c0:c1, :, :],
                                    in_=a_in[:, (kae + c0) * 2 * nt * P:
                                             (kae + c1) * 2 * nt * P])

                            def a_rhs(c2, ta, tb):
                                if c2 < kae:
                                    return a_early[:, c2, :, ta * P:tb * P]
                                return a_late[:, c2 - kae, :, ta * P:tb * P]

                            bufs = [(h_cur, hT_cur), (h_nxt, hT_nxt)]
                            for i in range(L):
                                h_prev, hT_prev = bufs[i % 2]
                                h_out, hT_out = bufs[(i + 1) % 2]
                                sage_pass(i, pd, a_rhs, h_prev, hT_prev,
                                          h_out, hT_out)
                    else:
                        for t in range(nt):
                            hof = smp.tile([P, D], F32, name="hof", tag="t1")
                            nc.vector.tensor_copy(out=hof[:],
                                                  in_=h_cur[:, t * D:(t + 1) * D])
                            nc.sync.dma_start(out=out_dram[t * P:(t + 1) * P, :],
                                              in_=hof[:])

    nc.compile()
    _nc_cache[key] = nc
    return nc


def _host_prep(x, src, dst, Wq, bq, Wk, bk, Wv, bv, Ws, bs, Wl, bl, Wr,
               gamma, beta, alpha_res):
    n, d = x.shape
    n_pad = ((n + NC * P - 1) // (NC * P)) * (NC * P)
    sh = n_pad // NC
    nt = sh // P
    n_tiles = n_pad // P

    order = np.argsort(dst, kind="stable")
    src_s, dst_s = src[order], dst[order]
    tile_of = dst_s // P
    counts = np.bincount(tile_of, minlength=n_tiles)
    starts = np.concatenate([[0], np.cumsum(counts)])
    s_all = np.maximum(1, (counts + P - 1) // P).astype(np.int64)
    s_pos = s_all.reshape(NC, nt).max(axis=0)
    s_list = tuple(int(v) for v in s_pos)
    soff = np.concatenate([[0], np.cumsum(s_pos)])
    stot = int(soff[-1])

    deg = np.bincount(dst, minlength=n_pad).astype(np.float32)
    invdeg_full = 1.0 / np.maximum(deg, 1.0)

    al = 1.0 / (1.0 + np.exp(-alpha_res))
    oma = float(1.0 - al)
    bn_scale = 1.0 / np.sqrt(1.0 + BN_EPS)
    scale = 1.0 / np.sqrt(float(d))

    x_pad = np.zeros((n_pad, D), np.float32)
    x_pad[:n] = x
    xT = x_pad.T.astype(NP_BF16)

    # fold the BN gain into the SAGE weights: z*Gx == mean@(Wl*Gx) + h@(Wr*Gx)
    GxF = [al * bn_scale * gamma[i] for i in range(L)]
    Wlg = [Wl[i] * GxF[i][None, :] for i in range(L)]
    Wrg = [Wr[i] * GxF[i][None, :] for i in range(L)]
    weights = [Wq, Wk, Wv, Ws, Wlg[0], Wrg[0], Wlg[1], Wrg[1], Wlg[2], Wrg[2]]
    wpack = np.zeros((P, (10 * DJ + 4) * D), NP_BF16)
    for w, W in enumerate(weights):
        for j in range(DJ):
            wpack[:, (w * DJ + j) * D:(w * DJ + j + 1) * D] = \
                W[j * P:(j + 1) * P, :].astype(NP_BF16)
    # bias rows (read from partition 0 only): bs, Bx0..Bx2
    BxF = [al * (bl[i] * bn_scale * gamma[i] + beta[i]) for i in range(L)]
    for bi, vec in enumerate([bs] + BxF):
        wpack[0, (10 * DJ + bi) * D:(10 * DJ + bi + 1) * D] = vec.astype(NP_BF16)

    Gx = [al * bn_scale * gamma[i] for i in range(L)]
    Bx = [al * (bl[i] * bn_scale * gamma[i] + beta[i]) for i in range(L)]
    vecs = [bk, bv, bs, Gx[0], Bx[0], Gx[1], Bx[1], Gx[2], Bx[2]]
    vpack = np.empty((P, 9 * D + DJ), np.float32)
    for k, v in enumerate(vecs):
        vpack[:, k * D:(k + 1) * D] = np.tile(v[None, :], (P, 1))
    for j in range(DJ):
        vpack[:, 9 * D + j] = bq[j * P:(j + 1) * P]

    # dense mean-normalized adjacency in DoubleRow block layout, per core:
    # a[k, ((c2*nt + t)*2 + i)*P + m] = mult(src=256*c2+128*i+k -> dst) / deg
    NC2 = (n_pad // P) // 2
    NBLK = NC2 * nt
    kk = (src_s % P).astype(np.int64)
    ii = (src_s // P) % 2
    c2 = src_s // (2 * P)
    w_e = np.ones(len(dst_s), np.float32)  # integer counts; exact in fp8

    in_maps = []
    for r in range(NC):
        idx_arr = np.zeros((P, stot * 8), np.int16)
        dst_arr = np.full((P, stot), 128.0, np.float32)
        for tloc in range(nt):
            g = r * nt + tloc
            e0, e1 = starts[g], starts[g + 1]
            cnt = e1 - e0
            st_ = int(s_pos[tloc])
            et_t = st_ * P
            srcs = np.zeros(et_t, np.int64)
            srcs[:cnt] = src_s[e0:e1]
            dl = np.full(et_t, 128, np.int64)
            dl[:cnt] = dst_s[e0:e1] - g * P
            o = int(soff[tloc])
            idx_arr[:, o * 8:(o + st_) * 8] = _wrap_idx(srcs)
            dst_arr[:, o:o + st_] = dl.reshape(st_, P).T

        e0, e1 = starts[r * nt], starts[(r + 1) * nt]
        tl = (dst_s[e0:e1] - r * sh) // P
        mm = (dst_s[e0:e1] - r * sh) % P
        # layout [c2][i][t*P + m]
        flat = (c2[e0:e1] * 2 + ii[e0:e1]) * (nt * P) + tl * P + mm
        a_arr = np.zeros((P, NBLK * 2 * P), np.float32)
        np.add.at(a_arr, (kk[e0:e1], flat), w_e[e0:e1])
        a_arr = a_arr.astype(NP_FP8)
        invdeg_r = invdeg_full[r * sh:(r + 1) * sh].reshape(nt, P).T.copy()

        xt_r = np.empty((P, DJ * sh), NP_BF16)
        for j in range(DJ):
            xt_r[:, j * sh:(j + 1) * sh] = xT[j * P:(j + 1) * P, r * sh:(r + 1) * sh]

        in_maps.append({
            "xt_in": xt_r,
            "wpack_in": wpack,
            "vpack_in": vpack,
            "idx_in": idx_arr,
            "dst_in": dst_arr,
            "invdeg_in": np.ascontiguousarray(invdeg_r),
            "a_in": a_arr,
        })
    return in_maps, (n_pad, sh, nt, s_list, scale, oma)


def kernel(**inputs):
    x = np.asarray(inputs["x"], np.float32)
    edge_index = np.asarray(inputs["edge_index"])
    args = dict(
        Wq=np.asarray(inputs["Wq"], np.float32), bq=np.asarray(inputs["bq"], np.float32),
        Wk=np.asarray(inputs["Wk"], np.float32), bk=np.asarray(inputs["bk"], np.float32),
        Wv=np.asarray(inputs["Wv"], np.float32), bv=np.asarray(inputs["bv"], np.float32),
        Ws=np.asarray(inputs["Ws"], np.float32), bs=np.asarray(inputs["bs"], np.float32),
        Wl=np.asarray(inputs["Wl"], np.float32), bl=np.asarray(inputs["bl"], np.float32),
        Wr=np.asarray(inputs["Wr"], np.float32),
        gamma=np.asarray(inputs["gamma"], np.float32),
        beta=np.asarray(inputs["beta"], np.float32),
        alpha_res=float(np.asarray(inputs["alpha_res"])),
    )
    src = edge_index[0].astype(np.int64)
    dst = edge_index[1].astype(np.int64)

    in_maps, params = _host_prep(x, src, dst, **args)
    t0 = time.time()
    nc = build_nc(*params)
    print(f"[kernel] build+compile {time.time()-t0:.1f}s", flush=True)
    t0 = time.time()
    res = run_bass_kernel_spmd(nc, in_maps, core_ids=list(range(NC)))
    print(f"[kernel] run {time.time()-t0:.1f}s", flush=True)
    out = np.concatenate([res.results[r]["out"] for r in range(NC)], axis=0)
    return out[:x.shape[0]]


# revision 57
# speedup vs baseline: 1.2654x; 1.0037x over previous
"""Trainium2 Bass kernel for nn_MixGNN (TransformerConv + 3x SAGEConv + BN + gated residual).

Strategy (8 NeuronCores, dst-node sharding; pad N 10000 -> 10240, core r owns
1280 dst nodes = 10 tiles of 128):
  - TransformerConv (per-edge gather): dma_gather of source k rows
    (transposed, bf16) + v rows; attention scores as KgT.T @ qT on PE;
    softmax without max-subtraction (logits are O(1)); per-chunk exp-weighted
    one-hot matmuls accumulate value sums + exp sums in PSUM.
  - SAGEConv x3 (dense fp8 DoubleRow — no gathers): host precomputes the
    integer-count adjacency A[src, dst] (exact in fp8e4) in PE DoubleRow
    block layout [c2][i][t*128+m]; each pass computes sumT[d, dst] =
    sum_src H[src, d] * A[src, dst] as 256-src-deep fp8 matmuls at 0.5
    cyc/row, batching 4 dst tiles per matmul (one [128,512] PSUM bank).
    1/deg is applied on the PSUM of sumT@Wl (per-dst-partition ACT scale);
    BN gain is folded into Wl/Wr, biases added via 1-partition matmuls.
    The full H table (fp8, tile-major) lives in SBUF, refreshed per pass
    via AllGather of fp8 shard packs. The 13 MB A table streams into SBUF
    partly during the transformer phase (small early pool) and the rest
    right after the gather pools release.
  - 5 collectives total: AllGather k, v (bf16), h0..h2 (fp8).
Output: fp32 [10000, 256].
"""
import os
import sys
import time

import numpy as np

for _p in ("/opt/trn_rl_repo",):
    if _p not in sys.path:
        sys.path.insert(0, _p)

import ml_dtypes  # noqa: E402
import concourse.bacc as bacc  # noqa: E402
import concourse.mybir as mybir  # noqa: E402
import concourse.tile as tile  # noqa: E402
from concourse.bass_utils import run_bass_kernel_spmd  # noqa: E402

P = 128
D = 256
DJ = D // P           # 2 d-chunks of 128
NC = 8                # cores
L = 3                 # SAGE layers
BN_EPS = 1e-5

F32 = mybir.dt.float32
BF16 = mybir.dt.bfloat16
FP8 = mybir.dt.float8e4
I16 = mybir.dt.int16
NP_FP8 = ml_dtypes.float8_e4m3
NP_BF16 = ml_dtypes.bfloat16

_nc_cache = {}


def _wrap_idx(a):
    """[S*128] int array -> [128, S*8] int16 wrapped gather-index layout."""
    w16 = a.reshape(-1, 16).T.astype(np.int16)   # [16, S*8]
    return np.tile(w16, (8, 1))                  # replicate to 8 Q7 stripes


def build_nc(n_pad, sh, nt, s_list, scale, oma):
    stages = int(os.environ.get("KSTAGES", "5"))
    nocc = os.environ.get("KNOCC") == "1"
    ksm = int(os.environ.get("KSM", "4"))
    kgp = int(os.environ.get("KGP", "2"))
    kpsc = int(os.environ.get("KPSC", "3"))
    kpagg = int(os.environ.get("KPAGG", "2"))
    kpmm = int(os.environ.get("KPMM", "2"))
    kptr = int(os.environ.get("KPTR", "1"))
    khalft = int(os.environ.get("KHALFT", "2"))  # transformer v-gather splits
    kapc = int(os.environ.get("KAPC", "8"))      # A-table late-load pieces
    # A c2-blocks loaded early; one is issued per transformer tile, so at
    # most nt blocks can be trickled in.
    kae = min(int(os.environ.get("KAEARLY", "10")), nt)
    key = (n_pad, sh, nt, s_list, round(scale, 9), round(oma, 9), stages,
           nocc, ksm, kgp, kpsc, kpagg, kpmm, kptr, khalft, kapc, kae,
           os.environ.get("KKGT"), os.environ.get("KVG"))
    if key in _nc_cache:
        return _nc_cache[key]

    S = max(s_list)
    soff = [0]
    for st_ in s_list:
        soff.append(soff[-1] + st_)
    stot = soff[-1]

    NCH = n_pad // P          # 80 source chunks of 128
    NC2 = NCH // 2            # 40 double-chunks of 256
    NBLK = NC2 * nt           # 400 A-blocks per core

    ndev = 1 if nocc else NC
    nc = bacc.Bacc("TRN2", target_bir_lowering=False, debug=False, num_devices=ndev)

    xt_in = nc.dram_tensor("xt_in", [P, DJ * sh], BF16, kind="ExternalInput")
    wpack_in = nc.dram_tensor("wpack_in", [P, (10 * DJ + 4) * D], BF16,
                              kind="ExternalInput")
    vpack_in = nc.dram_tensor("vpack_in", [P, 9 * D + DJ], F32, kind="ExternalInput")
    idx_in = nc.dram_tensor("idx_in", [P, stot * 8], I16, kind="ExternalInput")
    dst_in = nc.dram_tensor("dst_in", [P, stot], F32, kind="ExternalInput")
    invdeg_in = nc.dram_tensor("invdeg_in", [P, nt], F32, kind="ExternalInput")
    a_in = nc.dram_tensor("a_in", [P, NBLK * 2 * P], FP8, kind="ExternalInput")
    out_dram = nc.dram_tensor("out", [sh, D], F32, kind="ExternalOutput")

    WQ, WK, WV, WS = 0, 1, 2, 3
    WL = [4, 6, 8]
    WR = [5, 7, 9]
    WBIAS = 10 * DJ          # 4 bias rows (bs, Bx0..2) in wpack row 0
    VBK, VBV, VBS = 0, 1, 2

    with tile.TileContext(nc) as tc:
        with (
            tc.tile_pool(name="cst", bufs=1) as cst,
            tc.tile_pool(name="sb", bufs=1) as sb,
            tc.tile_pool(name="sm", bufs=ksm) as smp,
            tc.tile_pool(name="ps", bufs=2, space="PSUM") as ps,
            tc.tile_pool(name="dr", bufs=1, space="DRAM") as dr,
        ):
            # ---------------- constants / inputs to SBUF ----------------
            xt = cst.tile([P, DJ * sh], BF16)
            for _xi in range(2):
                _c0 = _xi * (DJ * sh // 2)
                _c1 = (_xi + 1) * (DJ * sh // 2)
                nc.sync.dma_start(out=xt[:, _c0:_c1], in_=xt_in[:, _c0:_c1])
            wp = cst.tile([P, (10 * DJ + 4) * D], BF16)
            nc.sync.dma_start(out=wp[:], in_=wpack_in[:])
            vp = cst.tile([P, 9 * D + DJ], F32)
            nc.sync.dma_start(out=vp[:], in_=vpack_in[:])
            invd = cst.tile([P, nt], F32)
            nc.sync.dma_start(out=invd[:], in_=invdeg_in[:])

            iota_i = cst.tile([P, P], mybir.dt.int32)
            nc.gpsimd.iota(iota_i[:], pattern=[[1, P]], base=0, channel_multiplier=0)
            ones_v = cst.tile([P, 1], BF16)
            nc.vector.memset(ones_v[:], 1.0)
            iota_part = cst.tile([P, 1], mybir.dt.int32)
            nc.gpsimd.iota(iota_part[:], pattern=[[1, 1]], base=0, channel_multiplier=1)
            iota_part_f = cst.tile([P, 1], F32)
            nc.vector.tensor_copy(out=iota_part_f[:], in_=iota_part[:])
            iota_f = cst.tile([P, P], F32)
            nc.vector.tensor_copy(out=iota_f[:], in_=iota_i[:])
            ident = cst.tile([P, P], F32)
            nc.vector.tensor_scalar(
                out=ident[:], in0=iota_f[:], scalar1=iota_part_f[:, :1], scalar2=None,
                op0=mybir.AluOpType.is_equal,
            )
            ident_b = cst.tile([P, P], BF16)
            nc.vector.tensor_copy(out=ident_b[:], in_=ident[:])
            iota_b = cst.tile([P, P], BF16)
            nc.vector.tensor_copy(out=iota_b[:], in_=iota_f[:])

            def wslice(w, j):
                return wp[:, (w * DJ + j) * D:(w * DJ + j + 1) * D]

            def vslice(k):
                return vp[:, k * D:(k + 1) * D]

            def xtile(j, t):
                return xt[:, j * sh + t * P: j * sh + (t + 1) * P]

            # ---------------- long-lived SBUF state ----------------
            qT = [sb.tile([P, sh], BF16, name=f"qT_{j}") for j in range(DJ)]
            h_cur = sb.tile([P, nt * D], BF16)
            h_nxt = sb.tile([P, nt * D], BF16)
            hT_cur = sb.tile([P, DJ * sh], BF16)
            hT_nxt = sb.tile([P, DJ * sh], BF16)
            h_sb = sb.tile([P, NCH, D], FP8)     # full H table, tile-major
            meanT_all = sb.tile([P, nt, D], BF16)  # paggT staging per pass

            ones_row = cst.tile([1, P], BF16)
            nc.vector.memset(ones_row[:], 1.0)

            # ---------------- DRAM tables ----------------
            k_ag_in = dr.tile([sh, D], BF16)
            v_ag_in = dr.tile([sh, D], BF16)
            k_full = dr.tile([n_pad, D], BF16, addr_space="Shared")
            v_full = dr.tile([n_pad, D], BF16, addr_space="Shared")
            hag_in = [dr.tile([P, nt * D], FP8, name=f"hag_in_{i}") for i in range(L)]
            h_full = [dr.tile([NC * P, nt * D], FP8, name=f"h_full_{i}",
                              addr_space="Shared") for i in range(L)]

            def allgather(in_t, out_t):
                if nocc:
                    nc.sync.dma_start(out=out_t[:in_t.shape[0]], in_=in_t[:])
                else:
                    nc.gpsimd.collective_compute(
                        "AllGather", mybir.AluOpType.bypass,
                        replica_groups=[list(range(NC))],
                        ins=[in_t[:]], outs=[out_t[:]],
                    )

            # ---------------- stage 0: k,v shard tables + AG, then qT --------
            for t in range(nt):
                pk = ps.tile([P, D], F32, name="pk", tag="pmm", bufs=kpmm)
                for ji in range(DJ):
                    nc.tensor.matmul(pk[:], lhsT=xtile(ji, t), rhs=wslice(WK, ji),
                                     start=(ji == 0), stop=(ji == DJ - 1))
                k_sb = smp.tile([P, D], BF16, name="k_sb")
                nc.vector.tensor_tensor(out=k_sb[:], in0=pk[:], in1=vslice(VBK),
                                        op=mybir.AluOpType.add)
                nc.sync.dma_start(out=k_ag_in[t * P:(t + 1) * P, :], in_=k_sb[:])
            allgather(k_ag_in, k_full)

            for t in range(nt):
                pv = ps.tile([P, D], F32, name="pv", tag="pmm", bufs=kpmm)
                for ji in range(DJ):
                    nc.tensor.matmul(pv[:], lhsT=xtile(ji, t), rhs=wslice(WV, ji),
                                     start=(ji == 0), stop=(ji == DJ - 1))
                v_sb = smp.tile([P, D], BF16, name="v_sb")
                nc.vector.tensor_tensor(out=v_sb[:], in0=pv[:], in1=vslice(VBV),
                                        op=mybir.AluOpType.add)
                nc.sync.dma_start(out=v_ag_in[t * P:(t + 1) * P, :], in_=v_sb[:])
            allgather(v_ag_in, v_full)

            for j in range(DJ):
                n0 = 0
                while n0 < sh:
                    nn = min(512, sh - n0)
                    pq = ps.tile([P, 512], F32, name="pq", tag="pmm", bufs=kpmm)
                    for ji in range(DJ):
                        nc.tensor.matmul(
                            pq[:, :nn],
                            lhsT=wslice(WQ, ji)[:, j * P:(j + 1) * P],
                            rhs=xt[:, ji * sh + n0: ji * sh + n0 + nn],
                            start=(ji == 0), stop=(ji == DJ - 1),
                        )
                    nc.vector.tensor_scalar(
                        out=qT[j][:, n0:n0 + nn], in0=pq[:, :nn],
                        scalar1=vp[:, 9 * D + j: 9 * D + j + 1], scalar2=None,
                        op0=mybir.AluOpType.add,
                    )
                    n0 += nn

            # ---------------- transformer pass (gather-based) ----------------
            def transformer_pass(gp, pst, h_out, hT_out, tile_hook):
                dstc = gp.tile([P, stot], F32, name="dstc", bufs=1)
                nc.sync.dma_start(out=dstc[:], in_=dst_in[:])
                idx_sb = gp.tile([P, stot * 8], I16, name="idx_sb", bufs=1)
                nc.sync.dma_start(out=idx_sb[:], in_=idx_in[:])
                for t in range(nt):
                    st = s_list[t]
                    et_t = st * P
                    splits = []
                    base = (st + khalft - 1) // khalft
                    c0 = 0
                    while c0 < st:
                        splits.append((c0, min(st, c0 + base)))
                        c0 += base
                    kgt = gp.tile([P, DJ, et_t], BF16, name="kgt", tag="kgt",
                                  bufs=int(os.environ.get("KKGT", "2")))
                    vg = gp.tile([P, S, D], BF16, name="vg", tag="vg",
                                 bufs=int(os.environ.get("KVG", "2")))
                    idx_tt = idx_sb[:, soff[t] * 8:(soff[t] + st) * 8]
                    nc.gpsimd.dma_gather(
                        out_ap=kgt[:], in_ap=k_full[:], idxs_ap=idx_tt,
                        num_idxs=et_t, num_idxs_reg=et_t, elem_size=D,
                        transpose=True, single_packet=False)
                    for (ca, cb) in splits:
                        nn_i = (cb - ca) * P
                        idx_t = idx_sb[:, (soff[t] + ca) * 8: (soff[t] + cb) * 8]
                        nc.gpsimd.dma_gather(
                            out_ap=vg[:, ca:cb, :], in_ap=v_full[:], idxs_ap=idx_t,
                            num_idxs=nn_i, num_idxs_reg=nn_i, elem_size=D,
                            single_packet=False)

                    pagg = pst.tile([P, D + 1], F32, name="pagg", tag="pagg",
                                    bufs=kpagg)
                    for c in range(st):
                        dcol = dstc[:, soff[t] + c: soff[t] + c + 1]
                        psc = pst.tile([P, P], F32, name="psc", tag="psc", bufs=kpsc)
                        for j in range(DJ):
                            nc.tensor.matmul(
                                psc[:],
                                lhsT=kgt[:, j, c * P:(c + 1) * P],
                                rhs=qT[j][:, t * P:(t + 1) * P],
                                start=(j == 0), stop=(j == DJ - 1))
                        exps = smp.tile([P, P], BF16, name="exps")
                        nc.scalar.activation(exps[:], psc[:],
                                             mybir.ActivationFunctionType.Exp,
                                             scale=scale)
                        w_b = smp.tile([P, P], BF16, name="w_b", tag="w_b")
                        nc.vector.scalar_tensor_tensor(
                            out=w_b[:], in0=iota_b[:], scalar=dcol, in1=exps[:],
                            op0=mybir.AluOpType.is_equal,
                            op1=mybir.AluOpType.mult)
                        nc.tensor.matmul(pagg[:, :D], lhsT=w_b[:], rhs=vg[:, c, :],
                                         start=(c == 0), stop=(c == st - 1))
                        nc.tensor.matmul(pagg[:, D:D + 1], lhsT=w_b[:],
                                         rhs=ones_v[:],
                                         start=False, stop=(c == st - 1))

                    smax = smp.tile([P, 1], F32, name="smax")
                    nc.vector.tensor_scalar(
                        out=smax[:], in0=pagg[:, D:D + 1], scalar1=1e-30,
                        scalar2=None, op0=mybir.AluOpType.max)
                    rs = smp.tile([P, 1], F32, name="rs")
                    nc.vector.reciprocal(rs[:], smax[:])
                    pskip = ps.tile([P, D], F32, name="pskip", tag="pmm", bufs=kpmm)
                    for ji in range(DJ):
                        nc.tensor.matmul(pskip[:], lhsT=xtile(ji, t),
                                         rhs=wslice(WS, ji),
                                         start=(ji == 0), stop=False)
                    nc.tensor.matmul(pskip[:], lhsT=ones_row[:],
                                     rhs=wp[:1, WBIAS * D:(WBIAS + 1) * D],
                                     start=False, stop=True)
                    t1 = smp.tile([P, D], F32, name="t1", tag="t1")
                    nc.scalar.activation(t1[:], pagg[:, :D],
                                         mybir.ActivationFunctionType.Copy,
                                         scale=rs[:, :1])
                    t2 = smp.tile([P, D], F32, name="t2", tag="t2")
                    nc.vector.tensor_tensor(out=t2[:], in0=t1[:], in1=pskip[:],
                                            op=mybir.AluOpType.add)
                    nc.scalar.activation(h_out[:, t * D:(t + 1) * D], t2[:],
                                         mybir.ActivationFunctionType.Relu)
                    epilogue_store(0, t, h_out, hT_out)
                    tile_hook(t)
                allgather(hag_in[0], h_full[0])

            def epilogue_store(li, t, h_out, hT_out):
                """Write fp8 pack row + transposed shard for layer output li."""
                hstage = smp.tile([P, D], FP8, name="hstage")
                nc.vector.tensor_copy(out=hstage[:],
                                      in_=h_out[:, t * D:(t + 1) * D])
                nc.sync.dma_start(out=hag_in[li][:, t * D:(t + 1) * D],
                                  in_=hstage[:])
                for j in range(DJ):
                    ptr2 = ps.tile([P, P], BF16, name="ptr2", tag="ptr", bufs=kptr)
                    nc.tensor.transpose(
                        out=ptr2[:],
                        in_=h_out[:, t * D + j * P: t * D + (j + 1) * P],
                        identity=ident_b[:])
                    nc.vector.tensor_copy(
                        out=hT_out[:, j * sh + t * P: j * sh + (t + 1) * P],
                        in_=ptr2[:])

            # ---------------- SAGE pass (dense fp8 DoubleRow) ----------------
            # dst tiles processed in groups of 4; one [P, 512] PSUM bank per
            # (group, dh), double-buffered so group g+1's adjacency sweep
            # overlaps group g's epilogue.
            def sage_pass(layer, pd, a_rhs, h_prev, hT_prev, h_out, hT_out):
                li = layer + 1
                # load H table (tile-major fp8) from the AllGathered pack
                for r in range(NC):
                    nc.sync.dma_start(
                        out=h_sb[:, r * nt:(r + 1) * nt, :],
                        in_=h_full[layer][r * P:(r + 1) * P, :])
                groups = [(g * 4, min(nt, (g + 1) * 4)) for g in range((nt + 3) // 4)]

                def sage_epilogue(t):
                    pzm = pd.tile([P, D], F32, name="pzm", tag="pzm", bufs=1)
                    for j in range(DJ):
                        nc.tensor.matmul(pzm[:],
                                         lhsT=meanT_all[:, t, j * P:(j + 1) * P],
                                         rhs=wslice(WL[layer], j),
                                         start=(j == 0), stop=(j == DJ - 1))
                    pz = ps.tile([P, D], F32, name="pz", tag="pmm", bufs=kpmm)
                    for j in range(DJ):
                        nc.tensor.matmul(
                            pz[:],
                            lhsT=hT_prev[:, j * sh + t * P: j * sh + (t + 1) * P],
                            rhs=wslice(WR[layer], j),
                            start=(j == 0), stop=False)
                    nc.tensor.matmul(
                        pz[:], lhsT=ones_row[:],
                        rhs=wp[:1, (WBIAS + 1 + layer) * D:(WBIAS + 2 + layer) * D],
                        start=False, stop=True)
                    t0 = smp.tile([P, D], F32, name="t0s", tag="t0")
                    nc.scalar.activation(t0[:], pzm[:],
                                         mybir.ActivationFunctionType.Copy,
                                         scale=invd[:, t:t + 1])
                    t1 = smp.tile([P, D], F32, name="t1s", tag="t2")
                    nc.vector.tensor_tensor(out=t1[:], in0=t0[:], in1=pz[:],
                                            op=mybir.AluOpType.add)
                    t3 = smp.tile([P, D], F32, name="t3s", tag="t4")
                    nc.vector.scalar_tensor_tensor(
                        out=t3[:], in0=h_prev[:, t * D:(t + 1) * D], scalar=oma,
                        in1=t1[:], op0=mybir.AluOpType.mult,
                        op1=mybir.AluOpType.add)
                    if layer < L - 1:
                        nc.scalar.activation(h_out[:, t * D:(t + 1) * D], t3[:],
                                             mybir.ActivationFunctionType.Relu)
                        epilogue_store(li, t, h_out, hT_out)
                    else:
                        hof = smp.tile([P, D], F32, name="hof", tag="t1")
                        nc.scalar.activation(hof[:], t3[:],
                                             mybir.ActivationFunctionType.Relu)
                        nc.sync.dma_start(out=out_dram[t * P:(t + 1) * P, :],
                                          in_=hof[:])

                for gi, (ta, tb) in enumerate(groups):
                    tw = (tb - ta) * P
                    pb = [pd.tile([P, 4 * P], F32, name=f"pdb_{gi % 2}_{dh}",
                                  tag=f"pdb_{gi % 2}_{dh}", bufs=1)
                          for dh in range(DJ)]
                    for dh in range(DJ):
                        for c2 in range(NC2):
                            nc.tensor.matmul(
                                pb[dh][:, :tw],
                                lhsT=h_sb[:, 2 * c2:2 * c2 + 2, dh * P:(dh + 1) * P],
                                rhs=a_rhs(c2, ta, tb),
                                start=(c2 == 0), stop=(c2 == NC2 - 1),
                                perf_mode=mybir.MatmulPerfMode.DoubleRow)
                        nc.vector.tensor_copy(
                            out=meanT_all[:, ta:tb, dh * P:(dh + 1) * P],
                            in_=pb[dh][:, :tw])
                    for t in range(ta, tb):
                        sage_epilogue(t)
                if layer < L - 1:
                    allgather(hag_in[li], h_full[li])

            if stages <= 1:
                tmpo = smp.tile([P, D], F32, name="tmpo")
                for t in range(nt):
                    nc.vector.tensor_copy(out=tmpo[:], in_=xt[:, :D])
                    nc.sync.dma_start(out=out_dram[t * P:(t + 1) * P, :],
                                      in_=tmpo[:])
            else:
                with tc.tile_pool(name="ae", bufs=1) as ape:
                    a_early = None
                    if stages >= 3 and kae > 0:
                        a_early = ape.tile([P, kae, 2, nt * P], FP8)

                    def tile_hook(t):
                        # trickle early A-table c2-blocks through transformer
                        if a_early is not None and t < kae:
                            nc.scalar.dma_start(
                                out=a_early[:, t, :, :],
                                in_=a_in[:, t * 2 * nt * P:(t + 1) * 2 * nt * P])

                    with (
                        tc.tile_pool(name="g", bufs=kgp) as gp,
                        tc.tile_pool(name="pst", bufs=1, space="PSUM") as pst,
                    ):
                        transformer_pass(gp, pst, h_cur, hT_cur, tile_hook)
                    if stages >= 3:
                        with (
                            tc.tile_pool(name="ap", bufs=1) as app,
                            tc.tile_pool(name="pd", bufs=1, space="PSUM") as pd,
                        ):
                            nlate = NC2 - kae
                            a_late = app.tile([P, max(1, nlate), 2, nt * P], FP8)
                            cw = (nlate + kapc - 1) // kapc
                            for p_ in range(kapc):
                                c0 = min(nlate, p_ * cw)
                                c1 = min(nlate, (p_ + 1) * cw)
                                if c0 >= c1:
                                    continue
                                # ACT-issued so H loads (SP queue) interleave
                                nc.scalar.dma_start(
                                    out=a_late[:, c0:c1, :, :],
                                    in_=a_in[:, (kae + c0) * 2 * nt * P:
                                             (kae + c1) * 2 * nt * P])

                            def a_rhs(c2, ta, tb):
                                if c2 < kae:
                                    return a_early[:, c2, :, ta * P:tb * P]
                                return a_late[:, c2 - kae, :, ta * P:tb * P]

                            bufs = [(h_cur, hT_cur), (h_nxt, hT_nxt)]
                            for i in range(L):
                                h_prev, hT_prev = bufs[i % 2]
                                h_out, hT_out = bufs[(i + 1) % 2]
                                sage_pass(i, pd, a_rhs, h_prev, hT_prev,
                                          h_out, hT_out)
                    else:
                        for t in range(nt):
                            hof = smp.tile([P, D], F32, name="hof", tag="t1")
                            nc.vector.tensor_copy(out=hof[:],
                                                  in_=h_cur[:, t * D:(t + 1) * D])
                            nc.sync.dma_start(out=out_dram[t * P:(t + 1) * P, :],
                                              in_=hof[:])

    nc.compile()
    _nc_cache[key] = nc
    return nc


def _host_prep(x, src, dst, Wq, bq, Wk, bk, Wv, bv, Ws, bs, Wl, bl, Wr,
               gamma, beta, alpha_res):
    n, d = x.shape
    n_pad = ((n + NC * P - 1) // (NC * P)) * (NC * P)
    sh = n_pad // NC
    nt = sh // P
    n_tiles = n_pad // P

    order = np.argsort(dst, kind="stable")
    src_s, dst_s = src[order], dst[order]
    tile_of = dst_s // P
    counts = np.bincount(tile_of, minlength=n_tiles)
    starts = np.concatenate([[0], np.cumsum(counts)])
    s_all = np.maximum(1, (counts + P - 1) // P).astype(np.int64)
    s_pos = s_all.reshape(NC, nt).max(axis=0)
    s_list = tuple(int(v) for v in s_pos)
    soff = np.concatenate([[0], np.cumsum(s_pos)])
    stot = int(soff[-1])

    deg = np.bincount(dst, minlength=n_pad).astype(np.float32)
    invdeg_full = 1.0 / np.maximum(deg, 1.0)

    al = 1.0 / (1.0 + np.exp(-alpha_res))
    oma = float(1.0 - al)
    bn_scale = 1.0 / np.sqrt(1.0 + BN_EPS)
    scale = 1.0 / np.sqrt(float(d))

    x_pad = np.zeros((n_pad, D), np.float32)
    x_pad[:n] = x
    xT = x_pad.T.astype(NP_BF16)

    # fold the BN gain into the SAGE weights: z*Gx == mean@(Wl*Gx) + h@(Wr*Gx)
    GxF = [al * bn_scale * gamma[i] for i in range(L)]
    Wlg = [Wl[i] * GxF[i][None, :] for i in range(L)]
    Wrg = [Wr[i] * GxF[i][None, :] for i in range(L)]
    weights = [Wq, Wk, Wv, Ws, Wlg[0], Wrg[0], Wlg[1], Wrg[1], Wlg[2], Wrg[2]]
    wpack = np.zeros((P, (10 * DJ + 4) * D), NP_BF16)
    for w, W in enumerate(weights):
        for j in range(DJ):
            wpack[:, (w * DJ + j) * D:(w * DJ + j + 1) * D] = \
                W[j * P:(j + 1) * P, :].astype(NP_BF16)
    # bias rows (read from partition 0 only): bs, Bx0..Bx2
    BxF = [al * (bl[i] * bn_scale * gamma[i] + beta[i]) for i in range(L)]
    for bi, vec in enumerate([bs] + BxF):
        wpack[0, (10 * DJ + bi) * D:(10 * DJ + bi + 1) * D] = vec.astype(NP_BF16)

    Gx = [al * bn_scale * gamma[i] for i in range(L)]
    Bx = [al * (bl[i] * bn_scale * gamma[i] + beta[i]) for i in range(L)]
    vecs = [bk, bv, bs, Gx[0], Bx[0], Gx[1], Bx[1], Gx[2], Bx[2]]
    vpack = np.empty((P, 9 * D + DJ), np.float32)
    for k, v in enumerate(vecs):
        vpack[:, k * D:(k + 1) * D] = np.tile(v[None, :], (P, 1))
    for j in range(DJ):
        vpack[:, 9 * D + j] = bq[j * P:(j + 1) * P]

    # dense mean-normalized adjacency in DoubleRow block layout, per core:
    # a[k, ((c2*nt + t)*2 + i)*P + m] = mult(src=256*c2+128*i+k -> dst) / deg
    NC2 = (n_pad // P) // 2
    NBLK = NC2 * nt
    kk = (src_s % P).astype(np.int64)
    ii = (src_s // P) % 2
    c2 = src_s // (2 * P)
    w_e = np.ones(len(dst_s), np.float32)  # integer counts; exact in fp8

    in_maps = []
    for r in range(NC):
        idx_arr = np.zeros((P, stot * 8), np.int16)
        dst_arr = np.full((P, stot), 128.0, np.float32)
        for tloc in range(nt):
            g = r * nt + tloc
            e0, e1 = starts[g], starts[g + 1]
            cnt = e1 - e0
            st_ = int(s_pos[tloc])
            et_t = st_ * P
            srcs = np.zeros(et_t, np.int64)
            srcs[:cnt] = src_s[e0:e1]
            dl = np.full(et_t, 128, np.int64)
            dl[:cnt] = dst_s[e0:e1] - g * P
            o = int(soff[tloc])
            idx_arr[:, o * 8:(o + st_) * 8] = _wrap_idx(srcs)
            dst_arr[:, o:o + st_] = dl.reshape(st_, P).T

        e0, e1 = starts[r * nt], starts[(r + 1) * nt]
        tl = (dst_s[e0:e1] - r * sh) // P
        mm = (dst_s[e0:e1] - r * sh) % P
        # layout [c2][i][t*P + m]
        flat = (c2[e0:e1] * 2 + ii[e0:e1]) * (nt * P) + tl * P + mm
        a_arr = np.zeros((P, NBLK * 2 * P), np.float32)
        np.add.at(a_arr, (kk[e0:e1], flat), w_e[e0:e1])
        a_arr = a_arr.astype(NP_FP8)
        invdeg_r = invdeg_full[r * sh:(r + 1) * sh].reshape(nt, P).T.copy()

        xt_r = np.empty((P, DJ * sh), NP_BF16)
        for j in range(DJ):
            xt_r[:, j * sh:(j + 1) * sh] = xT[j * P:(j + 1) * P, r * sh:(r + 1) * sh]

        in_maps.append({
            "xt_in": xt_r,
            "wpack_in": wpack,
            "vpack_in": vpack,
            "idx_in": idx_arr,
            "dst_in": dst_arr,
            "invdeg_in": np.ascontiguousarray(invdeg_r),
            "a_in": a_arr,
        })
    return in_maps, (n_pad, sh, nt, s_list, scale, oma)


def kernel(**inputs):
    x = np.asarray(inputs["x"], np.float32)
    edge_index = np.asarray(inputs["edge_index"])
    args = dict(
        Wq=np.asarray(inputs["Wq"], np.float32), bq=np.asarray(inputs["bq"], np.float32),
        Wk=np.asarray(inputs["Wk"], np.float32), bk=np.asarray(inputs["bk"], np.float32),
        Wv=np.asarray(inputs["Wv"], np.float32), bv=np.asarray(inputs["bv"], np.float32),
        Ws=np.asarray(inputs["Ws"], np.float32), bs=np.asarray(inputs["bs"], np.float32),
        Wl=np.asarray(inputs["Wl"], np.float32), bl=np.asarray(inputs["bl"], np.float32),
        Wr=np.asarray(inputs["Wr"], np.float32),
        gamma=np.asarray(inputs["gamma"], np.float32),
        beta=np.asarray(inputs["beta"], np.float32),
        alpha_res=float(np.asarray(inputs["alpha_res"])),
    )
    src = edge_index[0].astype(np.int64)
    dst = edge_index[1].astype(np.int64)

    in_maps, params = _host_prep(x, src, dst, **args)
    t0 = time.time()
    nc = build_nc(*params)
    print(f"[kernel] build+compile {time.time()-t0:.1f}s", flush=True)
    t0 = time.time()
    res = run_bass_kernel_spmd(nc, in_maps, core_ids=list(range(NC)))
    print(f"[kernel] run {time.time()-t0:.1f}s", flush=True)
    out = np.concatenate([res.results[r]["out"] for r in range(NC)], axis=0)
    return out[:x.shape[0]]


# revision 66
# speedup vs baseline: 1.3350x; 1.0550x over previous
"""Trainium2 Bass kernel for nn_MixGNN (TransformerConv + 3x SAGEConv + BN + gated residual).

Strategy (8 NeuronCores, dst-node sharding; pad N 10000 -> 10240, core r owns
1280 dst nodes = 10 tiles of 128):
  - TransformerConv (per-edge gather): dma_gather of source k rows
    (transposed, bf16) + v rows; attention scores as KgT.T @ qT on PE;
    softmax without max-subtraction (logits are O(1)); per-chunk exp-weighted
    one-hot matmuls accumulate value sums + exp sums in PSUM.
  - SAGEConv x3 (dense fp8 DoubleRow — no gathers): host precomputes the
    integer-count adjacency A[src, dst] (exact in fp8e4) in PE DoubleRow
    block layout [c2][i][t*128+m]; each pass computes sumT[d, dst] =
    sum_src H[src, d] * A[src, dst] as 256-src-deep fp8 matmuls at 0.5
    cyc/row, batching 4 dst tiles per matmul (one [128,512] PSUM bank).
    1/deg is applied on the PSUM of sumT@Wl (per-dst-partition ACT scale);
    BN gain is folded into Wl/Wr, biases added via 1-partition matmuls.
    The full H table (fp8, tile-major) lives in SBUF, refreshed per pass
    via AllGather of fp8 shard packs. The 13 MB A table streams into SBUF
    partly during the transformer phase (small early pool) and the rest
    right after the gather pools release.
  - 5 collectives total: AllGather k, v (bf16), h0..h2 (fp8).
Output: fp32 [10000, 256].
"""
import os
import sys
import time

import numpy as np

for _p in ("/opt/trn_rl_repo",):
    if _p not in sys.path:
        sys.path.insert(0, _p)

import ml_dtypes  # noqa: E402
import concourse.bacc as bacc  # noqa: E402
import concourse.mybir as mybir  # noqa: E402
import concourse.tile as tile  # noqa: E402
from concourse.bass_utils import run_bass_kernel_spmd  # noqa: E402

P = 128
D = 256
DJ = D // P           # 2 d-chunks of 128
NC = 8                # cores
L = 3                 # SAGE layers
BN_EPS = 1e-5

F32 = mybir.dt.float32
BF16 = mybir.dt.bfloat16
FP8 = mybir.dt.float8e4
I16 = mybir.dt.int16
NP_FP8 = ml_dtypes.float8_e4m3
NP_BF16 = ml_dtypes.bfloat16

_nc_cache = {}


def _wrap_idx(a):
    """[S*128] int array -> [128, S*8] int16 wrapped gather-index layout."""
    w16 = a.reshape(-1, 16).T.astype(np.int16)   # [16, S*8]
    return np.tile(w16, (8, 1))                  # replicate to 8 Q7 stripes


def build_nc(n_pad, sh, nt, s_list, scale, oma):
    stages = int(os.environ.get("KSTAGES", "5"))
    nocc = os.environ.get("KNOCC") == "1"
    ksm = int(os.environ.get("KSM", "4"))
    kgp = int(os.environ.get("KGP", "2"))
    kpsc = int(os.environ.get("KPSC", "3"))
    kpagg = int(os.environ.get("KPAGG", "2"))
    kpmm = int(os.environ.get("KPMM", "2"))
    kptr = int(os.environ.get("KPTR", "1"))
    khalft = int(os.environ.get("KHALFT", "2"))  # transformer v-gather splits
    kapc = int(os.environ.get("KAPC", "8"))      # A-table late-load pieces
    # A c2-blocks loaded early; up to two are issued per transformer tile,
    # so at most 2*nt blocks can be trickled in.
    kae = min(int(os.environ.get("KAEARLY", "19")), 2 * nt)
    key = (n_pad, sh, nt, s_list, round(scale, 9), round(oma, 9), stages,
           nocc, ksm, kgp, kpsc, kpagg, kpmm, kptr, khalft, kapc, kae,
           os.environ.get("KKGT"), os.environ.get("KVG"))
    if key in _nc_cache:
        return _nc_cache[key]

    S = max(s_list)
    soff = [0]
    for st_ in s_list:
        soff.append(soff[-1] + st_)
    stot = soff[-1]

    NCH = n_pad // P          # 80 source chunks of 128
    NC2 = NCH // 2            # 40 double-chunks of 256
    NBLK = NC2 * nt           # 400 A-blocks per core

    ndev = 1 if nocc else NC
    nc = bacc.Bacc("TRN2", target_bir_lowering=False, debug=False, num_devices=ndev)

    xt_in = nc.dram_tensor("xt_in", [P, DJ * sh], BF16, kind="ExternalInput")
    wpack_in = nc.dram_tensor("wpack_in", [P, (10 * DJ + 4) * D], BF16,
                              kind="ExternalInput")
    vpack_in = nc.dram_tensor("vpack_in", [P, 9 * D + DJ], F32, kind="ExternalInput")
    idx_in = nc.dram_tensor("idx_in", [P, stot * 8], I16, kind="ExternalInput")
    dst_in = nc.dram_tensor("dst_in", [P, stot], F32, kind="ExternalInput")
    invdeg_in = nc.dram_tensor("invdeg_in", [P, nt], F32, kind="ExternalInput")
    a_in = nc.dram_tensor("a_in", [P, NBLK * 2 * P], FP8, kind="ExternalInput")
    out_dram = nc.dram_tensor("out", [sh, D], F32, kind="ExternalOutput")

    WQ, WK, WV, WS = 0, 1, 2, 3
    WL = [4, 6, 8]
    WR = [5, 7, 9]
    WBIAS = 10 * DJ          # 4 bias rows (bs, Bx0..2) in wpack row 0
    VBK, VBV, VBS = 0, 1, 2

    with tile.TileContext(nc) as tc:
        with (
            tc.tile_pool(name="cst", bufs=1) as cst,
            tc.tile_pool(name="sb", bufs=1) as sb,
            tc.tile_pool(name="sm", bufs=ksm) as smp,
            tc.tile_pool(name="ps", bufs=2, space="PSUM") as ps,
            tc.tile_pool(name="dr", bufs=1, space="DRAM") as dr,
        ):
            # ---------------- constants / inputs to SBUF ----------------
            # load order tuned so the k/v matmuls (xt + WQ..WS + biases)
            # unblock as early as possible
            xt = cst.tile([P, DJ * sh], BF16)
            for _xi in range(2):
                _c0 = _xi * (DJ * sh // 2)
                _c1 = (_xi + 1) * (DJ * sh // 2)
                nc.sync.dma_start(out=xt[:, _c0:_c1], in_=xt_in[:, _c0:_c1])
            wp = cst.tile([P, (10 * DJ + 4) * D], BF16)
            _wsplit = 4 * DJ * D
            nc.sync.dma_start(out=wp[:, :_wsplit], in_=wpack_in[:, :_wsplit])
            vp = cst.tile([P, 9 * D + DJ], F32)
            _vsplit = 3 * D
            nc.sync.dma_start(out=vp[:, :_vsplit], in_=vpack_in[:, :_vsplit])
            nc.sync.dma_start(out=wp[:, _wsplit:], in_=wpack_in[:, _wsplit:])
            nc.sync.dma_start(out=vp[:, _vsplit:], in_=vpack_in[:, _vsplit:])
            invd = cst.tile([P, nt], F32)
            nc.sync.dma_start(out=invd[:], in_=invdeg_in[:])

            iota_i = cst.tile([P, P], mybir.dt.int32)
            nc.gpsimd.iota(iota_i[:], pattern=[[1, P]], base=0, channel_multiplier=0)
            ones_v = cst.tile([P, 1], BF16)
            nc.vector.memset(ones_v[:], 1.0)
            iota_part = cst.tile([P, 1], mybir.dt.int32)
            nc.gpsimd.iota(iota_part[:], pattern=[[1, 1]], base=0, channel_multiplier=1)
            iota_part_f = cst.tile([P, 1], F32)
            nc.vector.tensor_copy(out=iota_part_f[:], in_=iota_part[:])
            iota_f = cst.tile([P, P], F32)
            nc.vector.tensor_copy(out=iota_f[:], in_=iota_i[:])
            ident = cst.tile([P, P], F32)
            nc.vector.tensor_scalar(
                out=ident[:], in0=iota_f[:], scalar1=iota_part_f[:, :1], scalar2=None,
                op0=mybir.AluOpType.is_equal,
            )
            ident_b = cst.tile([P, P], BF16)
            nc.vector.tensor_copy(out=ident_b[:], in_=ident[:])
            iota_b = cst.tile([P, P], BF16)
            nc.vector.tensor_copy(out=iota_b[:], in_=iota_f[:])

            def wslice(w, j):
                return wp[:, (w * DJ + j) * D:(w * DJ + j + 1) * D]

            def vslice(k):
                return vp[:, k * D:(k + 1) * D]

            def xtile(j, t):
                return xt[:, j * sh + t * P: j * sh + (t + 1) * P]

            # ---------------- long-lived SBUF state ----------------
            qT = [sb.tile([P, sh], BF16, name=f"qT_{j}") for j in range(DJ)]
            h_cur = sb.tile([P, nt * D], BF16)
            h_nxt = sb.tile([P, nt * D], BF16)
            hT_cur = sb.tile([P, DJ * sh], BF16)
            hT_nxt = sb.tile([P, DJ * sh], BF16)
            h_sb = sb.tile([P, NCH, D], FP8)     # full H table, tile-major
            meanT_all = sb.tile([P, nt, D], BF16)  # paggT staging per pass

            ones_row = cst.tile([1, P], BF16)
            nc.vector.memset(ones_row[:], 1.0)

            # ---------------- DRAM tables ----------------
            # k and v packed per node into one row -> single AllGather, and
            # the gathers slice columns via elem_step=2D.
            kv_ag_in = dr.tile([sh, 2 * D], BF16)
            kv_full = dr.tile([n_pad, 2 * D], BF16, addr_space="Shared")
            hag_in = [dr.tile([P, nt * D], FP8, name=f"hag_in_{i}") for i in range(L)]
            h_full = [dr.tile([NC * P, nt * D], FP8, name=f"h_full_{i}",
                              addr_space="Shared") for i in range(L)]

            def allgather(in_t, out_t):
                if nocc:
                    nc.sync.dma_start(out=out_t[:in_t.shape[0]], in_=in_t[:])
                else:
                    nc.gpsimd.collective_compute(
                        "AllGather", mybir.AluOpType.bypass,
                        replica_groups=[list(range(NC))],
                        ins=[in_t[:]], outs=[out_t[:]],
                    )

            # ---------------- stage 0: packed k|v shard table + AG, then qT --
            for t in range(nt):
                kv_sb = smp.tile([P, 2 * D], BF16, name="kv_sb")
                pk = ps.tile([P, D], F32, name="pk", tag="pmm", bufs=kpmm)
                for ji in range(DJ):
                    nc.tensor.matmul(pk[:], lhsT=xtile(ji, t), rhs=wslice(WK, ji),
                                     start=(ji == 0), stop=(ji == DJ - 1))
                nc.vector.tensor_tensor(out=kv_sb[:, :D], in0=pk[:],
                                        in1=vslice(VBK), op=mybir.AluOpType.add)
                pv = ps.tile([P, D], F32, name="pv", tag="pmm", bufs=kpmm)
                for ji in range(DJ):
                    nc.tensor.matmul(pv[:], lhsT=xtile(ji, t), rhs=wslice(WV, ji),
                                     start=(ji == 0), stop=(ji == DJ - 1))
                nc.vector.tensor_tensor(out=kv_sb[:, D:], in0=pv[:],
                                        in1=vslice(VBV), op=mybir.AluOpType.add)
                nc.sync.dma_start(out=kv_ag_in[t * P:(t + 1) * P, :], in_=kv_sb[:])
            allgather(kv_ag_in, kv_full)

            for j in range(DJ):
                n0 = 0
                while n0 < sh:
                    nn = min(512, sh - n0)
                    pq = ps.tile([P, 512], F32, name="pq", tag="pmm", bufs=kpmm)
                    for ji in range(DJ):
                        nc.tensor.matmul(
                            pq[:, :nn],
                            lhsT=wslice(WQ, ji)[:, j * P:(j + 1) * P],
                            rhs=xt[:, ji * sh + n0: ji * sh + n0 + nn],
                            start=(ji == 0), stop=(ji == DJ - 1),
                        )
                    nc.vector.tensor_scalar(
                        out=qT[j][:, n0:n0 + nn], in0=pq[:, :nn],
                        scalar1=vp[:, 9 * D + j: 9 * D + j + 1], scalar2=None,
                        op0=mybir.AluOpType.add,
                    )
                    n0 += nn

            # ---------------- transformer pass (gather-based) ----------------
            def transformer_pass(gp, pst, h_out, hT_out, tile_hook):
                dstc = gp.tile([P, stot], F32, name="dstc", bufs=1)
                nc.sync.dma_start(out=dstc[:], in_=dst_in[:])
                idx_sb = gp.tile([P, stot * 8], I16, name="idx_sb", bufs=1)
                nc.sync.dma_start(out=idx_sb[:], in_=idx_in[:])
                for t in range(nt):
                    st = s_list[t]
                    nh = (st + 1) // 2
                    halves = [(0, nh), (nh, st)]
                    kgs, vgs = [], []
                    for (ha, hb) in halves:
                        hc = hb - ha
                        nn_i = hc * P
                        idx_t = idx_sb[:, (soff[t] + ha) * 8:(soff[t] + hb) * 8]
                        kg = gp.tile([P, DJ, nn_i], BF16, name="kgt", tag="kgt",
                                     bufs=int(os.environ.get("KKGT", "3")))
                        nc.gpsimd.dma_gather(
                            out_ap=kg[:], in_ap=kv_full[:, :D], idxs_ap=idx_t,
                            num_idxs=nn_i, num_idxs_reg=nn_i, elem_size=D,
                            elem_step=2 * D, transpose=True, single_packet=False)
                        vg = gp.tile([P, hc, D], BF16, name="vg", tag="vg",
                                     bufs=int(os.environ.get("KVG", "3")))
                        nc.gpsimd.dma_gather(
                            out_ap=vg[:], in_ap=kv_full[:, D:], idxs_ap=idx_t,
                            num_idxs=nn_i, num_idxs_reg=nn_i, elem_size=D,
                            elem_step=2 * D, single_packet=False)
                        kgs.append(kg)
                        vgs.append(vg)

                    pagg = pst.tile([P, D + 1], F32, name="pagg", tag="pagg",
                                    bufs=kpagg)
                    for c in range(st):
                        hi = 0 if c < nh else 1
                        cl = c - halves[hi][0]
                        dcol = dstc[:, soff[t] + c: soff[t] + c + 1]
                        psc = pst.tile([P, P], F32, name="psc", tag="psc", bufs=kpsc)
                        for j in range(DJ):
                            nc.tensor.matmul(
                                psc[:],
                                lhsT=kgs[hi][:, j, cl * P:(cl + 1) * P],
                                rhs=qT[j][:, t * P:(t + 1) * P],
                                start=(j == 0), stop=(j == DJ - 1))
                        exps = smp.tile([P, P], BF16, name="exps")
                        nc.scalar.activation(exps[:], psc[:],
                                             mybir.ActivationFunctionType.Exp,
                                             scale=scale)
                        w_b = smp.tile([P, P], BF16, name="w_b", tag="w_b")
                        nc.vector.scalar_tensor_tensor(
                            out=w_b[:], in0=iota_b[:], scalar=dcol, in1=exps[:],
                            op0=mybir.AluOpType.is_equal,
                            op1=mybir.AluOpType.mult)
                        nc.tensor.matmul(pagg[:, :D], lhsT=w_b[:],
                                         rhs=vgs[hi][:, cl, :],
                                         start=(c == 0), stop=(c == st - 1))
                        nc.tensor.matmul(pagg[:, D:D + 1], lhsT=w_b[:],
                                         rhs=ones_v[:],
                                         start=False, stop=(c == st - 1))

                    smax = smp.tile([P, 1], F32, name="smax")
                    nc.vector.tensor_scalar(
                        out=smax[:], in0=pagg[:, D:D + 1], scalar1=1e-30,
                        scalar2=None, op0=mybir.AluOpType.max)
                    rs = smp.tile([P, 1], F32, name="rs")
                    nc.vector.reciprocal(rs[:], smax[:])
                    pskip = ps.tile([P, D], F32, name="pskip", tag="pmm", bufs=kpmm)
                    for ji in range(DJ):
                        nc.tensor.matmul(pskip[:], lhsT=xtile(ji, t),
                                         rhs=wslice(WS, ji),
                                         start=(ji == 0), stop=False)
                    nc.tensor.matmul(pskip[:], lhsT=ones_row[:],
                                     rhs=wp[:1, WBIAS * D:(WBIAS + 1) * D],
                                     start=False, stop=True)
                    t1 = smp.tile([P, D], F32, name="t1", tag="t1")
                    nc.scalar.activation(t1[:], pagg[:, :D],
                                         mybir.ActivationFunctionType.Copy,
                                         scale=rs[:, :1])
                    t2 = smp.tile([P, D], F32, name="t2", tag="t2")
                    nc.vector.tensor_tensor(out=t2[:], in0=t1[:], in1=pskip[:],
                                            op=mybir.AluOpType.add)
                    nc.scalar.activation(h_out[:, t * D:(t + 1) * D], t2[:],
                                         mybir.ActivationFunctionType.Relu)
                    epilogue_store(0, t, h_out, hT_out)
                    tile_hook(t)
                allgather(hag_in[0], h_full[0])

            def epilogue_store(li, t, h_out, hT_out):
                """Write fp8 pack row + transposed shard for layer output li.

                Copies go to DVE during the transformer (ACT is busy with
                exp) and to ACT during SAGE passes (DVE runs the epilogue).
                """
                cpy = nc.vector.tensor_copy if li == 0 else (
                    lambda out, in_: nc.scalar.copy(out=out, in_=in_))
                hstage = smp.tile([P, D], FP8, name="hstage")
                cpy(out=hstage[:], in_=h_out[:, t * D:(t + 1) * D])
                nc.sync.dma_start(out=hag_in[li][:, t * D:(t + 1) * D],
                                  in_=hstage[:])
                for j in range(DJ):
                    ptr2 = ps.tile([P, P], BF16, name="ptr2", tag="ptr", bufs=kptr)
                    nc.tensor.transpose(
                        out=ptr2[:],
                        in_=h_out[:, t * D + j * P: t * D + (j + 1) * P],
                        identity=ident_b[:])
                    cpy(out=hT_out[:, j * sh + t * P: j * sh + (t + 1) * P],
                        in_=ptr2[:])

            # ---------------- SAGE pass (dense fp8 DoubleRow) ----------------
            # dst tiles processed in groups of 4; one [P, 512] PSUM bank per
            # (group, dh), double-buffered so group g+1's adjacency sweep
            # overlaps group g's epilogue.
            def sage_pass(layer, pd, a_rhs, h_prev, hT_prev, h_out, hT_out):
                li = layer + 1
                # load H table (tile-major fp8) from the AllGathered pack
                for r in range(NC):
                    nc.sync.dma_start(
                        out=h_sb[:, r * nt:(r + 1) * nt, :],
                        in_=h_full[layer][r * P:(r + 1) * P, :])
                groups = [(g * 4, min(nt, (g + 1) * 4)) for g in range((nt + 3) // 4)]

                def sage_epilogue(t):
                    pzm = pd.tile([P, D], F32, name="pzm", tag="pzm", bufs=1)
                    for j in range(DJ):
                        nc.tensor.matmul(pzm[:],
                                         lhsT=meanT_all[:, t, j * P:(j + 1) * P],
                                         rhs=wslice(WL[layer], j),
                                         start=(j == 0), stop=(j == DJ - 1))
                    pz = ps.tile([P, D], F32, name="pz", tag="pmm", bufs=kpmm)
                    for j in range(DJ):
                        nc.tensor.matmul(
                            pz[:],
                            lhsT=hT_prev[:, j * sh + t * P: j * sh + (t + 1) * P],
                            rhs=wslice(WR[layer], j),
                            start=(j == 0), stop=False)
                    nc.tensor.matmul(
                        pz[:], lhsT=ones_row[:],
                        rhs=wp[:1, (WBIAS + 1 + layer) * D:(WBIAS + 2 + layer) * D],
                        start=False, stop=True)
                    t0 = smp.tile([P, D], F32, name="t0s", tag="t0")
                    nc.scalar.activation(t0[:], pzm[:],
                                         mybir.ActivationFunctionType.Copy,
                                         scale=invd[:, t:t + 1])
                    t1 = smp.tile([P, D], F32, name="t1s", tag="t2")
                    nc.vector.tensor_tensor(out=t1[:], in0=t0[:], in1=pz[:],
                                            op=mybir.AluOpType.add)
                    t3 = smp.tile([P, D], F32, name="t3s", tag="t4")
                    nc.vector.scalar_tensor_tensor(
                        out=t3[:], in0=h_prev[:, t * D:(t + 1) * D], scalar=oma,
                        in1=t1[:], op0=mybir.AluOpType.mult,
                        op1=mybir.AluOpType.add)
                    if layer < L - 1:
                        nc.scalar.activation(h_out[:, t * D:(t + 1) * D], t3[:],
                                             mybir.ActivationFunctionType.Relu)
                        epilogue_store(li, t, h_out, hT_out)
                    else:
                        hof = smp.tile([P, D], F32, name="hof", tag="t1")
                        nc.scalar.activation(hof[:], t3[:],
                                             mybir.ActivationFunctionType.Relu)
                        nc.sync.dma_start(out=out_dram[t * P:(t + 1) * P, :],
                                          in_=hof[:])

                for gi, (ta, tb) in enumerate(groups):
                    tw = (tb - ta) * P
                    pb = [pd.tile([P, 4 * P], F32, name=f"pdb_{gi % 2}_{dh}",
                                  tag=f"pdb_{gi % 2}_{dh}", bufs=1)
                          for dh in range(DJ)]
                    for dh in range(DJ):
                        for c2 in range(NC2):
                            nc.tensor.matmul(
                                pb[dh][:, :tw],
                                lhsT=h_sb[:, 2 * c2:2 * c2 + 2, dh * P:(dh + 1) * P],
                                rhs=a_rhs(c2, ta, tb),
                                start=(c2 == 0), stop=(c2 == NC2 - 1),
                                perf_mode=mybir.MatmulPerfMode.DoubleRow)
                        nc.vector.tensor_copy(
                            out=meanT_all[:, ta:tb, dh * P:(dh + 1) * P],
                            in_=pb[dh][:, :tw])
                    for t in range(ta, tb):
                        sage_epilogue(t)
                if layer < L - 1:
                    allgather(hag_in[li], h_full[li])

            if stages <= 1:
                tmpo = smp.tile([P, D], F32, name="tmpo")
                for t in range(nt):
                    nc.vector.tensor_copy(out=tmpo[:], in_=xt[:, :D])
                    nc.sync.dma_start(out=out_dram[t * P:(t + 1) * P, :],
                                      in_=tmpo[:])
            else:
                with tc.tile_pool(name="ae", bufs=1) as ape:
                    a_early = None
                    if stages >= 3 and kae > 0:
                        a_early = ape.tile([P, kae, 2, nt * P], FP8)

                    def tile_hook(t):
                        # trickle early A-table c2-blocks through transformer
                        if a_early is None:
                            return
                        for b in (2 * t, 2 * t + 1):
                            if b < kae:
                                nc.scalar.dma_start(
                                    out=a_early[:, b, :, :],
                                    in_=a_in[:, b * 2 * nt * P:
                                             (b + 1) * 2 * nt * P])

                    with (
                        tc.tile_pool(name="g", bufs=kgp) as gp,
                        tc.tile_pool(name="pst", bufs=1, space="PSUM") as pst,
                    ):
                        transformer_pass(gp, pst, h_cur, hT_cur, tile_hook)
                    if stages >= 3:
                        with (
                            tc.tile_pool(name="ap", bufs=1) as app,
                            tc.tile_pool(name="pd", bufs=1, space="PSUM") as pd,
                        ):
                            nlate = NC2 - kae
                            a_late = app.tile([P, max(1, nlate), 2, nt * P], FP8)
                            cw = (nlate + kapc - 1) // kapc
                            for p_ in range(kapc):
                                c0 = min(nlate, p_ * cw)
                                c1 = min(nlate, (p_ + 1) * cw)
                                if c0 >= c1:
                                    continue
                                # ACT-issued so H loads (SP queue) interleave
                                nc.scalar.dma_start(
                                    out=a_late[:, c0:c1, :, :],
                                    in_=a_in[:, (kae + c0) * 2 * nt * P:
                                             (kae + c1) * 2 * nt * P])

                            def a_rhs(c2, ta, tb):
                                if c2 < kae:
                                    return a_early[:, c2, :, ta * P:tb * P]
                                return a_late[:, c2 - kae, :, ta * P:tb * P]

                            bufs = [(h_cur, hT_cur), (h_nxt, hT_nxt)]
                            for i in range(L):
                                h_prev, hT_prev = bufs[i % 2]
                                h_out, hT_out = bufs[(i + 1) % 2]
                                sage_pass(i, pd, a_rhs, h_prev, hT_prev,
                                          h_out, hT_out)
                    else:
                        for t in range(nt):
                            hof = smp.tile([P, D], F32, name="hof", tag="t1")
                            nc.vector.tensor_copy(out=hof[:],
                                                  in_=h_cur[:, t * D:(t + 1) * D])
                            nc.sync.dma_start(out=out_dram[t * P:(t + 1) * P, :],
                                              in_=hof[:])

    nc.compile()
    _nc_cache[key] = nc
    return nc


def _host_prep(x, src, dst, Wq, bq, Wk, bk, Wv, bv, Ws, bs, Wl, bl, Wr,
               gamma, beta, alpha_res):
    n, d = x.shape
    n_pad = ((n + NC * P - 1) // (NC * P)) * (NC * P)
    sh = n_pad // NC
    nt = sh // P
    n_tiles = n_pad // P

    order = np.argsort(dst, kind="stable")
    src_s, dst_s = src[order], dst[order]
    tile_of = dst_s // P
    counts = np.bincount(tile_of, minlength=n_tiles)
    starts = np.concatenate([[0], np.cumsum(counts)])
    s_all = np.maximum(1, (counts + P - 1) // P).astype(np.int64)
    s_pos = s_all.reshape(NC, nt).max(axis=0)
    s_list = tuple(int(v) for v in s_pos)
    soff = np.concatenate([[0], np.cumsum(s_pos)])
    stot = int(soff[-1])

    deg = np.bincount(dst, minlength=n_pad).astype(np.float32)
    invdeg_full = 1.0 / np.maximum(deg, 1.0)

    al = 1.0 / (1.0 + np.exp(-alpha_res))
    oma = float(1.0 - al)
    bn_scale = 1.0 / np.sqrt(1.0 + BN_EPS)
    scale = 1.0 / np.sqrt(float(d))

    x_pad = np.zeros((n_pad, D), np.float32)
    x_pad[:n] = x
    xT = x_pad.T.astype(NP_BF16)

    # fold the BN gain into the SAGE weights: z*Gx == mean@(Wl*Gx) + h@(Wr*Gx)
    GxF = [al * bn_scale * gamma[i] for i in range(L)]
    Wlg = [Wl[i] * GxF[i][None, :] for i in range(L)]
    Wrg = [Wr[i] * GxF[i][None, :] for i in range(L)]
    weights = [Wq, Wk, Wv, Ws, Wlg[0], Wrg[0], Wlg[1], Wrg[1], Wlg[2], Wrg[2]]
    wpack = np.zeros((P, (10 * DJ + 4) * D), NP_BF16)
    for w, W in enumerate(weights):
        for j in range(DJ):
            wpack[:, (w * DJ + j) * D:(w * DJ + j + 1) * D] = \
                W[j * P:(j + 1) * P, :].astype(NP_BF16)
    # bias rows (read from partition 0 only): bs, Bx0..Bx2
    BxF = [al * (bl[i] * bn_scale * gamma[i] + beta[i]) for i in range(L)]
    for bi, vec in enumerate([bs] + BxF):
        wpack[0, (10 * DJ + bi) * D:(10 * DJ + bi + 1) * D] = vec.astype(NP_BF16)

    Gx = [al * bn_scale * gamma[i] for i in range(L)]
    Bx = [al * (bl[i] * bn_scale * gamma[i] + beta[i]) for i in range(L)]
    vecs = [bk, bv, bs, Gx[0], Bx[0], Gx[1], Bx[1], Gx[2], Bx[2]]
    vpack = np.empty((P, 9 * D + DJ), np.float32)
    for k, v in enumerate(vecs):
        vpack[:, k * D:(k + 1) * D] = np.tile(v[None, :], (P, 1))
    for j in range(DJ):
        vpack[:, 9 * D + j] = bq[j * P:(j + 1) * P]

    # dense mean-normalized adjacency in DoubleRow block layout, per core:
    # a[k, ((c2*nt + t)*2 + i)*P + m] = mult(src=256*c2+128*i+k -> dst) / deg
    NC2 = (n_pad // P) // 2
    NBLK = NC2 * nt
    kk = (src_s % P).astype(np.int64)
    ii = (src_s // P) % 2
    c2 = src_s // (2 * P)
    w_e = np.ones(len(dst_s), np.float32)  # integer counts; exact in fp8

    in_maps = []
    for r in range(NC):
        idx_arr = np.zeros((P, stot * 8), np.int16)
        dst_arr = np.full((P, stot), 128.0, np.float32)
        for tloc in range(nt):
            g = r * nt + tloc
            e0, e1 = starts[g], starts[g + 1]
            cnt = e1 - e0
            st_ = int(s_pos[tloc])
            et_t = st_ * P
            srcs = np.zeros(et_t, np.int64)
            srcs[:cnt] = src_s[e0:e1]
            dl = np.full(et_t, 128, np.int64)
            dl[:cnt] = dst_s[e0:e1] - g * P
            o = int(soff[tloc])
            idx_arr[:, o * 8:(o + st_) * 8] = _wrap_idx(srcs)
            dst_arr[:, o:o + st_] = dl.reshape(st_, P).T

        e0, e1 = starts[r * nt], starts[(r + 1) * nt]
        tl = (dst_s[e0:e1] - r * sh) // P
        mm = (dst_s[e0:e1] - r * sh) % P
        # layout [c2][i][t*P + m]
        flat = (c2[e0:e1] * 2 + ii[e0:e1]) * (nt * P) + tl * P + mm
        a_arr = np.zeros((P, NBLK * 2 * P), np.float32)
        np.add.at(a_arr, (kk[e0:e1], flat), w_e[e0:e1])
        a_arr = a_arr.astype(NP_FP8)
        invdeg_r = invdeg_full[r * sh:(r + 1) * sh].reshape(nt, P).T.copy()

        xt_r = np.empty((P, DJ * sh), NP_BF16)
        for j in range(DJ):
            xt_r[:, j * sh:(j + 1) * sh] = xT[j * P:(j + 1) * P, r * sh:(r + 1) * sh]

        in_maps.append({
            "xt_in": xt_r,
            "wpack_in": wpack,
            "vpack_in": vpack,
            "idx_in": idx_arr,
            "dst_in": dst_arr,
            "invdeg_in": np.ascontiguousarray(invdeg_r),
            "a_in": a_arr,
        })
    return in_maps, (n_pad, sh, nt, s_list, scale, oma)


def kernel(**inputs):
    x = np.asarray(inputs["x"], np.float32)
    edge_index = np.asarray(inputs["edge_index"])
    args = dict(
        Wq=np.asarray(inputs["Wq"], np.float32), bq=np.asarray(inputs["bq"], np.float32),
        Wk=np.asarray(inputs["Wk"], np.float32), bk=np.asarray(inputs["bk"], np.float32),
        Wv=np.asarray(inputs["Wv"], np.float32), bv=np.asarray(inputs["bv"], np.float32),
        Ws=np.asarray(inputs["Ws"], np.float32), bs=np.asarray(inputs["bs"], np.float32),
        Wl=np.asarray(inputs["Wl"], np.float32), bl=np.asarray(inputs["bl"], np.float32),
        Wr=np.asarray(inputs["Wr"], np.float32),
        gamma=np.asarray(inputs["gamma"], np.float32),
        beta=np.asarray(inputs["beta"], np.float32),
        alpha_res=float(np.asarray(inputs["alpha_res"])),
    )
    src = edge_index[0].astype(np.int64)
    dst = edge_index[1].astype(np.int64)

    in_maps, params = _host_prep(x, src, dst, **args)
    t0 = time.time()
    nc = build_nc(*params)
    print(f"[kernel] build+compile {time.time()-t0:.1f}s", flush=True)
    t0 = time.time()
    res = run_bass_kernel_spmd(nc, in_maps, core_ids=list(range(NC)))
    print(f"[kernel] run {time.time()-t0:.1f}s", flush=True)
    out = np.concatenate([res.results[r]["out"] for r in range(NC)], axis=0)
    return out[:x.shape[0]]


# revision 67
# speedup vs baseline: 1.3658x; 1.0231x over previous
"""Trainium2 Bass kernel for nn_MixGNN (TransformerConv + 3x SAGEConv + BN + gated residual).

Strategy (8 NeuronCores, dst-node sharding; pad N 10000 -> 10240, core r owns
1280 dst nodes = 10 tiles of 128):
  - TransformerConv (per-edge gather): dma_gather of source k rows
    (transposed, bf16) + v rows; attention scores as KgT.T @ qT on PE;
    softmax without max-subtraction (logits are O(1)); per-chunk exp-weighted
    one-hot matmuls accumulate value sums + exp sums in PSUM.
  - SAGEConv x3 (dense fp8 DoubleRow — no gathers): host precomputes the
    integer-count adjacency A[src, dst] (exact in fp8e4) in PE DoubleRow
    block layout [c2][i][t*128+m]; each pass computes sumT[d, dst] =
    sum_src H[src, d] * A[src, dst] as 256-src-deep fp8 matmuls at 0.5
    cyc/row, batching 4 dst tiles per matmul (one [128,512] PSUM bank).
    1/deg is applied on the PSUM of sumT@Wl (per-dst-partition ACT scale);
    BN gain is folded into Wl/Wr, biases added via 1-partition matmuls.
    The full H table (fp8, tile-major) lives in SBUF, refreshed per pass
    via AllGather of fp8 shard packs. The 13 MB A table streams into SBUF
    partly during the transformer phase (small early pool) and the rest
    right after the gather pools release.
  - 5 collectives total: AllGather k, v (bf16), h0..h2 (fp8).
Output: fp32 [10000, 256].
"""
import os
import sys
import time

import numpy as np

for _p in ("/opt/trn_rl_repo",):
    if _p not in sys.path:
        sys.path.insert(0, _p)

import ml_dtypes  # noqa: E402
import concourse.bacc as bacc  # noqa: E402
import concourse.mybir as mybir  # noqa: E402
import concourse.tile as tile  # noqa: E402
from concourse.bass_utils import run_bass_kernel_spmd  # noqa: E402

P = 128
D = 256
DJ = D // P           # 2 d-chunks of 128
NC = 8                # cores
L = 3                 # SAGE layers
BN_EPS = 1e-5

F32 = mybir.dt.float32
BF16 = mybir.dt.bfloat16
FP8 = mybir.dt.float8e4
I16 = mybir.dt.int16
NP_FP8 = ml_dtypes.float8_e4m3
NP_BF16 = ml_dtypes.bfloat16

_nc_cache = {}


def _wrap_idx(a):
    """[S*128] int array -> [128, S*8] int16 wrapped gather-index layout."""
    w16 = a.reshape(-1, 16).T.astype(np.int16)   # [16, S*8]
    return np.tile(w16, (8, 1))                  # replicate to 8 Q7 stripes


def build_nc(n_pad, sh, nt, s_list, scale, oma):
    stages = int(os.environ.get("KSTAGES", "5"))
    nocc = os.environ.get("KNOCC") == "1"
    ksm = int(os.environ.get("KSM", "4"))
    kgp = int(os.environ.get("KGP", "2"))
    kpsc = int(os.environ.get("KPSC", "3"))
    kpagg = int(os.environ.get("KPAGG", "2"))
    kpmm = int(os.environ.get("KPMM", "2"))
    kptr = int(os.environ.get("KPTR", "1"))
    khalft = int(os.environ.get("KHALFT", "2"))  # transformer v-gather splits
    kapc = int(os.environ.get("KAPC", "21"))     # A-table late-load pieces
    # A c2-blocks loaded early; up to two are issued per transformer tile,
    # so at most 2*nt blocks can be trickled in.
    kae = min(int(os.environ.get("KAEARLY", "19")), 2 * nt)
    key = (n_pad, sh, nt, s_list, round(scale, 9), round(oma, 9), stages,
           nocc, ksm, kgp, kpsc, kpagg, kpmm, kptr, khalft, kapc, kae,
           os.environ.get("KKGT"), os.environ.get("KVG"))
    if key in _nc_cache:
        return _nc_cache[key]

    S = max(s_list)
    soff = [0]
    for st_ in s_list:
        soff.append(soff[-1] + st_)
    stot = soff[-1]

    NCH = n_pad // P          # 80 source chunks of 128
    NC2 = NCH // 2            # 40 double-chunks of 256
    NBLK = NC2 * nt           # 400 A-blocks per core

    ndev = 1 if nocc else NC
    nc = bacc.Bacc("TRN2", target_bir_lowering=False, debug=False, num_devices=ndev)

    xt_in = nc.dram_tensor("xt_in", [P, DJ * sh], BF16, kind="ExternalInput")
    wpack_in = nc.dram_tensor("wpack_in", [P, (10 * DJ + 4) * D], BF16,
                              kind="ExternalInput")
    vpack_in = nc.dram_tensor("vpack_in", [P, 9 * D + DJ], F32, kind="ExternalInput")
    idx_in = nc.dram_tensor("idx_in", [P, stot * 8], I16, kind="ExternalInput")
    dst_in = nc.dram_tensor("dst_in", [P, stot], F32, kind="ExternalInput")
    invdeg_in = nc.dram_tensor("invdeg_in", [P, nt], F32, kind="ExternalInput")
    a_in = nc.dram_tensor("a_in", [P, NBLK * 2 * P], FP8, kind="ExternalInput")
    out_dram = nc.dram_tensor("out", [sh, D], F32, kind="ExternalOutput")

    WQ, WK, WV, WS = 0, 1, 2, 3
    WL = [4, 6, 8]
    WR = [5, 7, 9]
    WBIAS = 10 * DJ          # 4 bias rows (bs, Bx0..2) in wpack row 0
    VBK, VBV, VBS = 0, 1, 2

    with tile.TileContext(nc) as tc:
        with (
            tc.tile_pool(name="cst", bufs=1) as cst,
            tc.tile_pool(name="sb", bufs=1) as sb,
            tc.tile_pool(name="sm", bufs=ksm) as smp,
            tc.tile_pool(name="ps", bufs=2, space="PSUM") as ps,
            tc.tile_pool(name="dr", bufs=1, space="DRAM") as dr,
        ):
            # ---------------- constants / inputs to SBUF ----------------
            # load order tuned so the k/v matmuls (xt + WQ..WS + biases)
            # unblock as early as possible
            xt = cst.tile([P, DJ * sh], BF16)
            for _xi in range(2):
                _c0 = _xi * (DJ * sh // 2)
                _c1 = (_xi + 1) * (DJ * sh // 2)
                nc.sync.dma_start(out=xt[:, _c0:_c1], in_=xt_in[:, _c0:_c1])
            wp = cst.tile([P, (10 * DJ + 4) * D], BF16)
            _wsplit = 4 * DJ * D
            nc.sync.dma_start(out=wp[:, :_wsplit], in_=wpack_in[:, :_wsplit])
            vp = cst.tile([P, 9 * D + DJ], F32)
            _vsplit = 3 * D
            nc.sync.dma_start(out=vp[:, :_vsplit], in_=vpack_in[:, :_vsplit])
            nc.sync.dma_start(out=wp[:, _wsplit:], in_=wpack_in[:, _wsplit:])
            nc.sync.dma_start(out=vp[:, _vsplit:], in_=vpack_in[:, _vsplit:])
            invd = cst.tile([P, nt], F32)
            nc.sync.dma_start(out=invd[:], in_=invdeg_in[:])

            iota_i = cst.tile([P, P], mybir.dt.int32)
            nc.gpsimd.iota(iota_i[:], pattern=[[1, P]], base=0, channel_multiplier=0)
            ones_v = cst.tile([P, 1], BF16)
            nc.vector.memset(ones_v[:], 1.0)
            iota_part = cst.tile([P, 1], mybir.dt.int32)
            nc.gpsimd.iota(iota_part[:], pattern=[[1, 1]], base=0, channel_multiplier=1)
            iota_part_f = cst.tile([P, 1], F32)
            nc.vector.tensor_copy(out=iota_part_f[:], in_=iota_part[:])
            iota_f = cst.tile([P, P], F32)
            nc.vector.tensor_copy(out=iota_f[:], in_=iota_i[:])
            ident = cst.tile([P, P], F32)
            nc.vector.tensor_scalar(
                out=ident[:], in0=iota_f[:], scalar1=iota_part_f[:, :1], scalar2=None,
                op0=mybir.AluOpType.is_equal,
            )
            ident_b = cst.tile([P, P], BF16)
            nc.vector.tensor_copy(out=ident_b[:], in_=ident[:])
            iota_b = cst.tile([P, P], BF16)
            nc.vector.tensor_copy(out=iota_b[:], in_=iota_f[:])

            def wslice(w, j):
                return wp[:, (w * DJ + j) * D:(w * DJ + j + 1) * D]

            def vslice(k):
                return vp[:, k * D:(k + 1) * D]

            def xtile(j, t):
                return xt[:, j * sh + t * P: j * sh + (t + 1) * P]

            # ---------------- long-lived SBUF state ----------------
            qT = [sb.tile([P, sh], BF16, name=f"qT_{j}") for j in range(DJ)]
            h_cur = sb.tile([P, nt * D], BF16)
            h_nxt = sb.tile([P, nt * D], BF16)
            hT_cur = sb.tile([P, DJ * sh], BF16)
            hT_nxt = sb.tile([P, DJ * sh], BF16)
            h_sb = sb.tile([P, NCH, D], FP8)     # full H table, tile-major
            meanT_all = sb.tile([P, nt, D], BF16)  # paggT staging per pass

            ones_row = cst.tile([1, P], BF16)
            nc.vector.memset(ones_row[:], 1.0)

            # ---------------- DRAM tables ----------------
            # k and v packed per node into one row -> single AllGather, and
            # the gathers slice columns via elem_step=2D.
            kv_ag_in = dr.tile([sh, 2 * D], BF16)
            kv_full = dr.tile([n_pad, 2 * D], BF16, addr_space="Shared")
            hag_in = [dr.tile([P, nt * D], FP8, name=f"hag_in_{i}") for i in range(L)]
            h_full = [dr.tile([NC * P, nt * D], FP8, name=f"h_full_{i}",
                              addr_space="Shared") for i in range(L)]

            def allgather(in_t, out_t):
                if nocc:
                    nc.sync.dma_start(out=out_t[:in_t.shape[0]], in_=in_t[:])
                else:
                    nc.gpsimd.collective_compute(
                        "AllGather", mybir.AluOpType.bypass,
                        replica_groups=[list(range(NC))],
                        ins=[in_t[:]], outs=[out_t[:]],
                    )

            # ---------------- stage 0: packed k|v shard table + AG, then qT --
            for t in range(nt):
                kv_sb = smp.tile([P, 2 * D], BF16, name="kv_sb")
                pk = ps.tile([P, D], F32, name="pk", tag="pmm", bufs=kpmm)
                for ji in range(DJ):
                    nc.tensor.matmul(pk[:], lhsT=xtile(ji, t), rhs=wslice(WK, ji),
                                     start=(ji == 0), stop=(ji == DJ - 1))
                nc.vector.tensor_tensor(out=kv_sb[:, :D], in0=pk[:],
                                        in1=vslice(VBK), op=mybir.AluOpType.add)
                pv = ps.tile([P, D], F32, name="pv", tag="pmm", bufs=kpmm)
                for ji in range(DJ):
                    nc.tensor.matmul(pv[:], lhsT=xtile(ji, t), rhs=wslice(WV, ji),
                                     start=(ji == 0), stop=(ji == DJ - 1))
                nc.vector.tensor_tensor(out=kv_sb[:, D:], in0=pv[:],
                                        in1=vslice(VBV), op=mybir.AluOpType.add)
                nc.sync.dma_start(out=kv_ag_in[t * P:(t + 1) * P, :], in_=kv_sb[:])
            allgather(kv_ag_in, kv_full)

            for j in range(DJ):
                n0 = 0
                while n0 < sh:
                    nn = min(512, sh - n0)
                    pq = ps.tile([P, 512], F32, name="pq", tag="pmm", bufs=kpmm)
                    for ji in range(DJ):
                        nc.tensor.matmul(
                            pq[:, :nn],
                            lhsT=wslice(WQ, ji)[:, j * P:(j + 1) * P],
                            rhs=xt[:, ji * sh + n0: ji * sh + n0 + nn],
                            start=(ji == 0), stop=(ji == DJ - 1),
                        )
                    nc.vector.tensor_scalar(
                        out=qT[j][:, n0:n0 + nn], in0=pq[:, :nn],
                        scalar1=vp[:, 9 * D + j: 9 * D + j + 1], scalar2=None,
                        op0=mybir.AluOpType.add,
                    )
                    n0 += nn

            # ---------------- transformer pass (gather-based) ----------------
            def transformer_pass(gp, pst, h_out, hT_out, tile_hook):
                dstc = gp.tile([P, stot], F32, name="dstc", bufs=1)
                nc.sync.dma_start(out=dstc[:], in_=dst_in[:])
                idx_sb = gp.tile([P, stot * 8], I16, name="idx_sb", bufs=1)
                nc.sync.dma_start(out=idx_sb[:], in_=idx_in[:])
                for t in range(nt):
                    st = s_list[t]
                    nh = (st + 1) // 2
                    halves = [(0, nh), (nh, st)]
                    kgs, vgs = [], []
                    for (ha, hb) in halves:
                        hc = hb - ha
                        nn_i = hc * P
                        idx_t = idx_sb[:, (soff[t] + ha) * 8:(soff[t] + hb) * 8]
                        kg = gp.tile([P, DJ, nn_i], BF16, name="kgt", tag="kgt",
                                     bufs=int(os.environ.get("KKGT", "3")))
                        nc.gpsimd.dma_gather(
                            out_ap=kg[:], in_ap=kv_full[:, :D], idxs_ap=idx_t,
                            num_idxs=nn_i, num_idxs_reg=nn_i, elem_size=D,
                            elem_step=2 * D, transpose=True, single_packet=False)
                        vg = gp.tile([P, hc, D], BF16, name="vg", tag="vg",
                                     bufs=int(os.environ.get("KVG", "3")))
                        nc.gpsimd.dma_gather(
                            out_ap=vg[:], in_ap=kv_full[:, D:], idxs_ap=idx_t,
                            num_idxs=nn_i, num_idxs_reg=nn_i, elem_size=D,
                            elem_step=2 * D, single_packet=False)
                        kgs.append(kg)
                        vgs.append(vg)

                    pagg = pst.tile([P, D + 1], F32, name="pagg", tag="pagg",
                                    bufs=kpagg)
                    for c in range(st):
                        hi = 0 if c < nh else 1
                        cl = c - halves[hi][0]
                        dcol = dstc[:, soff[t] + c: soff[t] + c + 1]
                        psc = pst.tile([P, P], F32, name="psc", tag="psc", bufs=kpsc)
                        for j in range(DJ):
                            nc.tensor.matmul(
                                psc[:],
                                lhsT=kgs[hi][:, j, cl * P:(cl + 1) * P],
                                rhs=qT[j][:, t * P:(t + 1) * P],
                                start=(j == 0), stop=(j == DJ - 1))
                        exps = smp.tile([P, P], BF16, name="exps")
                        nc.scalar.activation(exps[:], psc[:],
                                             mybir.ActivationFunctionType.Exp,
                                             scale=scale)
                        w_b = smp.tile([P, P], BF16, name="w_b", tag="w_b")
                        nc.vector.scalar_tensor_tensor(
                            out=w_b[:], in0=iota_b[:], scalar=dcol, in1=exps[:],
                            op0=mybir.AluOpType.is_equal,
                            op1=mybir.AluOpType.mult)
                        nc.tensor.matmul(pagg[:, :D], lhsT=w_b[:],
                                         rhs=vgs[hi][:, cl, :],
                                         start=(c == 0), stop=(c == st - 1))
                        nc.tensor.matmul(pagg[:, D:D + 1], lhsT=w_b[:],
                                         rhs=ones_v[:],
                                         start=False, stop=(c == st - 1))

                    smax = smp.tile([P, 1], F32, name="smax")
                    nc.vector.tensor_scalar(
                        out=smax[:], in0=pagg[:, D:D + 1], scalar1=1e-30,
                        scalar2=None, op0=mybir.AluOpType.max)
                    rs = smp.tile([P, 1], F32, name="rs")
                    nc.vector.reciprocal(rs[:], smax[:])
                    pskip = ps.tile([P, D], F32, name="pskip", tag="pmm", bufs=kpmm)
                    for ji in range(DJ):
                        nc.tensor.matmul(pskip[:], lhsT=xtile(ji, t),
                                         rhs=wslice(WS, ji),
                                         start=(ji == 0), stop=False)
                    nc.tensor.matmul(pskip[:], lhsT=ones_row[:],
                                     rhs=wp[:1, WBIAS * D:(WBIAS + 1) * D],
                                     start=False, stop=True)
                    t1 = smp.tile([P, D], F32, name="t1", tag="t1")
                    nc.scalar.activation(t1[:], pagg[:, :D],
                                         mybir.ActivationFunctionType.Copy,
                                         scale=rs[:, :1])
                    t2 = smp.tile([P, D], F32, name="t2", tag="t2")
                    nc.vector.tensor_tensor(out=t2[:], in0=t1[:], in1=pskip[:],
                                            op=mybir.AluOpType.add)
                    nc.scalar.activation(h_out[:, t * D:(t + 1) * D], t2[:],
                                         mybir.ActivationFunctionType.Relu)
                    epilogue_store(0, t, h_out, hT_out)
                    tile_hook(t)
                allgather(hag_in[0], h_full[0])

            def epilogue_store(li, t, h_out, hT_out):
                """Write fp8 pack row + transposed shard for layer output li.

                Copies go to DVE during the transformer (ACT is busy with
                exp) and to ACT during SAGE passes (DVE runs the epilogue).
                """
                cpy = nc.vector.tensor_copy if li == 0 else (
                    lambda out, in_: nc.scalar.copy(out=out, in_=in_))
                hstage = smp.tile([P, D], FP8, name="hstage")
                cpy(out=hstage[:], in_=h_out[:, t * D:(t + 1) * D])
                nc.sync.dma_start(out=hag_in[li][:, t * D:(t + 1) * D],
                                  in_=hstage[:])
                for j in range(DJ):
                    ptr2 = ps.tile([P, P], BF16, name="ptr2", tag="ptr", bufs=kptr)
                    nc.tensor.transpose(
                        out=ptr2[:],
                        in_=h_out[:, t * D + j * P: t * D + (j + 1) * P],
                        identity=ident_b[:])
                    cpy(out=hT_out[:, j * sh + t * P: j * sh + (t + 1) * P],
                        in_=ptr2[:])

            # ---------------- SAGE pass (dense fp8 DoubleRow) ----------------
            # dst tiles processed in groups of 4; one [P, 512] PSUM bank per
            # (group, dh), double-buffered so group g+1's adjacency sweep
            # overlaps group g's epilogue.
            def sage_pass(layer, pd, a_rhs, h_prev, hT_prev, h_out, hT_out):
                li = layer + 1
                # load H table (tile-major fp8) from the AllGathered pack
                for r in range(NC):
                    nc.sync.dma_start(
                        out=h_sb[:, r * nt:(r + 1) * nt, :],
                        in_=h_full[layer][r * P:(r + 1) * P, :])
                groups = [(g * 4, min(nt, (g + 1) * 4)) for g in range((nt + 3) // 4)]

                def sage_epilogue(t):
                    pzm = pd.tile([P, D], F32, name="pzm", tag="pzm", bufs=1)
                    for j in range(DJ):
                        nc.tensor.matmul(pzm[:],
                                         lhsT=meanT_all[:, t, j * P:(j + 1) * P],
                                         rhs=wslice(WL[layer], j),
                                         start=(j == 0), stop=(j == DJ - 1))
                    pz = ps.tile([P, D], F32, name="pz", tag="pmm", bufs=kpmm)
                    for j in range(DJ):
                        nc.tensor.matmul(
                            pz[:],
                            lhsT=hT_prev[:, j * sh + t * P: j * sh + (t + 1) * P],
                            rhs=wslice(WR[layer], j),
                            start=(j == 0), stop=False)
                    nc.tensor.matmul(
                        pz[:], lhsT=ones_row[:],
                        rhs=wp[:1, (WBIAS + 1 + layer) * D:(WBIAS + 2 + layer) * D],
                        start=False, stop=True)
                    t0 = smp.tile([P, D], F32, name="t0s", tag="t0")
                    nc.scalar.activation(t0[:], pzm[:],
                                         mybir.ActivationFunctionType.Copy,
                                         scale=invd[:, t:t + 1])
                    t1 = smp.tile([P, D], F32, name="t1s", tag="t2")
                    nc.vector.tensor_tensor(out=t1[:], in0=t0[:], in1=pz[:],
                                            op=mybir.AluOpType.add)
                    t3 = smp.tile([P, D], F32, name="t3s", tag="t4")
                    nc.vector.scalar_tensor_tensor(
                        out=t3[:], in0=h_prev[:, t * D:(t + 1) * D], scalar=oma,
                        in1=t1[:], op0=mybir.AluOpType.mult,
                        op1=mybir.AluOpType.add)
                    if layer < L - 1:
                        nc.scalar.activation(h_out[:, t * D:(t + 1) * D], t3[:],
                                             mybir.ActivationFunctionType.Relu)
                        epilogue_store(li, t, h_out, hT_out)
                    else:
                        hof = smp.tile([P, D], F32, name="hof", tag="t1")
                        nc.scalar.activation(hof[:], t3[:],
                                             mybir.ActivationFunctionType.Relu)
                        nc.sync.dma_start(out=out_dram[t * P:(t + 1) * P, :],
                                          in_=hof[:])

                for gi, (ta, tb) in enumerate(groups):
                    tw = (tb - ta) * P
                    pb = [pd.tile([P, 4 * P], F32, name=f"pdb_{gi % 2}_{dh}",
                                  tag=f"pdb_{gi % 2}_{dh}", bufs=1)
                          for dh in range(DJ)]
                    for dh in range(DJ):
                        for c2 in range(NC2):
                            nc.tensor.matmul(
                                pb[dh][:, :tw],
                                lhsT=h_sb[:, 2 * c2:2 * c2 + 2, dh * P:(dh + 1) * P],
                                rhs=a_rhs(c2, ta, tb),
                                start=(c2 == 0), stop=(c2 == NC2 - 1),
                                perf_mode=mybir.MatmulPerfMode.DoubleRow)
                        nc.vector.tensor_copy(
                            out=meanT_all[:, ta:tb, dh * P:(dh + 1) * P],
                            in_=pb[dh][:, :tw])
                    for t in range(ta, tb):
                        sage_epilogue(t)
                if layer < L - 1:
                    allgather(hag_in[li], h_full[li])

            if stages <= 1:
                tmpo = smp.tile([P, D], F32, name="tmpo")
                for t in range(nt):
                    nc.vector.tensor_copy(out=tmpo[:], in_=xt[:, :D])
                    nc.sync.dma_start(out=out_dram[t * P:(t + 1) * P, :],
                                      in_=tmpo[:])
            else:
                with tc.tile_pool(name="ae", bufs=1) as ape:
                    a_early = None
                    if stages >= 3 and kae > 0:
                        a_early = ape.tile([P, kae, 2, nt * P], FP8)

                    def tile_hook(t):
                        # trickle early A-table c2-blocks through transformer
                        if a_early is None:
                            return
                        for b in (2 * t, 2 * t + 1):
                            if b < kae:
                                nc.scalar.dma_start(
                                    out=a_early[:, b, :, :],
                                    in_=a_in[:, b * 2 * nt * P:
                                             (b + 1) * 2 * nt * P])

                    with (
                        tc.tile_pool(name="g", bufs=kgp) as gp,
                        tc.tile_pool(name="pst", bufs=1, space="PSUM") as pst,
                    ):
                        transformer_pass(gp, pst, h_cur, hT_cur, tile_hook)
                    if stages >= 3:
                        with (
                            tc.tile_pool(name="ap", bufs=1) as app,
                            tc.tile_pool(name="pd", bufs=1, space="PSUM") as pd,
                        ):
                            nlate = NC2 - kae
                            a_late = app.tile([P, max(1, nlate), 2, nt * P], FP8)
                            cw = (nlate + kapc - 1) // kapc
                            for p_ in range(kapc):
                                c0 = min(nlate, p_ * cw)
                                c1 = min(nlate, (p_ + 1) * cw)
                                if c0 >= c1:
                                    continue
                                # ACT-issued so H loads (SP queue) interleave
                                nc.scalar.dma_start(
                                    out=a_late[:, c0:c1, :, :],
                                    in_=a_in[:, (kae + c0) * 2 * nt * P:
                                             (kae + c1) * 2 * nt * P])

                            def a_rhs(c2, ta, tb):
                                if c2 < kae:
                                    return a_early[:, c2, :, ta * P:tb * P]
                                return a_late[:, c2 - kae, :, ta * P:tb * P]

                            bufs = [(h_cur, hT_cur), (h_nxt, hT_nxt)]
                            for i in range(L):
                                h_prev, hT_prev = bufs[i % 2]
                                h_out, hT_out = bufs[(i + 1) % 2]
                                sage_pass(i, pd, a_rhs, h_prev, hT_prev,
                                          h_out, hT_out)
                    else:
                        for t in range(nt):
                            hof = smp.tile([P, D], F32, name="hof", tag="t1")
                            nc.vector.tensor_copy(out=hof[:],
                                                  in_=h_cur[:, t * D:(t + 1) * D])
                            nc.sync.dma_start(out=out_dram[t * P:(t + 1) * P, :],
                                              in_=hof[:])

    nc.compile()
    _nc_cache[key] = nc
    return nc


def _host_prep(x, src, dst, Wq, bq, Wk, bk, Wv, bv, Ws, bs, Wl, bl, Wr,
               gamma, beta, alpha_res):
    n, d = x.shape
    n_pad = ((n + NC * P - 1) // (NC * P)) * (NC * P)
    sh = n_pad // NC
    nt = sh // P
    n_tiles = n_pad // P

    order = np.argsort(dst, kind="stable")
    src_s, dst_s = src[order], dst[order]
    tile_of = dst_s // P
    counts = np.bincount(tile_of, minlength=n_tiles)
    starts = np.concatenate([[0], np.cumsum(counts)])
    s_all = np.maximum(1, (counts + P - 1) // P).astype(np.int64)
    s_pos = s_all.reshape(NC, nt).max(axis=0)
    s_list = tuple(int(v) for v in s_pos)
    soff = np.concatenate([[0], np.cumsum(s_pos)])
    stot = int(soff[-1])

    deg = np.bincount(dst, minlength=n_pad).astype(np.float32)
    invdeg_full = 1.0 / np.maximum(deg, 1.0)

    al = 1.0 / (1.0 + np.exp(-alpha_res))
    oma = float(1.0 - al)
    bn_scale = 1.0 / np.sqrt(1.0 + BN_EPS)
    scale = 1.0 / np.sqrt(float(d))

    x_pad = np.zeros((n_pad, D), np.float32)
    x_pad[:n] = x
    xT = x_pad.T.astype(NP_BF16)

    # fold the BN gain into the SAGE weights: z*Gx == mean@(Wl*Gx) + h@(Wr*Gx)
    GxF = [al * bn_scale * gamma[i] for i in range(L)]
    Wlg = [Wl[i] * GxF[i][None, :] for i in range(L)]
    Wrg = [Wr[i] * GxF[i][None, :] for i in range(L)]
    weights = [Wq, Wk, Wv, Ws, Wlg[0], Wrg[0], Wlg[1], Wrg[1], Wlg[2], Wrg[2]]
    wpack = np.zeros((P, (10 * DJ + 4) * D), NP_BF16)
    for w, W in enumerate(weights):
        for j in range(DJ):
            wpack[:, (w * DJ + j) * D:(w * DJ + j + 1) * D] = \
                W[j * P:(j + 1) * P, :].astype(NP_BF16)
    # bias rows (read from partition 0 only): bs, Bx0..Bx2
    BxF = [al * (bl[i] * bn_scale * gamma[i] + beta[i]) for i in range(L)]
    for bi, vec in enumerate([bs] + BxF):
        wpack[0, (10 * DJ + bi) * D:(10 * DJ + bi + 1) * D] = vec.astype(NP_BF16)

    Gx = [al * bn_scale * gamma[i] for i in range(L)]
    Bx = [al * (bl[i] * bn_scale * gamma[i] + beta[i]) for i in range(L)]
    vecs = [bk, bv, bs, Gx[0], Bx[0], Gx[1], Bx[1], Gx[2], Bx[2]]
    vpack = np.empty((P, 9 * D + DJ), np.float32)
    for k, v in enumerate(vecs):
        vpack[:, k * D:(k + 1) * D] = np.tile(v[None, :], (P, 1))
    for j in range(DJ):
        vpack[:, 9 * D + j] = bq[j * P:(j + 1) * P]

    # dense mean-normalized adjacency in DoubleRow block layout, per core:
    # a[k, ((c2*nt + t)*2 + i)*P + m] = mult(src=256*c2+128*i+k -> dst) / deg
    NC2 = (n_pad // P) // 2
    NBLK = NC2 * nt
    kk = (src_s % P).astype(np.int64)
    ii = (src_s // P) % 2
    c2 = src_s // (2 * P)
    w_e = np.ones(len(dst_s), np.float32)  # integer counts; exact in fp8

    in_maps = []
    for r in range(NC):
        idx_arr = np.zeros((P, stot * 8), np.int16)
        dst_arr = np.full((P, stot), 128.0, np.float32)
        for tloc in range(nt):
            g = r * nt + tloc
            e0, e1 = starts[g], starts[g + 1]
            cnt = e1 - e0
            st_ = int(s_pos[tloc])
            et_t = st_ * P
            srcs = np.zeros(et_t, np.int64)
            srcs[:cnt] = src_s[e0:e1]
            dl = np.full(et_t, 128, np.int64)
            dl[:cnt] = dst_s[e0:e1] - g * P
            o = int(soff[tloc])
            idx_arr[:, o * 8:(o + st_) * 8] = _wrap_idx(srcs)
            dst_arr[:, o:o + st_] = dl.reshape(st_, P).T

        e0, e1 = starts[r * nt], starts[(r + 1) * nt]
        tl = (dst_s[e0:e1] - r * sh) // P
        mm = (dst_s[e0:e1] - r * sh) % P
        # layout [c2][i][t*P + m]
        flat = (c2[e0:e1] * 2 + ii[e0:e1]) * (nt * P) + tl * P + mm
        a_arr = np.zeros((P, NBLK * 2 * P), np.float32)
        np.add.at(a_arr, (kk[e0:e1], flat), w_e[e0:e1])
        a_arr = a_arr.astype(NP_FP8)
        invdeg_r = invdeg_full[r * sh:(r + 1) * sh].reshape(nt, P).T.copy()

        xt_r = np.empty((P, DJ * sh), NP_BF16)
        for j in range(DJ):
            xt_r[:, j * sh:(j + 1) * sh] = xT[j * P:(j + 1) * P, r * sh:(r + 1) * sh]

        in_maps.append({
            "xt_in": xt_r,
            "wpack_in": wpack,
            "vpack_in": vpack,
            "idx_in": idx_arr,
            "dst_in": dst_arr,
            "invdeg_in": np.ascontiguousarray(invdeg_r),
            "a_in": a_arr,
        })
    return in_maps, (n_pad, sh, nt, s_list, scale, oma)


def kernel(**inputs):
    x = np.asarray(inputs["x"], np.float32)
    edge_index = np.asarray(inputs["edge_index"])
    args = dict(
        Wq=np.asarray(inputs["Wq"], np.float32), bq=np.asarray(inputs["bq"], np.float32),
        Wk=np.asarray(inputs["Wk"], np.float32), bk=np.asarray(inputs["bk"], np.float32),
        Wv=np.asarray(inputs["Wv"], np.float32), bv=np.asarray(inputs["bv"], np.float32),
        Ws=np.asarray(inputs["Ws"], np.float32), bs=np.asarray(inputs["bs"], np.float32),
        Wl=np.asarray(inputs["Wl"], np.float32), bl=np.asarray(inputs["bl"], np.float32),
        Wr=np.asarray(inputs["Wr"], np.float32),
        gamma=np.asarray(inputs["gamma"], np.float32),
        beta=np.asarray(inputs["beta"], np.float32),
        alpha_res=float(np.asarray(inputs["alpha_res"])),
    )
    src = edge_index[0].astype(np.int64)
    dst = edge_index[1].astype(np.int64)

    in_maps, params = _host_prep(x, src, dst, **args)
    t0 = time.time()
    nc = build_nc(*params)
    print(f"[kernel] build+compile {time.time()-t0:.1f}s", flush=True)
    t0 = time.time()
    res = run_bass_kernel_spmd(nc, in_maps, core_ids=list(range(NC)))
    print(f"[kernel] run {time.time()-t0:.1f}s", flush=True)
    out = np.concatenate([res.results[r]["out"] for r in range(NC)], axis=0)
    return out[:x.shape[0]]
